# revision 1
# baseline (speedup 1.0000x reference)
"""Tensor-parallel decoder layer on 8 TRN2 NeuronCores.

Sharding:
  - Attention: 16 heads -> 2 per core. Per-core partial attn_out is
    ReduceScattered (fp16) so core c owns rows [256c, 256c+256).
  - Global LayerNorm (scalar mean/var over the whole [S,E] tensor):
    per-core partial (sum, sumsq) AllReduced as a tiny fp32 tensor.
  - FFN: hidden dim 8192 -> 1024 per core; partial [S,E] output
    ReduceScattered per 512-column chunk (fp16), overlapping FFN2.
  - h is AllGathered transposed (bf16) since every matmul contracting
    over E needs h^T as the moving operand.

Matmul layout notes (PE computes out = lhsT.T @ rhs, contraction on the
partition dim):
  - x^T resident in SBUF (bf16) feeds Q/K/V projections.
  - scores are built transposed: S^T[t,s] tiles, so exp(S^T) tiles feed
    attn@v directly as lhsT with no transposes; softmax normalization is
    deferred: rowsum via a ones-column matmul, applied as a per-partition
    scale on the PSUM->SBUF copy of attn@v output.
"""

import math
import sys

sys.path.insert(0, "/opt/trn_rl_repo")

import numpy as np
import ml_dtypes

_bf16 = ml_dtypes.bfloat16

import concourse.bass as bass
import concourse.mybir as mybir
import concourse.tile as tile
from concourse import bacc
from concourse.bass_utils import run_bass_kernel_spmd

S, E, H, KD, FF = 2048, 2048, 16, 128, 8192
EPS = 1e-5
NCORES = 8
HPC = H // NCORES          # heads per core = 2
FSH = FF // NCORES         # ffn hidden shard = 1024
RROWS = S // NCORES        # row shard = 256
NTOT = float(S * E)
ISCALE = 1.0 / math.sqrt(KD)

F32 = mybir.dt.float32
BF16 = mybir.dt.bfloat16
F16 = mybir.dt.float16
AF = mybir.ActivationFunctionType
AL = mybir.AluOpType
AX = mybir.AxisListType

# packed triangular offsets for eT tiles: tile(tc, sb) at TRI[sb] + tc
TRI = [0, 4, 12, 24]
NTRI = 40


def _build():
    nc = bacc.Bacc(
        "TRN2",
        target_bir_lowering=False,
        debug=False,
        enable_asserts=True,
        num_devices=NCORES,
    )

    # ---- external I/O (per-core shards prepared on the host) ----
    xtb_d = nc.dram_tensor("xtb", [128, 16, S], BF16, kind="ExternalInput")
    wq_d = nc.dram_tensor("wqt", [HPC, 128, 16, KD], F32, kind="ExternalInput")
    wk_d = nc.dram_tensor("wkt", [HPC, 128, 16, KD], F32, kind="ExternalInput")
    wv_d = nc.dram_tensor("wvt", [HPC, 128, 16, E], F32, kind="ExternalInput")
    w1_d = nc.dram_tensor("w1t", [128, 8, 2048], F32, kind="ExternalInput")
    w2_d = nc.dram_tensor("w2t", [128, 8, 4, 512], F32, kind="ExternalInput")
    bq_d = nc.dram_tensor("bqs", [128, HPC], F32, kind="ExternalInput")
    bk_d = nc.dram_tensor("bks", [128, HPC], F32, kind="ExternalInput")
    b1_d = nc.dram_tensor("b1s", [128, 8], F32, kind="ExternalInput")
    yb1_d = nc.dram_tensor("yb1", [128, E], F32, kind="ExternalInput")
    yb2_d = nc.dram_tensor("yb2", [128, E], F32, kind="ExternalInput")
    xr_d = nc.dram_tensor("xr", [RROWS, E], F32, kind="ExternalInput")
    lng_d = nc.dram_tensor("lngr", [RROWS, E], BF16, kind="ExternalInput")
    lnb_d = nc.dram_tensor("lnbr", [RROWS, E], BF16, kind="ExternalInput")
    mask_d = nc.dram_tensor("mask", [4, 128, 512], F32, kind="ExternalInput")
    id_d = nc.dram_tensor("ident", [128, 128], F32, kind="ExternalInput")
    ones_d = nc.dram_tensor("ones", [128, 8], F32, kind="ExternalInput")
    onesr_d = nc.dram_tensor("onesr", [1, 128], F32, kind="ExternalInput")
    out_d = nc.dram_tensor("out", [RROWS, E], F32, kind="ExternalOutput")

    RG = [list(range(NCORES))]

    with tile.TileContext(nc) as tc:
        with (
            tc.tile_pool(name="persist", bufs=1) as pp,
            tc.tile_pool(name="dram", bufs=1, space="DRAM") as dp,
            tc.tile_pool(name="ps512", bufs=4, space="PSUM") as ps512,
            tc.tile_pool(name="psT", bufs=2, space="PSUM") as psT,
            tc.tile_pool(name="psR", bufs=2, space="PSUM") as psR,
        ):
            # ---- collective bounce buffers (internal DRAM) ----
            att_in = [
                [
                    dp.tile([S, FSH], F16, name=f"att_in_{h}_{fh}", tag=f"ati{h}{fh}")
                    for fh in range(2)
                ]
                for h in range(HPC)
            ]
            att_out = [
                [
                    dp.tile(
                        [RROWS, FSH],
                        F16,
                        name=f"att_out_{h}_{fh}",
                        tag=f"ato{h}{fh}",
                    )
                    for fh in range(2)
                ]
                for h in range(HPC)
            ]
            st1_in = dp.tile([1, 8], F32, name="st1_in", tag="st1i")
            st1_out = dp.tile([1, 8], F32, name="st1_out", tag="st1o", addr_space="Shared")
            st2_in = dp.tile([1, 8], F32, name="st2_in", tag="st2i")
            st2_out = dp.tile([1, 8], F32, name="st2_out", tag="st2o", addr_space="Shared")
            ag_in = [
                dp.tile([RROWS, E // 2], BF16, name=f"ag_in{j}", tag=f"agi{j}")
                for j in range(2)
            ]
            ag_out = [
                dp.tile([S, E // 2], BF16, name=f"ag_out{j}", tag=f"ago{j}", addr_space="Shared")
                for j in range(2)
            ]
            ffn_in = [
                dp.tile([S, 512], F16, name=f"ffn_in_{eb}", tag=f"ffi{eb}")
                for eb in range(4)
            ]
            ffn_out = [
                dp.tile([RROWS, 512], F16, name=f"ffn_out_{eb}", tag=f"ffo{eb}")
                for eb in range(4)
            ]

            # ---- persistent small tiles ----
            ident = pp.tile([128, 128], F32, name="ident")
            nc.sync.dma_start(ident[:], id_d[:])
            onesc = pp.tile([128, 8], F32, name="onesc")
            nc.sync.dma_start(onesc[:], ones_d[:])
            onesr = pp.tile([1, 128], F32, name="onesr")
            nc.sync.dma_start(onesr[:], onesr_d[:])
            ones_bf = pp.tile([128, 1], BF16, name="ones_bf")
            nc.vector.tensor_copy(out=ones_bf[:], in_=onesc[:, 0:1])
            bq_sb = pp.tile([128, HPC], F32, name="bq_sb")
            nc.sync.dma_start(bq_sb[:], bq_d[:])
            bk_sb = pp.tile([128, HPC], F32, name="bk_sb")
            nc.sync.dma_start(bk_sb[:], bk_d[:])
            b1_sb = pp.tile([128, 8], F32, name="b1_sb")
            nc.sync.dma_start(b1_sb[:], b1_d[:])
            qkT = pp.tile([128, 2, HPC, S], BF16, name="qkT")  # [d, q/k, head, s]
            recips = pp.tile([128, HPC, 16], F32, name="recips")

            # =========== phase 0 + attention ===========
            with tc.tile_pool(name="attn", bufs=1) as ap_:
                xT = ap_.tile([128, 16, S], BF16, name="xT")  # x^T, e on partitions
                maskb = ap_.tile([128, 4, 512], BF16, name="maskb")
                with tc.tile_pool(name="prep", bufs=3) as prep, nc.named_scope("prep"):
                    # HAM warmup: ~8us of back-to-back matmuls to unthrottle PE
                    wtile = prep.tile([128, 512], BF16, name="wtile", tag="wtile", bufs=1)
                    nc.vector.memset(wtile[:], 0.0)
                    for _w in range(24):
                        pw = ps512.tile([128, 512], F32, name="pw", tag="p512")
                        nc.tensor.matmul(pw[:], wtile[:, :128], wtile[:], start=True, stop=True)
                    maskf = prep.tile([128, 4, 512], F32, name="maskf", tag="maskf", bufs=1)
                    nc.sync.dma_start(maskf[:], mask_d.ap().rearrange("j p s -> p j s"))
                    nc.vector.tensor_copy(out=maskb[:], in_=maskf[:])
                    # x^T arrives pre-transposed/pre-cast from the host: 16 DMAs
                    # (one per eo) so Q/K accumulation can start as chunks land
                    for eo in range(16):
                        nc.sync.dma_start(xT[:, eo, :], xtb_d[:, eo, :])

                # ---- Q/K projections for both heads (scaled/biased) ----
                with tc.tile_pool(name="qkw", bufs=2) as qkw, nc.named_scope("qkproj"):
                    for h in range(HPC):
                        for qi, (w_d, b_sb, scl) in enumerate(
                            ((wq_d, bq_sb, ISCALE), (wk_d, bk_sb, 1.0))
                        ):
                            wf = qkw.tile([128, 16, KD], F32, name="wf", tag="wf")
                            nc.sync.dma_start(wf[:], w_d[h])
                            wb = qkw.tile([128, 16, KD], BF16, name="wb", tag="wb")
                            nc.vector.tensor_copy(out=wb[:], in_=wf[:])
                            pqs = [
                                ps512.tile([128, 512], F32, name=f"pq{sb}", tag="p512")
                                for sb in range(4)
                            ]
                            for eo in range(16):
                                for sb in range(4):
                                    nc.tensor.matmul(
                                        pqs[sb][:],
                                        wb[:, eo, :],
                                        xT[:, eo, sb * 512 : (sb + 1) * 512],
                                        start=(eo == 0),
                                        stop=(eo == 15),
                                    )
                            for sb in range(4):
                                nc.scalar.activation(
                                    qkT[:, qi, h, sb * 512 : (sb + 1) * 512],
                                    pqs[sb][:],
                                    AF.Identity,
                                    bias=b_sb[:, h : h + 1],
                                    scale=scl,
                                )

                # ---- per-head attention ----
                eT = ap_.tile([128, NTRI, 512], BF16, name="eT")
                v_sb = ap_.tile([128, 16, FSH], BF16, name="v_sb")
                with (
                    tc.tile_pool(name="wvp", bufs=3) as wvp,
                    tc.tile_pool(name="wvb", bufs=1) as wvbp,
                    tc.tile_pool(name="astg", bufs=4) as astg,
                ):
                    for h in range(HPC):
                      with nc.named_scope(f"scores{h}"):
                        for sb in range(4):
                            for tcn in range(4 * sb + 4):
                                psc = ps512.tile([128, 512], F32, name="psc", tag="p512")
                                nc.tensor.matmul(
                                    psc[:],
                                    qkT[:, 1, h, tcn * 128 : (tcn + 1) * 128],
                                    qkT[:, 0, h, sb * 512 : (sb + 1) * 512],
                                    start=True,
                                    stop=True,
                                )
                                dst = eT[:, TRI[sb] + tcn, :]
                                if tcn >= 4 * sb:
                                    etmp = astg.tile(
                                        [128, 512], BF16, name="etmp", tag="etmp"
                                    )
                                    nc.scalar.activation(etmp[:], psc[:], AF.Exp)
                                    nc.vector.tensor_tensor(
                                        dst, etmp[:], maskb[:, tcn - 4 * sb, :], AL.mult
                                    )
                                else:
                                    nc.scalar.activation(dst, psc[:], AF.Exp)

                        # pass B: per f-half: v-projection then attn@v
                        for fh in range(2):
                          with nc.named_scope(f"vproj{h}{fh}"):
                            for fb in range(2):
                                wvb = wvbp.tile([128, 16, 512], BF16, name="wvb", tag="wvb")
                                for eo in range(16):
                                    wvf = wvp.tile([128, 512], F32, name="wvf", tag="wvf")
                                    nc.sync.dma_start(
                                        wvf[:],
                                        wv_d[
                                            h,
                                            :,
                                            eo,
                                            fh * 1024 + fb * 512 : fh * 1024 + (fb + 1) * 512,
                                        ],
                                    )
                                    nc.vector.tensor_copy(out=wvb[:, eo, :], in_=wvf[:])
                                for tcn in range(16):
                                    pv = ps512.tile([128, 512], F32, name="pv", tag="p512")
                                    for eo in range(16):
                                        nc.tensor.matmul(
                                            pv[:],
                                            xT[:, eo, tcn * 128 : (tcn + 1) * 128],
                                            wvb[:, eo, :],
                                            start=(eo == 0),
                                            stop=(eo == 15),
                                        )
                                    nc.vector.tensor_copy(
                                        out=v_sb[:, tcn, fb * 512 : (fb + 1) * 512],
                                        in_=pv[:],
                                    )

                          with nc.named_scope(f"attnv{h}{fh}"):
                            for i in range(15, -1, -1):
                                sb, so = i // 4, (i % 4) * 128
                                pa = [
                                    ps512.tile([128, 512], F32, name=f"pa{fb}", tag="p512")
                                    for fb in range(2)
                                ]
                                if fh == 0:
                                    pr = psR.tile([128, 1], F32, name="pr", tag="pr")
                                for tcn in range(i + 1):
                                    lhs = eT[:, TRI[sb] + tcn, so : so + 128]
                                    for fb in range(2):
                                        nc.tensor.matmul(
                                            pa[fb][:],
                                            lhs,
                                            v_sb[:, tcn, fb * 512 : (fb + 1) * 512],
                                            start=(tcn == 0),
                                            stop=(tcn == i),
                                        )
                                    if fh == 0:
                                        nc.tensor.matmul(
                                            pr[:],
                                            lhs,
                                            ones_bf[:],
                                            start=(tcn == 0),
                                            stop=(tcn == i),
                                        )
                                if fh == 0:
                                    rsf = astg.tile([128, 1], F32, name="rsf", tag="rsf")
                                    nc.vector.tensor_copy(out=rsf[:], in_=pr[:])
                                    nc.vector.reciprocal(recips[:, h, i : i + 1], rsf[:])
                                stg = astg.tile([128, 1024], F16, name="stg", tag="stg")
                                for fb in range(2):
                                    nc.scalar.activation(
                                        stg[:, fb * 512 : (fb + 1) * 512],
                                        pa[fb][:],
                                        AF.Copy,
                                        scale=recips[:, h, i : i + 1],
                                    )
                                nc.sync.dma_start(
                                    att_in[h][fh][i * 128 : (i + 1) * 128, :], stg[:]
                                )
                            nc.gpsimd.collective_compute(
                                "ReduceScatter",
                                AL.add,
                                replica_groups=RG,
                                ins=[att_in[h][fh][:]],
                                outs=[att_out[h][fh][:]],
                            )

            # =========== LN1 (global mean/var) ===========
            with tc.tile_pool(name="mid", bufs=1) as midp:
              h_own = midp.tile([128, 2, E], BF16, name="h_own")
              with tc.tile_pool(name="ln1", bufs=1) as lp, nc.named_scope("ln1"):
                  ys = lp.tile([128, 2, E], F32, name="ys")
                  yb1t = lp.tile([128, E], F32, name="yb1t")
                  nc.sync.dma_start(yb1t[:], yb1_d[:])
                  for rt in range(2):
                      xrt = lp.tile([128, E], F32, name="xrt", tag="xrt", bufs=2)
                      nc.sync.dma_start(xrt[:], xr_d[rt * 128 : (rt + 1) * 128, :])
                      nc.vector.tensor_tensor(ys[:, rt, :], xrt[:], yb1t[:], AL.add)
                      for h in range(HPC):
                          for fh in range(2):
                              rof = lp.tile([128, FSH], F16, name="rof", tag="rof", bufs=2)
                              nc.sync.dma_start(
                                  rof[:], att_out[h][fh][rt * 128 : (rt + 1) * 128, :]
                              )
                              dstv = ys[:, rt, fh * FSH : (fh + 1) * FSH]
                              nc.vector.tensor_tensor(dstv, dstv, rof[:], AL.add)

                  _stats_ln(nc, tc, lp, psT, ys, onesc, onesr, st1_in, st1_out, RG)
                  bc = _ln_scalars(nc, lp, psT, onesr, st1_out)
                  lngt = midp.tile([128, 2, E], BF16, name="lngt")
                  nc.sync.dma_start(lngt[:], lng_d.ap().rearrange("(t p) e -> p t e", p=128))
                  lnbt = midp.tile([128, 2, E], BF16, name="lnbt")
                  nc.sync.dma_start(lnbt[:], lnb_d.ap().rearrange("(t p) e -> p t e", p=128))
                  ht_f32 = lp.tile([128, E], F32, name="ht_f32", tag="htf", bufs=2)
                  for rt in range(2):
                      nc.scalar.activation(
                          ht_f32[:],
                          ys[:, rt, :],
                          AF.Identity,
                          bias=bc[:, 0:1],
                          scale=bc[:, 1:2],
                      )
                      nc.vector.tensor_tensor(
                          ht_f32[:], ht_f32[:], lngt[:, rt, :], AL.mult
                      )
                      nc.vector.tensor_tensor(
                          h_own[:, rt, :], ht_f32[:], lnbt[:, rt, :], AL.add
                      )
                  agb = h_own
                  for j in range(2):
                      nc.sync.dma_start(
                          ag_in[j].rearrange("(t p) e -> p t e", p=128),
                          agb[:, :, j * (E // 2) : (j + 1) * (E // 2)],
                      )
                      nc.gpsimd.collective_compute(
                          "AllGather",
                          AL.bypass,
                          replica_groups=RG,
                          ins=[ag_in[j][:]],
                          outs=[ag_out[j][:]],
                      )

              # =========== FFN (hidden shard 1024) ===========
              with tc.tile_pool(name="ffn", bufs=1) as fp, nc.named_scope("ffn"):
                  hT = fp.tile([128, 16, S], BF16, name="hT")
                  zT = fp.tile([128, 8, S], BF16, name="zT")
                  with tc.tile_pool(name="wst", bufs=2) as wst:
                      hidb = wst.tile([128, 128], BF16, name="hidb", tag="hidb", bufs=1)
                      nc.vector.tensor_copy(out=hidb[:], in_=ident[:])
                      for j in range(2):
                          for st in range(16):
                              hrow = wst.tile([128, E // 2], BF16, name="hrow", tag="hrow", bufs=3)
                              nc.sync.dma_start(
                                  hrow[:], ag_out[j][st * 128 : (st + 1) * 128, :]
                              )
                              for eh in range(8):
                                  eo = j * 8 + eh
                                  pth = psT.tile([128, 128], BF16, name="pth", tag="pt")
                                  nc.tensor.transpose(
                                      pth[:], hrow[:, eh * 128 : (eh + 1) * 128], hidb[:]
                                  )
                                  if eo % 2 == 0:
                                      nc.vector.tensor_copy(
                                          out=hT[:, eo, st * 128 : (st + 1) * 128], in_=pth[:]
                                      )
                                  else:
                                      nc.scalar.copy(
                                          hT[:, eo, st * 128 : (st + 1) * 128], pth[:]
                                      )
                      for ft in range(8):
                          w1f = wst.tile([128, 2048], F32, name="w1f", tag="w1f", bufs=1)
                          nc.sync.dma_start(w1f[:], w1_d[:, ft, :])
                          w1b = wst.tile([128, 16, KD], BF16, name="w1b", tag="w1b")
                          nc.vector.tensor_copy(
                              out=w1b[:], in_=w1f.rearrange("p (eo f) -> p eo f", eo=16)
                          )
                          pzs = [
                              ps512.tile([128, 512], F32, name=f"pz{sb}", tag="p512")
                              for sb in range(4)
                          ]
                          for eo in range(16):
                              for sb in range(4):
                                  nc.tensor.matmul(
                                      pzs[sb][:],
                                      w1b[:, eo, :],
                                      hT[:, eo, sb * 512 : (sb + 1) * 512],
                                      start=(eo == 0),
                                      stop=(eo == 15),
                                  )
                          for sb in range(4):
                              nc.scalar.activation(
                                  zT[:, ft, sb * 512 : (sb + 1) * 512],
                                  pzs[sb][:],
                                  AF.Relu,
                                  bias=b1_sb[:, ft : ft + 1],
                              )
                      for eb in range(4):
                          w2b = wst.tile([128, 8, 512], BF16, name="w2b", tag="w2b")
                          for wh in range(2):
                              w2f = wst.tile([128, 4, 512], F32, name="w2f", tag="w2f", bufs=1)
                              nc.sync.dma_start(w2f[:], w2_d[:, wh * 4 : (wh + 1) * 4, eb, :])
                              nc.vector.tensor_copy(out=w2b[:, wh * 4 : (wh + 1) * 4, :], in_=w2f[:])
                          for i in range(15, -1, -1):
                              pf = ps512.tile([128, 512], F32, name="pf", tag="p512")
                              for fc in range(8):
                                  nc.tensor.matmul(
                                      pf[:],
                                      zT[:, fc, i * 128 : (i + 1) * 128],
                                      w2b[:, fc, :],
                                      start=(fc == 0),
                                      stop=(fc == 7),
                                  )
                              fstg = wst.tile([128, 512], F16, name="fstg", tag="fstg", bufs=4)
                              nc.scalar.activation(fstg[:], pf[:], AF.Copy)
                              nc.sync.dma_start(
                                  ffn_in[eb][i * 128 : (i + 1) * 128, :], fstg[:]
                              )
                          nc.gpsimd.collective_compute(
                              "ReduceScatter",
                              AL.add,
                              replica_groups=RG,
                              ins=[ffn_in[eb][:]],
                              outs=[ffn_out[eb][:]],
                          )

              # =========== LN2 + output ===========
              with tc.tile_pool(name="ln2", bufs=1) as l2, nc.named_scope("ln2"):
                  ys2 = l2.tile([128, 2, E], F32, name="ys2")
                  yb2t = l2.tile([128, E], F32, name="yb2t")
                  nc.sync.dma_start(yb2t[:], yb2_d[:])
                  for rt in range(2):
                      nc.vector.tensor_tensor(
                          ys2[:, rt, :], h_own[:, rt, :], yb2t[:], AL.add
                      )
                      for eb in range(4):
                          fot = l2.tile([128, 512], F16, name="fot", tag="fot", bufs=2)
                          nc.sync.dma_start(
                              fot[:], ffn_out[eb][rt * 128 : (rt + 1) * 128, :]
                          )
                          dstv = ys2[:, rt, eb * 512 : (eb + 1) * 512]
                          nc.vector.tensor_tensor(dstv, dstv, fot[:], AL.add)

                  _stats_ln(nc, tc, l2, psT, ys2, onesc, onesr, st2_in, st2_out, RG)
                  bc2 = _ln_scalars(nc, l2, psT, onesr, st2_out)
                  lngt2 = lngt
                  lnbt2 = lnbt
                  for rt in range(2):
                      ot = l2.tile([128, E], F32, name="ot", tag="ot", bufs=2)
                      nc.scalar.activation(
                          ot[:],
                          ys2[:, rt, :],
                          AF.Identity,
                          bias=bc2[:, 0:1],
                          scale=bc2[:, 1:2],
                      )
                      nc.vector.tensor_tensor(ot[:], ot[:], lngt2[:, rt, :], AL.mult)
                      nc.vector.tensor_tensor(ot[:], ot[:], lnbt2[:, rt, :], AL.add)
                      nc.sync.dma_start(out_d[rt * 128 : (rt + 1) * 128, :], ot[:])

    nc.compile()
    return nc


def _stats_ln(nc, tc, pool, psT, ys, onesc, onesr, st_in, st_out, RG):
    """partial sum/sumsq of ys [128, 2, E] -> tiny fp32 AllReduce.

    Computed per (row-tile, column-half) so each partial only depends on the
    ReduceScatter chunks feeding that half (starts before the last RS lands).
    """
    parts = pool.tile([128, 8], F32, name="parts", tag="parts")
    sqs = pool.tile([128, E // 2], BF16, name="sqs", tag="sqs")
    for rt in range(2):
        for ch in range(2):
            idx = rt * 2 + ch
            ysl = ys[:, rt, ch * (E // 2) : (ch + 1) * (E // 2)]
            nc.vector.tensor_reduce(parts[:, idx : idx + 1], ysl, axis=AX.X, op=AL.add)
            nc.scalar.activation(
                sqs[:], ysl, AF.Square, accum_out=parts[:, 4 + idx : 5 + idx]
            )
    pstat = psT.tile([128, 128], F32, name="pstat", tag="pt")
    nc.tensor.matmul(pstat[:1, :8], onesc[:, 0:1], parts[:], start=True, stop=True)
    st4s = pool.tile([1, 8], F32, name="st4s", tag="st4s")
    nc.vector.tensor_copy(out=st4s[:], in_=pstat[:1, :8])
    st4 = pool.tile([1, 8], F32, name="st4", tag="st4")
    nc.vector.memset(st4[:], 0.0)
    nc.vector.tensor_reduce(st4[:, 0:1], st4s[:, 0:4], axis=AX.X, op=AL.add)
    nc.vector.tensor_reduce(st4[:, 1:2], st4s[:, 4:8], axis=AX.X, op=AL.add)
    nc.sync.dma_start(st_in[:], st4[:])
    nc.gpsimd.collective_compute(
        "AllReduce", AL.add, replica_groups=RG, ins=[st_in[:]], outs=[st_out[:]]
    )


def _ln_scalars(nc, pool, psT, onesr, st_out):
    """AllReduced (sum, sumsq) -> bc [128, 2] = (-m*rstd, rstd) broadcast."""
    so = pool.tile([1, 8], F32, name="so", tag="so")
    nc.sync.dma_start(so[:], st_out[:])
    sc = pool.tile([1, 8], F32, name="sc", tag="sc")
    # sc0 = m, sc1 = E[y^2], sc2 = m^2, sc3 = var, sc4 = rstd, sc5 = -m*rstd
    nc.scalar.mul(sc[:, 0:1], so[:, 0:1], 1.0 / NTOT)
    nc.scalar.mul(sc[:, 1:2], so[:, 1:2], 1.0 / NTOT)
    nc.scalar.activation(sc[:, 2:3], sc[:, 0:1], AF.Square)
    nc.vector.tensor_tensor(sc[:, 3:4], sc[:, 1:2], sc[:, 2:3], AL.subtract)
    nc.vector.tensor_scalar_add(sc[:, 2:3], sc[:, 3:4], EPS)  # var + eps
    # rstd = exp(-0.5 * ln(var + eps)) (keeps ACT on the exp/ln table)
    nc.scalar.activation(sc[:, 6:7], sc[:, 2:3], AF.Ln)
    nc.scalar.activation(sc[:, 4:5], sc[:, 6:7], AF.Exp, scale=-0.5)
    nc.vector.tensor_tensor(sc[:, 7:8], sc[:, 0:1], sc[:, 4:5], AL.mult)
    nc.scalar.mul(sc[:, 5:6], sc[:, 7:8], -1.0)
    s2 = pool.tile([1, 2], F32, name="s2", tag="s2")
    nc.vector.tensor_copy(out=s2[:, 0:1], in_=sc[:, 5:6])
    nc.vector.tensor_copy(out=s2[:, 1:2], in_=sc[:, 4:5])
    pb = psT.tile([128, 128], F32, name="pb", tag="pt")
    nc.tensor.matmul(pb[:, :2], onesr[:], s2[:], start=True, stop=True)
    bc = pool.tile([128, 2], F32, name="bc", tag="bc")
    nc.vector.tensor_copy(out=bc[:], in_=pb[:, :2])
    return bc


_NC_CACHE = None


def _get_nc():
    global _NC_CACHE
    if _NC_CACHE is None:
        _NC_CACHE = _build()
    return _NC_CACHE


def _prep_core(c, inputs):
    f32 = np.float32
    x = np.ascontiguousarray(inputs["input"], dtype=f32)
    Wq, Wk, Wv = inputs["Wq"], inputs["Wk"], inputs["Wv"]
    bq, bk, bv = inputs["bq"], inputs["bk"], inputs["bv"]
    W1, b1, W2, b2 = inputs["W1"], inputs["b1"], inputs["W2"], inputs["b2"]
    ln_g, ln_b = inputs["ln_g"], inputs["ln_b"]
    h0 = c * HPC
    wqt = np.ascontiguousarray(
        np.stack(
            [Wq[h0 + h].reshape(16, 128, KD).transpose(1, 0, 2) for h in range(HPC)]
        ),
        dtype=f32,
    )
    wkt = np.ascontiguousarray(
        np.stack(
            [Wk[h0 + h].reshape(16, 128, KD).transpose(1, 0, 2) for h in range(HPC)]
        ),
        dtype=f32,
    )
    wvt = np.ascontiguousarray(
        np.stack(
            [Wv[h0 + h].reshape(16, 128, E).transpose(1, 0, 2) for h in range(HPC)]
        ),
        dtype=f32,
    )
    W1s = W1[:, c * FSH : (c + 1) * FSH]
    w1t = np.ascontiguousarray(
        W1s.reshape(16, 128, 8, 128).transpose(1, 2, 0, 3).reshape(128, 8, 2048),
        dtype=f32,
    )
    W2s = W2[c * FSH : (c + 1) * FSH, :]
    w2t = np.ascontiguousarray(
        W2s.reshape(8, 128, 4, 512).transpose(1, 0, 2, 3), dtype=f32
    )
    bqs = np.ascontiguousarray((bq[h0 : h0 + HPC] * ISCALE).T, dtype=f32)
    bks = np.ascontiguousarray(bk[h0 : h0 + HPC].T, dtype=f32)
    b1s = np.ascontiguousarray(b1[c * FSH : (c + 1) * FSH].reshape(8, 128).T, dtype=f32)
    yb1 = np.ascontiguousarray(np.broadcast_to(bv.sum(axis=0), (128, E)), dtype=f32)
    yb2 = np.ascontiguousarray(np.broadcast_to(b2, (128, E)), dtype=f32)
    rows = slice(c * RROWS, (c + 1) * RROWS)
    jj, tp, sf = np.meshgrid(
        np.arange(4), np.arange(128), np.arange(512), indexing="ij"
    )
    mask = ((128 * jj + tp) <= sf).astype(f32)
    xtb = np.ascontiguousarray(
        x.T.reshape(16, 128, S).transpose(1, 0, 2).astype(_bf16)
    )
    return {
        "xtb": xtb,
        "wqt": wqt,
        "wkt": wkt,
        "wvt": wvt,
        "w1t": w1t,
        "w2t": w2t,
        "bqs": bqs,
        "bks": bks,
        "b1s": b1s,
        "yb1": yb1,
        "yb2": yb2,
        "xr": np.ascontiguousarray(x[rows], dtype=f32),
        "lngr": np.ascontiguousarray(np.asarray(ln_g[rows], dtype=f32).astype(_bf16)),
        "lnbr": np.ascontiguousarray(np.asarray(ln_b[rows], dtype=f32).astype(_bf16)),
        "mask": np.ascontiguousarray(mask),
        "ident": np.eye(128, dtype=f32),
        "ones": np.ones((128, 8), dtype=f32),
        "onesr": np.ones((1, 128), dtype=f32),
    }


def kernel(**inputs):
    nc = _get_nc()
    inputs = {k: np.asarray(v, dtype=np.float32) for k, v in inputs.items()}
    in_maps = [_prep_core(c, inputs) for c in range(NCORES)]
    res = run_bass_kernel_spmd(nc, in_maps, core_ids=list(range(NCORES)))
    out = np.concatenate([res.results[c]["out"] for c in range(NCORES)], axis=0)
    return np.ascontiguousarray(out, dtype=np.float32)



# revision 11
# speedup vs baseline: 1.3623x; 1.3623x over previous
"""Tensor-parallel decoder layer on 8 TRN2 NeuronCores.

Sharding:
  - Attention: 16 heads -> 2 per core. Per-core partial attn_out is
    ReduceScattered (fp16) so core c owns rows [256c, 256c+256).
  - Global LayerNorm (scalar mean/var over the whole [S,E] tensor):
    per-core partial (sum, sumsq) AllReduced as a tiny fp32 tensor.
  - FFN: hidden dim 8192 -> 1024 per core; partial [S,E] output
    ReduceScattered per 512-column chunk (fp16), overlapping FFN2.
  - h is AllGathered transposed (bf16) since every matmul contracting
    over E needs h^T as the moving operand.

Matmul layout notes (PE computes out = lhsT.T @ rhs, contraction on the
partition dim):
  - x^T resident in SBUF (bf16 for Q/K, fp8e4 for V) feeds projections.
  - The value path runs in fp8e4 DoubleRow (2 k-tiles per instruction,
    2x MAC rate): V projection contracts eo pairs, attn@v contracts
    t-tile pairs. exp is shifted by a constant (exp(s - SH)) so scores
    fit fp8's +/-240 range; the shift cancels exactly in num/rowsum.
  - scores are built transposed: S^T[t,s] tiles, so exp(S^T) tiles feed
    attn@v directly as lhsT with no transposes; softmax normalization is
    deferred: rowsum via a fp8 ones-column matmul, applied as a
    per-partition scale on the PSUM->SBUF copy of attn@v output.
  - causal masking folds into the score: a -30 additive mask lands on
    the PSUM tile before the exp, so masked entries underflow to 0.
"""

import math
import sys

sys.path.insert(0, "/opt/trn_rl_repo")

import numpy as np
import ml_dtypes

_bf16 = ml_dtypes.bfloat16
_f8 = ml_dtypes.float8_e4m3

import concourse.bass as bass
import concourse.mybir as mybir
import concourse.tile as tile
from concourse import bacc
from concourse.bass_utils import run_bass_kernel_spmd

S, E, H, KD, FF = 2048, 2048, 16, 128, 8192
EPS = 1e-5
NCORES = 8
HPC = H // NCORES          # heads per core = 2
FSH = FF // NCORES         # ffn hidden shard = 1024
RROWS = S // NCORES        # row shard = 256
NTOT = float(S * E)
ISCALE = 1.0 / math.sqrt(KD)

F32 = mybir.dt.float32
BF16 = mybir.dt.bfloat16
F16 = mybir.dt.float16
FP8 = mybir.dt.float8e4
AF = mybir.ActivationFunctionType
AL = mybir.AluOpType
AX = mybir.AxisListType
DR = mybir.MatmulPerfMode.DoubleRow

# packed triangular offsets for eT tiles: tile(tc, sb) at TRI[sb] + tc
TRI = [0, 4, 12, 24]
NTRI = 40
SH = 2.5  # constant exp shift: keeps exp(score - SH) within fp8e4 range


def _build():
    nc = bacc.Bacc(
        "TRN2",
        target_bir_lowering=False,
        debug=False,
        enable_asserts=True,
        num_devices=NCORES,
    )

    # ---- external I/O (per-core shards prepared on the host) ----
    xtb_d = nc.dram_tensor("xtb", [128, 16, S], BF16, kind="ExternalInput")
    xt8_d = nc.dram_tensor("xt8", [128, 16, S], FP8, kind="ExternalInput")
    wq_d = nc.dram_tensor("wqt", [HPC, 128, 16, KD], BF16, kind="ExternalInput")
    wk_d = nc.dram_tensor("wkt", [HPC, 128, 16, KD], BF16, kind="ExternalInput")
    wv_d = nc.dram_tensor("wvt", [HPC, 128, 16, E], FP8, kind="ExternalInput")
    w1_d = nc.dram_tensor("w1t", [128, 8, 2048], BF16, kind="ExternalInput")
    w2_d = nc.dram_tensor("w2t", [128, 8, 4, 512], BF16, kind="ExternalInput")
    bq_d = nc.dram_tensor("bqs", [128, HPC], F32, kind="ExternalInput")
    bk_d = nc.dram_tensor("bks", [128, HPC], F32, kind="ExternalInput")
    b1_d = nc.dram_tensor("b1s", [128, 8], F32, kind="ExternalInput")
    yb1_d = nc.dram_tensor("yb1", [128, E], F32, kind="ExternalInput")
    yb2_d = nc.dram_tensor("yb2", [128, E], F32, kind="ExternalInput")
    xr_d = nc.dram_tensor("xr", [RROWS, E], F32, kind="ExternalInput")
    lng_d = nc.dram_tensor("lngr", [RROWS, E], BF16, kind="ExternalInput")
    lnb_d = nc.dram_tensor("lnbr", [RROWS, E], BF16, kind="ExternalInput")
    mask_d = nc.dram_tensor("mask", [4, 128, 512], F32, kind="ExternalInput")
    id_d = nc.dram_tensor("ident", [128, 128], F32, kind="ExternalInput")
    ones_d = nc.dram_tensor("ones", [128, 8], F32, kind="ExternalInput")
    onesr_d = nc.dram_tensor("onesr", [1, 128], F32, kind="ExternalInput")
    out_d = nc.dram_tensor("out", [RROWS, E], F32, kind="ExternalOutput")

    RG = [list(range(NCORES))]

    with tile.TileContext(nc) as tc:
        with (
            tc.tile_pool(name="persist", bufs=1) as pp,
            tc.tile_pool(name="dram", bufs=1, space="DRAM") as dp,
            tc.tile_pool(name="ps512", bufs=4, space="PSUM") as ps512,
            tc.tile_pool(name="psT", bufs=2, space="PSUM") as psT,
            tc.tile_pool(name="psR", bufs=2, space="PSUM") as psR,
        ):
            # ---- collective bounce buffers (internal DRAM) ----
            att_in = [
                [
                    dp.tile([S, FSH], F16, name=f"att_in_{h}_{fh}", tag=f"ati{h}{fh}")
                    for fh in range(2)
                ]
                for h in range(HPC)
            ]
            att_out = [
                [
                    dp.tile(
                        [RROWS, FSH],
                        F16,
                        name=f"att_out_{h}_{fh}",
                        tag=f"ato{h}{fh}",
                    )
                    for fh in range(2)
                ]
                for h in range(HPC)
            ]
            st1_in = dp.tile([1, 8], F32, name="st1_in", tag="st1i")
            st1_out = dp.tile([1, 8], F32, name="st1_out", tag="st1o", addr_space="Shared")
            st2_in = dp.tile([1, 8], F32, name="st2_in", tag="st2i")
            st2_out = dp.tile([1, 8], F32, name="st2_out", tag="st2o", addr_space="Shared")
            ag_in = [
                dp.tile([RROWS, E // 2], BF16, name=f"ag_in{j}", tag=f"agi{j}")
                for j in range(2)
            ]
            ag_out = [
                dp.tile([S, E // 2], BF16, name=f"ag_out{j}", tag=f"ago{j}", addr_space="Shared")
                for j in range(2)
            ]
            ffn_in = [
                dp.tile([S, 512], F16, name=f"ffn_in_{eb}", tag=f"ffi{eb}")
                for eb in range(4)
            ]
            ffn_out = [
                dp.tile([RROWS, 512], F16, name=f"ffn_out_{eb}", tag=f"ffo{eb}")
                for eb in range(4)
            ]

            # ---- persistent small tiles ----
            ident = pp.tile([128, 128], F32, name="ident")
            nc.sync.dma_start(ident[:], id_d[:])
            onesc = pp.tile([128, 8], F32, name="onesc")
            nc.sync.dma_start(onesc[:], ones_d[:])
            onesr = pp.tile([1, 128], F32, name="onesr")
            nc.sync.dma_start(onesr[:], onesr_d[:])
            bq_sb = pp.tile([128, HPC], F32, name="bq_sb")
            nc.sync.dma_start(bq_sb[:], bq_d[:])
            bk_sb = pp.tile([128, HPC], F32, name="bk_sb")
            nc.sync.dma_start(bk_sb[:], bk_d[:])
            b1_sb = pp.tile([128, 8], F32, name="b1_sb")
            nc.sync.dma_start(b1_sb[:], b1_d[:])
            qkT = pp.tile([128, 2, HPC, S], BF16, name="qkT")  # [d, q/k, head, s]
            recips = pp.tile([128, HPC, 16], F32, name="recips")

            # =========== phase 0 + attention ===========
            with tc.tile_pool(name="attn", bufs=1) as ap_:
                xT8 = ap_.tile([128, 16, S], FP8, name="xT8")  # x^T fp8 for V
                maskf = ap_.tile([128, 4, 512], F32, name="maskf")  # (mask-1)*30
                shm = ap_.tile([128, 1], F32, name="shm")  # exp shift bias
                ones8 = ap_.tile([128, 2, 1], FP8, name="ones8")
                with tc.tile_pool(name="qkx", bufs=1) as qkx:
                    xT = qkx.tile([128, 16, S], BF16, name="xT")  # x^T bf16 for Q/K
                    wqk = [
                        [
                            qkx.tile([128, 16, KD], BF16, name=f"wqk{h}{qi}")
                            for qi in range(2)
                        ]
                        for h in range(HPC)
                    ]
                    with tc.tile_pool(name="prep", bufs=1) as prep, nc.named_scope("prep"):
                        # HAM warmup: back-to-back matmuls to unthrottle PE,
                        # issued while the first weight/x DMAs land
                        wtile = prep.tile([128, 512], BF16, name="wtile", tag="wtile", bufs=1)
                        nc.vector.memset(wtile[:], 0.0)
                        nc.vector.memset(shm[:], -SH)
                        nc.vector.memset(ones8[:], 1.0)
                        # small weight DMAs first so qkproj isn't stuck
                        # behind the bulk x^T transfer
                        for h in range(HPC):
                            nc.sync.dma_start(wqk[h][0][:], wq_d[h])
                            nc.sync.dma_start(wqk[h][1][:], wk_d[h])
                        nc.sync.dma_start(maskf[:], mask_d.ap().rearrange("j p s -> p j s"))
                        for _w in range(24):
                            pw = ps512.tile([128, 512], F32, name="pw", tag="p512")
                            nc.tensor.matmul(pw[:], wtile[:, :128], wtile[:], start=True, stop=True)
                        # x^T pre-transposed/pre-cast on the host: 16 DMAs
                        # (one per eo) so Q/K accumulation starts as chunks land
                        for eo in range(16):
                            nc.sync.dma_start(xT[:, eo, :], xtb_d[:, eo, :])
                        for eo in range(16):
                            nc.sync.dma_start(xT8[:, eo, :], xt8_d[:, eo, :])

                    # ---- Q/K projections for both heads (scaled/biased) ----
                    with nc.named_scope("qkproj"):
                        for h in range(HPC):
                            for qi, (b_sb, scl) in enumerate(
                                ((bq_sb, ISCALE), (bk_sb, 1.0))
                            ):
                                wb = wqk[h][qi]
                                pqs = [
                                    ps512.tile([128, 512], F32, name=f"pq{sb}", tag="p512")
                                    for sb in range(4)
                                ]
                                for eo in range(16):
                                    for sb in range(4):
                                        nc.tensor.matmul(
                                            pqs[sb][:],
                                            wb[:, eo, :],
                                            xT[:, eo, sb * 512 : (sb + 1) * 512],
                                            start=(eo == 0),
                                            stop=(eo == 15),
                                        )
                                for sb in range(4):
                                    nc.scalar.activation(
                                        qkT[:, qi, h, sb * 512 : (sb + 1) * 512],
                                        pqs[sb][:],
                                        AF.Identity,
                                        bias=b_sb[:, h : h + 1],
                                        scale=scl,
                                    )

                # ---- per-head attention (value path in fp8 DoubleRow) ----
                eT = ap_.tile([128, NTRI, 512], FP8, name="eT")
                v_sb = ap_.tile([128, 16, FSH], FP8, name="v_sb")
                with (
                    tc.tile_pool(name="wvb", bufs=2) as wvbp,
                    tc.tile_pool(name="astg", bufs=4) as astg,
                ):
                    for h in range(HPC):
                      with nc.named_scope(f"scores{h}"):
                        for sb in range(4):
                            for tcn in range(4 * sb + 4):
                                psc = ps512.tile([128, 512], F32, name="psc", tag="p512")
                                nc.tensor.matmul(
                                    psc[:],
                                    qkT[:, 1, h, tcn * 128 : (tcn + 1) * 128],
                                    qkT[:, 0, h, sb * 512 : (sb + 1) * 512],
                                    start=True,
                                    stop=True,
                                )
                                if tcn >= 4 * sb:
                                    # diagonal tile: -30 mask onto PSUM, then
                                    # masked entries underflow to 0 in the exp
                                    nc.vector.tensor_tensor(
                                        psc[:], psc[:], maskf[:, tcn - 4 * sb, :], AL.add
                                    )
                                nc.scalar.activation(
                                    eT[:, TRI[sb] + tcn, :], psc[:], AF.Exp, bias=shm[:]
                                )

                        # pass B: per f-half: v-projection then attn@v
                        for fh in range(2):
                          with nc.named_scope(f"vproj{h}{fh}"):
                            for fb in range(2):
                                wvb = wvbp.tile([128, 16, 512], FP8, name="wvb", tag="wvb")
                                nc.sync.dma_start(
                                    wvb[:],
                                    wv_d[h][
                                        :, :, fh * 1024 + fb * 512 : fh * 1024 + (fb + 1) * 512
                                    ],
                                )
                                for tcn in range(16):
                                    pv = ps512.tile([128, 512], F32, name="pv", tag="p512")
                                    for j in range(8):
                                        nc.tensor.matmul(
                                            pv[:],
                                            xT8[:, 2 * j : 2 * j + 2, tcn * 128 : (tcn + 1) * 128],
                                            wvb[:, 2 * j : 2 * j + 2, :],
                                            start=(j == 0),
                                            stop=(j == 7),
                                            perf_mode=DR,
                                        )
                                    if tcn % 2 == 0:
                                        nc.vector.tensor_copy(
                                            out=v_sb[:, tcn, fb * 512 : (fb + 1) * 512],
                                            in_=pv[:],
                                        )
                                    else:
                                        nc.scalar.copy(
                                            v_sb[:, tcn, fb * 512 : (fb + 1) * 512], pv[:]
                                        )

                          with nc.named_scope(f"attnv{h}{fh}"):
                            for i in range(15, -1, -1):
                                sb, so = i // 4, (i % 4) * 128
                                npair = (i + 1) // 2
                                odd = (i + 1) % 2
                                pa = [
                                    ps512.tile([128, 512], F32, name=f"pa{fb}", tag="p512")
                                    for fb in range(2)
                                ]
                                if fh == 0:
                                    pr = psR.tile([128, 1], F32, name="pr", tag="pr")
                                for tp in range(npair):
                                    lhs = eT[:, TRI[sb] + 2 * tp : TRI[sb] + 2 * tp + 2, so : so + 128]
                                    last = (tp == npair - 1) and not odd
                                    for fb in range(2):
                                        nc.tensor.matmul(
                                            pa[fb][:],
                                            lhs,
                                            v_sb[:, 2 * tp : 2 * tp + 2, fb * 512 : (fb + 1) * 512],
                                            start=(tp == 0),
                                            stop=last,
                                            perf_mode=DR,
                                        )
                                    if fh == 0:
                                        nc.tensor.matmul(
                                            pr[:],
                                            lhs,
                                            ones8[:],
                                            start=(tp == 0),
                                            stop=last,
                                            perf_mode=DR,
                                        )
                                if odd:
                                    lhs1 = eT[:, TRI[sb] + i, so : so + 128]
                                    for fb in range(2):
                                        nc.tensor.matmul(
                                            pa[fb][:],
                                            lhs1,
                                            v_sb[:, i, fb * 512 : (fb + 1) * 512],
                                            start=(npair == 0),
                                            stop=True,
                                        )
                                    if fh == 0:
                                        nc.tensor.matmul(
                                            pr[:],
                                            lhs1,
                                            ones8[:, 0, :],
                                            start=(npair == 0),
                                            stop=True,
                                        )
                                if fh == 0:
                                    rsf = astg.tile([128, 1], F32, name="rsf", tag="rsf")
                                    nc.vector.tensor_copy(out=rsf[:], in_=pr[:])
                                    nc.vector.reciprocal(recips[:, h, i : i + 1], rsf[:])
                                stg = astg.tile([128, 1024], F16, name="stg", tag="stg")
                                for fb in range(2):
                                    nc.scalar.activation(
                                        stg[:, fb * 512 : (fb + 1) * 512],
                                        pa[fb][:],
                                        AF.Copy,
                                        scale=recips[:, h, i : i + 1],
                                    )
                                nc.sync.dma_start(
                                    att_in[h][fh][i * 128 : (i + 1) * 128, :], stg[:]
                                )
                            nc.gpsimd.collective_compute(
                                "ReduceScatter",
                                AL.add,
                                replica_groups=RG,
                                ins=[att_in[h][fh][:]],
                                outs=[att_out[h][fh][:]],
                            )

            # =========== LN1 (global mean/var) ===========
            with tc.tile_pool(name="mid", bufs=1) as midp:
              h_own = midp.tile([128, 2, E], BF16, name="h_own")
              with tc.tile_pool(name="ln1", bufs=1) as lp, nc.named_scope("ln1"):
                  ys = lp.tile([128, 2, E], F32, name="ys")
                  yb1t = lp.tile([128, E], F32, name="yb1t")
                  nc.sync.dma_start(yb1t[:], yb1_d[:])
                  for rt in range(2):
                      xrt = lp.tile([128, E], F32, name="xrt", tag="xrt", bufs=2)
                      nc.sync.dma_start(xrt[:], xr_d[rt * 128 : (rt + 1) * 128, :])
                      nc.vector.tensor_tensor(ys[:, rt, :], xrt[:], yb1t[:], AL.add)
                      for h in range(HPC):
                          for fh in range(2):
                              rof = lp.tile([128, FSH], F16, name="rof", tag="rof", bufs=2)
                              nc.sync.dma_start(
                                  rof[:], att_out[h][fh][rt * 128 : (rt + 1) * 128, :]
                              )
                              dstv = ys[:, rt, fh * FSH : (fh + 1) * FSH]
                              nc.vector.tensor_tensor(dstv, dstv, rof[:], AL.add)

                  _stats_ln(nc, tc, lp, psT, ys, onesc, onesr, st1_in, st1_out, RG)
                  bc = _ln_scalars(nc, lp, psT, onesr, st1_out)
                  lngt = midp.tile([128, 2, E], BF16, name="lngt")
                  nc.sync.dma_start(lngt[:], lng_d.ap().rearrange("(t p) e -> p t e", p=128))
                  lnbt = midp.tile([128, 2, E], BF16, name="lnbt")
                  nc.sync.dma_start(lnbt[:], lnb_d.ap().rearrange("(t p) e -> p t e", p=128))
                  ht_f32 = lp.tile([128, E], F32, name="ht_f32", tag="htf", bufs=2)
                  for rt in range(2):
                      nc.scalar.activation(
                          ht_f32[:],
                          ys[:, rt, :],
                          AF.Identity,
                          bias=bc[:, 0:1],
                          scale=bc[:, 1:2],
                      )
                      nc.vector.tensor_tensor(
                          ht_f32[:], ht_f32[:], lngt[:, rt, :], AL.mult
                      )
                      nc.vector.tensor_tensor(
                          h_own[:, rt, :], ht_f32[:], lnbt[:, rt, :], AL.add
                      )
                  agb = h_own
                  for j in range(2):
                      nc.sync.dma_start(
                          ag_in[j].rearrange("(t p) e -> p t e", p=128),
                          agb[:, :, j * (E // 2) : (j + 1) * (E // 2)],
                      )
                      nc.gpsimd.collective_compute(
                          "AllGather",
                          AL.bypass,
                          replica_groups=RG,
                          ins=[ag_in[j][:]],
                          outs=[ag_out[j][:]],
                      )

              # =========== FFN (hidden shard 1024) ===========
              with tc.tile_pool(name="ffn", bufs=1) as fp, nc.named_scope("ffn"):
                  hT = fp.tile([128, 16, S], BF16, name="hT")
                  zT = fp.tile([128, 8, S], BF16, name="zT")
                  with tc.tile_pool(name="wst", bufs=2) as wst:
                      hidb = wst.tile([128, 128], BF16, name="hidb", tag="hidb", bufs=1)
                      nc.vector.tensor_copy(out=hidb[:], in_=ident[:])
                      for j in range(2):
                          for st in range(16):
                              hrow = wst.tile([128, E // 2], BF16, name="hrow", tag="hrow", bufs=3)
                              nc.sync.dma_start(
                                  hrow[:], ag_out[j][st * 128 : (st + 1) * 128, :]
                              )
                              for eh in range(8):
                                  eo = j * 8 + eh
                                  pth = psT.tile([128, 128], BF16, name="pth", tag="pt")
                                  nc.tensor.transpose(
                                      pth[:], hrow[:, eh * 128 : (eh + 1) * 128], hidb[:]
                                  )
                                  if eo % 2 == 0:
                                      nc.vector.tensor_copy(
                                          out=hT[:, eo, st * 128 : (st + 1) * 128], in_=pth[:]
                                      )
                                  else:
                                      nc.scalar.copy(
                                          hT[:, eo, st * 128 : (st + 1) * 128], pth[:]
                                      )
                      for ft in range(8):
                          w1b = wst.tile([128, 2048], BF16, name="w1b", tag="w1b")
                          nc.sync.dma_start(w1b[:], w1_d[:, ft, :])
                          pzs = [
                              ps512.tile([128, 512], F32, name=f"pz{sb}", tag="p512")
                              for sb in range(4)
                          ]
                          for eo in range(16):
                              for sb in range(4):
                                  nc.tensor.matmul(
                                      pzs[sb][:],
                                      w1b[:, eo * KD : (eo + 1) * KD],
                                      hT[:, eo, sb * 512 : (sb + 1) * 512],
                                      start=(eo == 0),
                                      stop=(eo == 15),
                                  )
                          for sb in range(4):
                              nc.scalar.activation(
                                  zT[:, ft, sb * 512 : (sb + 1) * 512],
                                  pzs[sb][:],
                                  AF.Relu,
                                  bias=b1_sb[:, ft : ft + 1],
                              )
                      for eb in range(4):
                          w2b = wst.tile([128, 8, 512], BF16, name="w2b", tag="w2b")
                          nc.sync.dma_start(w2b[:], w2_d[:, :, eb, :])
                          for i in range(15, -1, -1):
                              pf = ps512.tile([128, 512], F32, name="pf", tag="p512")
                              for fc in range(8):
                                  nc.tensor.matmul(
                                      pf[:],
                                      zT[:, fc, i * 128 : (i + 1) * 128],
                                      w2b[:, fc, :],
                                      start=(fc == 0),
                                      stop=(fc == 7),
                                  )
                              fstg = wst.tile([128, 512], F16, name="fstg", tag="fstg", bufs=4)
                              nc.scalar.activation(fstg[:], pf[:], AF.Copy)
                              nc.sync.dma_start(
                                  ffn_in[eb][i * 128 : (i + 1) * 128, :], fstg[:]
                              )
                          nc.gpsimd.collective_compute(
                              "ReduceScatter",
                              AL.add,
                              replica_groups=RG,
                              ins=[ffn_in[eb][:]],
                              outs=[ffn_out[eb][:]],
                          )

              # =========== LN2 + output ===========
              with tc.tile_pool(name="ln2", bufs=1) as l2, nc.named_scope("ln2"):
                  ys2 = l2.tile([128, 2, E], F32, name="ys2")
                  yb2t = l2.tile([128, E], F32, name="yb2t")
                  nc.sync.dma_start(yb2t[:], yb2_d[:])
                  for rt in range(2):
                      nc.vector.tensor_tensor(
                          ys2[:, rt, :], h_own[:, rt, :], yb2t[:], AL.add
                      )
                      for eb in range(4):
                          fot = l2.tile([128, 512], F16, name="fot", tag="fot", bufs=2)
                          nc.sync.dma_start(
                              fot[:], ffn_out[eb][rt * 128 : (rt + 1) * 128, :]
                          )
                          dstv = ys2[:, rt, eb * 512 : (eb + 1) * 512]
                          nc.vector.tensor_tensor(dstv, dstv, fot[:], AL.add)

                  _stats_ln(nc, tc, l2, psT, ys2, onesc, onesr, st2_in, st2_out, RG)
                  bc2 = _ln_scalars(nc, l2, psT, onesr, st2_out)
                  lngt2 = lngt
                  lnbt2 = lnbt
                  for rt in range(2):
                      ot = l2.tile([128, E], F32, name="ot", tag="ot", bufs=2)
                      nc.scalar.activation(
                          ot[:],
                          ys2[:, rt, :],
                          AF.Identity,
                          bias=bc2[:, 0:1],
                          scale=bc2[:, 1:2],
                      )
                      nc.vector.tensor_tensor(ot[:], ot[:], lngt2[:, rt, :], AL.mult)
                      nc.vector.tensor_tensor(ot[:], ot[:], lnbt2[:, rt, :], AL.add)
                      nc.sync.dma_start(out_d[rt * 128 : (rt + 1) * 128, :], ot[:])

    nc.compile()
    return nc


def _stats_ln(nc, tc, pool, psT, ys, onesc, onesr, st_in, st_out, RG):
    """partial sum/sumsq of ys [128, 2, E] -> tiny fp32 AllReduce.

    Computed per (row-tile, column-half) so each partial only depends on the
    ReduceScatter chunks feeding that half (starts before the last RS lands).
    """
    parts = pool.tile([128, 8], F32, name="parts", tag="parts")
    sqs = pool.tile([128, E // 2], BF16, name="sqs", tag="sqs")
    for rt in range(2):
        for ch in range(2):
            idx = rt * 2 + ch
            ysl = ys[:, rt, ch * (E // 2) : (ch + 1) * (E // 2)]
            nc.vector.tensor_reduce(parts[:, idx : idx + 1], ysl, axis=AX.X, op=AL.add)
            nc.scalar.activation(
                sqs[:], ysl, AF.Square, accum_out=parts[:, 4 + idx : 5 + idx]
            )
    pstat = psT.tile([128, 128], F32, name="pstat", tag="pt")
    nc.tensor.matmul(pstat[:1, :8], onesc[:, 0:1], parts[:], start=True, stop=True)
    st4s = pool.tile([1, 8], F32, name="st4s", tag="st4s")
    nc.vector.tensor_copy(out=st4s[:], in_=pstat[:1, :8])
    st4 = pool.tile([1, 8], F32, name="st4", tag="st4")
    nc.vector.memset(st4[:], 0.0)
    nc.vector.tensor_reduce(st4[:, 0:1], st4s[:, 0:4], axis=AX.X, op=AL.add)
    nc.vector.tensor_reduce(st4[:, 1:2], st4s[:, 4:8], axis=AX.X, op=AL.add)
    nc.sync.dma_start(st_in[:], st4[:])
    nc.gpsimd.collective_compute(
        "AllReduce", AL.add, replica_groups=RG, ins=[st_in[:]], outs=[st_out[:]]
    )


def _ln_scalars(nc, pool, psT, onesr, st_out):
    """AllReduced (sum, sumsq) -> bc [128, 2] = (-m*rstd, rstd) broadcast."""
    so = pool.tile([1, 8], F32, name="so", tag="so")
    nc.sync.dma_start(so[:], st_out[:])
    sc = pool.tile([1, 8], F32, name="sc", tag="sc")
    # sc0 = m, sc1 = E[y^2], sc2 = m^2, sc3 = var, sc4 = rstd, sc5 = -m*rstd
    nc.scalar.mul(sc[:, 0:1], so[:, 0:1], 1.0 / NTOT)
    nc.scalar.mul(sc[:, 1:2], so[:, 1:2], 1.0 / NTOT)
    nc.scalar.activation(sc[:, 2:3], sc[:, 0:1], AF.Square)
    nc.vector.tensor_tensor(sc[:, 3:4], sc[:, 1:2], sc[:, 2:3], AL.subtract)
    nc.vector.tensor_scalar_add(sc[:, 2:3], sc[:, 3:4], EPS)  # var + eps
    # rstd = exp(-0.5 * ln(var + eps)) (keeps ACT on the exp/ln table)
    nc.scalar.activation(sc[:, 6:7], sc[:, 2:3], AF.Ln)
    nc.scalar.activation(sc[:, 4:5], sc[:, 6:7], AF.Exp, scale=-0.5)
    nc.vector.tensor_tensor(sc[:, 7:8], sc[:, 0:1], sc[:, 4:5], AL.mult)
    nc.scalar.mul(sc[:, 5:6], sc[:, 7:8], -1.0)
    s2 = pool.tile([1, 2], F32, name="s2", tag="s2")
    nc.vector.tensor_copy(out=s2[:, 0:1], in_=sc[:, 5:6])
    nc.vector.tensor_copy(out=s2[:, 1:2], in_=sc[:, 4:5])
    pb = psT.tile([128, 128], F32, name="pb", tag="pt")
    nc.tensor.matmul(pb[:, :2], onesr[:], s2[:], start=True, stop=True)
    bc = pool.tile([128, 2], F32, name="bc", tag="bc")
    nc.vector.tensor_copy(out=bc[:], in_=pb[:, :2])
    return bc


_NC_CACHE = None


def _get_nc():
    global _NC_CACHE
    if _NC_CACHE is None:
        _NC_CACHE = _build()
    return _NC_CACHE


def _prep_core(c, inputs):
    f32 = np.float32
    x = np.ascontiguousarray(inputs["input"], dtype=f32)
    Wq, Wk, Wv = inputs["Wq"], inputs["Wk"], inputs["Wv"]
    bq, bk, bv = inputs["bq"], inputs["bk"], inputs["bv"]
    W1, b1, W2, b2 = inputs["W1"], inputs["b1"], inputs["W2"], inputs["b2"]
    ln_g, ln_b = inputs["ln_g"], inputs["ln_b"]
    h0 = c * HPC
    wqt = np.ascontiguousarray(
        np.stack(
            [Wq[h0 + h].reshape(16, 128, KD).transpose(1, 0, 2) for h in range(HPC)]
        ).astype(_bf16)
    )
    wkt = np.ascontiguousarray(
        np.stack(
            [Wk[h0 + h].reshape(16, 128, KD).transpose(1, 0, 2) for h in range(HPC)]
        ).astype(_bf16)
    )
    wvt = np.ascontiguousarray(
        np.stack(
            [Wv[h0 + h].reshape(16, 128, E).transpose(1, 0, 2) for h in range(HPC)]
        ).astype(_f8)
    )
    W1s = W1[:, c * FSH : (c + 1) * FSH]
    w1t = np.ascontiguousarray(
        W1s.reshape(16, 128, 8, 128).transpose(1, 2, 0, 3).reshape(128, 8, 2048).astype(_bf16)
    )
    W2s = W2[c * FSH : (c + 1) * FSH, :]
    w2t = np.ascontiguousarray(
        W2s.reshape(8, 128, 4, 512).transpose(1, 0, 2, 3).astype(_bf16)
    )
    bqs = np.ascontiguousarray((bq[h0 : h0 + HPC] * ISCALE).T, dtype=f32)
    bks = np.ascontiguousarray(bk[h0 : h0 + HPC].T, dtype=f32)
    b1s = np.ascontiguousarray(b1[c * FSH : (c + 1) * FSH].reshape(8, 128).T, dtype=f32)
    yb1 = np.ascontiguousarray(np.broadcast_to(bv.sum(axis=0), (128, E)), dtype=f32)
    yb2 = np.ascontiguousarray(np.broadcast_to(b2, (128, E)), dtype=f32)
    rows = slice(c * RROWS, (c + 1) * RROWS)
    jj, tp, sf = np.meshgrid(
        np.arange(4), np.arange(128), np.arange(512), indexing="ij"
    )
    mask = (((128 * jj + tp) <= sf).astype(f32) - 1.0) * 30.0
    xt = x.T.reshape(16, 128, S).transpose(1, 0, 2)
    xtb = np.ascontiguousarray(xt.astype(_bf16))
    xt8 = np.ascontiguousarray(xt.astype(_f8))
    return {
        "xtb": xtb,
        "xt8": xt8,
        "wqt": wqt,
        "wkt": wkt,
        "wvt": wvt,
        "w1t": w1t,
        "w2t": w2t,
        "bqs": bqs,
        "bks": bks,
        "b1s": b1s,
        "yb1": yb1,
        "yb2": yb2,
        "xr": np.ascontiguousarray(x[rows], dtype=f32),
        "lngr": np.ascontiguousarray(np.asarray(ln_g[rows], dtype=f32).astype(_bf16)),
        "lnbr": np.ascontiguousarray(np.asarray(ln_b[rows], dtype=f32).astype(_bf16)),
        "mask": np.ascontiguousarray(mask),
        "ident": np.eye(128, dtype=f32),
        "ones": np.ones((128, 8), dtype=f32),
        "onesr": np.ones((1, 128), dtype=f32),
    }


def kernel(**inputs):
    nc = _get_nc()
    inputs = {k: np.asarray(v, dtype=np.float32) for k, v in inputs.items()}
    in_maps = [_prep_core(c, inputs) for c in range(NCORES)]
    res = run_bass_kernel_spmd(nc, in_maps, core_ids=list(range(NCORES)))
    out = np.concatenate([res.results[c]["out"] for c in range(NCORES)], axis=0)
    return np.ascontiguousarray(out, dtype=np.float32)



# revision 25
# speedup vs baseline: 1.3646x; 1.0017x over previous
"""Tensor-parallel decoder layer on 8 TRN2 NeuronCores.

Sharding:
  - Attention: 16 heads -> 2 per core. Per-core partial attn_out is
    ReduceScattered (fp16) so core c owns rows [256c, 256c+256).
  - Global LayerNorm (scalar mean/var over the whole [S,E] tensor):
    per-core partial (sum, sumsq) AllReduced as a tiny fp32 tensor.
  - FFN: hidden dim 8192 -> 1024 per core; partial [S,E] output
    ReduceScattered per 512-column chunk (fp16), overlapping FFN2.
  - h is AllGathered transposed (bf16) since every matmul contracting
    over E needs h^T as the moving operand.

Matmul layout notes (PE computes out = lhsT.T @ rhs, contraction on the
partition dim):
  - x^T resident in SBUF (bf16 for Q/K, fp8e4 for V) feeds projections.
  - The value path runs in fp8e4 DoubleRow (2 k-tiles per instruction,
    2x MAC rate): V projection contracts eo pairs, attn@v contracts
    t-tile pairs. exp is shifted by a constant (exp(s - SH)) so scores
    fit fp8's +/-240 range; the shift cancels exactly in num/rowsum.
  - scores are built transposed: S^T[t,s] tiles, so exp(S^T) tiles feed
    attn@v directly as lhsT with no transposes; softmax normalization is
    deferred: rowsum via a fp8 ones-column matmul, applied as a
    per-partition scale on the PSUM->SBUF copy of attn@v output.
  - causal masking folds into the score: a -30 additive mask lands on
    the PSUM tile before the exp, so masked entries underflow to 0.
"""

import math
import sys

sys.path.insert(0, "/opt/trn_rl_repo")

import numpy as np
import ml_dtypes

_bf16 = ml_dtypes.bfloat16
_f8 = ml_dtypes.float8_e4m3

import concourse.bass as bass
import concourse.mybir as mybir
import concourse.tile as tile
from concourse import bacc
from concourse.bass_utils import run_bass_kernel_spmd

S, E, H, KD, FF = 2048, 2048, 16, 128, 8192
EPS = 1e-5
NCORES = 8
HPC = H // NCORES          # heads per core = 2
FSH = FF // NCORES         # ffn hidden shard = 1024
RROWS = S // NCORES        # row shard = 256
NTOT = float(S * E)
ISCALE = 1.0 / math.sqrt(KD)

F32 = mybir.dt.float32
BF16 = mybir.dt.bfloat16
F16 = mybir.dt.float16
FP8 = mybir.dt.float8e4
AF = mybir.ActivationFunctionType
AL = mybir.AluOpType
AX = mybir.AxisListType
DR = mybir.MatmulPerfMode.DoubleRow

# packed triangular offsets for eT tiles: tile(tc, sb) at TRI[sb] + tc
TRI = [0, 4, 12, 24]
NTRI = 40
SH = 2.5  # constant exp shift: keeps exp(score - SH) within fp8e4 range


def _build():
    nc = bacc.Bacc(
        "TRN2",
        target_bir_lowering=False,
        debug=False,
        enable_asserts=True,
        num_devices=NCORES,
    )

    # ---- external I/O (per-core shards prepared on the host) ----
    xtb_d = nc.dram_tensor("xtb", [128, 16, S], BF16, kind="ExternalInput")
    xt8_d = nc.dram_tensor("xt8", [128, 16, S], FP8, kind="ExternalInput")
    wq_d = nc.dram_tensor("wqt", [HPC, 128, 16, KD], BF16, kind="ExternalInput")
    wk_d = nc.dram_tensor("wkt", [HPC, 128, 16, KD], BF16, kind="ExternalInput")
    wv_d = nc.dram_tensor("wvt", [HPC, 128, 16, E], FP8, kind="ExternalInput")
    w1_d = nc.dram_tensor("w1t", [128, 8, 2048], BF16, kind="ExternalInput")
    w2_d = nc.dram_tensor("w2t", [128, 8, 4, 512], BF16, kind="ExternalInput")
    bq_d = nc.dram_tensor("bqs", [128, HPC], F32, kind="ExternalInput")
    bk_d = nc.dram_tensor("bks", [128, HPC], F32, kind="ExternalInput")
    b1_d = nc.dram_tensor("b1s", [128, 8], F32, kind="ExternalInput")
    yb1_d = nc.dram_tensor("yb1", [128, E], F32, kind="ExternalInput")
    yb2_d = nc.dram_tensor("yb2", [128, E], F32, kind="ExternalInput")
    xr_d = nc.dram_tensor("xr", [RROWS, E], F32, kind="ExternalInput")
    lng_d = nc.dram_tensor("lngr", [RROWS, E], BF16, kind="ExternalInput")
    lnb_d = nc.dram_tensor("lnbr", [RROWS, E], BF16, kind="ExternalInput")
    lngT_d = nc.dram_tensor("lngT", [16, 128, RROWS], BF16, kind="ExternalInput")
    lnbT_d = nc.dram_tensor("lnbT", [16, 128, RROWS], BF16, kind="ExternalInput")
    mask_d = nc.dram_tensor("mask", [4, 128, 512], F32, kind="ExternalInput")
    id_d = nc.dram_tensor("ident", [128, 128], F32, kind="ExternalInput")
    ones_d = nc.dram_tensor("ones", [128, 8], F32, kind="ExternalInput")
    onesr_d = nc.dram_tensor("onesr", [1, 128], F32, kind="ExternalInput")
    out_d = nc.dram_tensor("out", [RROWS, E], F32, kind="ExternalOutput")

    RG = [list(range(NCORES))]

    with tile.TileContext(nc) as tc:
        with (
            tc.tile_pool(name="persist", bufs=1) as pp,
            tc.tile_pool(name="dram", bufs=1, space="DRAM") as dp,
            tc.tile_pool(name="ps512", bufs=4, space="PSUM") as ps512,
            tc.tile_pool(name="psT", bufs=2, space="PSUM") as psT,
            tc.tile_pool(name="psR", bufs=2, space="PSUM") as psR,
        ):
            # ---- collective bounce buffers (internal DRAM) ----
            att_in = [
                [
                    [
                        dp.tile([S, 512], F16, name=f"att_in_{h}_{fh}_{fb}", tag=f"ati{h}{fh}{fb}")
                        for fb in range(2)
                    ]
                    for fh in range(2)
                ]
                for h in range(HPC)
            ]
            att_out = [
                [
                    [
                        dp.tile([RROWS, 512], F16, name=f"att_out_{h}_{fh}_{fb}", tag=f"ato{h}{fh}{fb}")
                        for fb in range(2)
                    ]
                    for fh in range(2)
                ]
                for h in range(HPC)
            ]
            st1_in = dp.tile([1, 8], F32, name="st1_in", tag="st1i")
            st1_out = dp.tile([1, 8], F32, name="st1_out", tag="st1o", addr_space="Shared")
            st2_in = dp.tile([1, 8], F32, name="st2_in", tag="st2i")
            st2_out = dp.tile([1, 8], F32, name="st2_out", tag="st2o", addr_space="Shared")
            # transposed h AllGather, split by own-row halves so FFN1 can
            # start on the first half while the second gathers
            agt_in = [
                dp.tile([16, 128, 128], BF16, name=f"agt_in{j}", tag=f"agti{j}")
                for j in range(2)
            ]
            agt_out = [
                dp.tile([128, 128, 128], BF16, name=f"agt_out{j}", tag=f"agto{j}", addr_space="Shared")
                for j in range(2)
            ]
            # ffn output chunks: col offset/width
            EBW = [(0, 512), (512, 512), (1024, 512), (1536, 512)]
            ffn_in = [
                dp.tile([S, w], F16, name=f"ffn_in_{eb}", tag=f"ffi{eb}")
                for eb, (c0, w) in enumerate(EBW)
            ]
            ffn_out = [
                dp.tile([RROWS, w], F16, name=f"ffn_out_{eb}", tag=f"ffo{eb}")
                for eb, (c0, w) in enumerate(EBW)
            ]

            # ---- persistent small tiles ----
            ident = pp.tile([128, 128], F32, name="ident")
            nc.sync.dma_start(ident[:], id_d[:])
            hidb = pp.tile([128, 128], BF16, name="hidb")
            nc.vector.tensor_copy(out=hidb[:], in_=ident[:])
            onesc = pp.tile([128, 8], F32, name="onesc")
            nc.sync.dma_start(onesc[:], ones_d[:])
            onesr = pp.tile([1, 128], F32, name="onesr")
            nc.sync.dma_start(onesr[:], onesr_d[:])
            bq_sb = pp.tile([128, HPC], F32, name="bq_sb")
            nc.sync.dma_start(bq_sb[:], bq_d[:])
            bk_sb = pp.tile([128, HPC], F32, name="bk_sb")
            nc.sync.dma_start(bk_sb[:], bk_d[:])
            b1_sb = pp.tile([128, 8], F32, name="b1_sb")
            nc.sync.dma_start(b1_sb[:], b1_d[:])
            qkT = pp.tile([128, 2, HPC, S], BF16, name="qkT")  # [d, q/k, head, s]
            recips = pp.tile([128, HPC, 16], F32, name="recips")

            # =========== phase 0 + attention ===========
            with tc.tile_pool(name="attn", bufs=1) as ap_:
                xT8 = ap_.tile([128, 16, S], FP8, name="xT8")  # x^T fp8 for V
                maskf = ap_.tile([128, 4, 512], F32, name="maskf")  # (mask-1)*30
                shm = ap_.tile([128, 1], F32, name="shm")  # exp shift bias
                ones8 = ap_.tile([128, 2, 1], FP8, name="ones8")
                with tc.tile_pool(name="qkx", bufs=1) as qkx:
                    xT = qkx.tile([128, 16, S], BF16, name="xT")  # x^T bf16 for Q/K
                    wqk = [
                        [
                            qkx.tile([128, 16, KD], BF16, name=f"wqk{h}{qi}")
                            for qi in range(2)
                        ]
                        for h in range(HPC)
                    ]
                    with tc.tile_pool(name="prep", bufs=1) as prep, nc.named_scope("prep"):
                        # HAM warmup: back-to-back matmuls to unthrottle PE,
                        # issued while the first weight/x DMAs land
                        wtile = prep.tile([128, 512], BF16, name="wtile", tag="wtile", bufs=1)
                        nc.vector.memset(wtile[:], 0.0)
                        nc.vector.memset(shm[:], -SH)
                        nc.vector.memset(ones8[:], 1.0)
                        # interleave the first x^T chunks with the small
                        # weight DMAs so qkproj's first matmul gates on
                        # neither the bulk x transfer nor a queued weight
                        qkdmas = [
                            (wqk[h][qi], (wq_d if qi == 0 else wk_d)[h])
                            for h in range(HPC)
                            for qi in range(2)
                        ]
                        for eo in range(4):
                            nc.sync.dma_start(xT[:, eo, :], xtb_d[:, eo, :])
                            dst, src = qkdmas[eo]
                            nc.sync.dma_start(dst[:], src)
                        for _w in range(24):
                            pw = ps512.tile([128, 512], F32, name="pw", tag="p512")
                            nc.tensor.matmul(pw[:], wtile[:, :128], wtile[:], start=True, stop=True)
                        for eo in range(4, 16):
                            nc.sync.dma_start(xT[:, eo, :], xtb_d[:, eo, :])
                        nc.sync.dma_start(maskf[:], mask_d.ap().rearrange("j p s -> p j s"))
                        for eo in range(16):
                            nc.sync.dma_start(xT8[:, eo, :], xt8_d[:, eo, :])

                    # ---- Q/K projections for both heads (scaled/biased) ----
                    with nc.named_scope("qkproj"):
                        for h in range(HPC):
                            for qi, (b_sb, scl) in enumerate(
                                ((bq_sb, ISCALE), (bk_sb, 1.0))
                            ):
                                wb = wqk[h][qi]
                                pqs = [
                                    ps512.tile([128, 512], F32, name=f"pq{sb}", tag="p512")
                                    for sb in range(4)
                                ]
                                for eo in range(16):
                                    for sb in range(4):
                                        nc.tensor.matmul(
                                            pqs[sb][:],
                                            wb[:, eo, :],
                                            xT[:, eo, sb * 512 : (sb + 1) * 512],
                                            start=(eo == 0),
                                            stop=(eo == 15),
                                        )
                                for sb in range(4):
                                    nc.scalar.activation(
                                        qkT[:, qi, h, sb * 512 : (sb + 1) * 512],
                                        pqs[sb][:],
                                        AF.Identity,
                                        bias=b_sb[:, h : h + 1],
                                        scale=scl,
                                    )

                # ---- per-head attention (value path in fp8 DoubleRow) ----
                eT = ap_.tile([128, NTRI, 512], FP8, name="eT")
                v_sb = ap_.tile([128, 16, FSH], FP8, name="v_sb")
                with (
                    tc.tile_pool(name="wvb", bufs=2) as wvbp,
                    tc.tile_pool(name="astg", bufs=4) as astg,
                ):
                    for h in range(HPC):
                      with nc.named_scope(f"scores{h}"):
                        for sb in range(4):
                            for tcn in range(4 * sb + 4):
                                psc = ps512.tile([128, 512], F32, name="psc", tag="p512")
                                nc.tensor.matmul(
                                    psc[:],
                                    qkT[:, 1, h, tcn * 128 : (tcn + 1) * 128],
                                    qkT[:, 0, h, sb * 512 : (sb + 1) * 512],
                                    start=True,
                                    stop=True,
                                )
                                if tcn >= 4 * sb:
                                    # diagonal tile: -30 mask onto PSUM, then
                                    # masked entries underflow to 0 in the exp
                                    nc.vector.tensor_tensor(
                                        psc[:], psc[:], maskf[:, tcn - 4 * sb, :], AL.add
                                    )
                                nc.scalar.activation(
                                    eT[:, TRI[sb] + tcn, :], psc[:], AF.Exp, bias=shm[:]
                                )

                        # pass B: per f-half: v-projection then attn@v
                        for fh in range(2):
                          with nc.named_scope(f"vproj{h}{fh}"):
                            for fb in range(2):
                                wvb = wvbp.tile([128, 16, 512], FP8, name="wvb", tag="wvb")
                                nc.sync.dma_start(
                                    wvb[:],
                                    wv_d[h][
                                        :, :, fh * 1024 + fb * 512 : fh * 1024 + (fb + 1) * 512
                                    ],
                                )
                                for tcn in range(16):
                                    pv = ps512.tile([128, 512], F32, name="pv", tag="p512")
                                    for j in range(8):
                                        nc.tensor.matmul(
                                            pv[:],
                                            xT8[:, 2 * j : 2 * j + 2, tcn * 128 : (tcn + 1) * 128],
                                            wvb[:, 2 * j : 2 * j + 2, :],
                                            start=(j == 0),
                                            stop=(j == 7),
                                            perf_mode=DR,
                                        )
                                    if tcn % 2 == 0:
                                        nc.vector.tensor_copy(
                                            out=v_sb[:, tcn, fb * 512 : (fb + 1) * 512],
                                            in_=pv[:],
                                        )
                                    else:
                                        nc.scalar.copy(
                                            v_sb[:, tcn, fb * 512 : (fb + 1) * 512], pv[:]
                                        )

                          with nc.named_scope(f"attnv{h}{fh}"):
                            # fb-outer: each 512-col half ReduceScatters as
                            # soon as it is complete, halving the exposed
                            # tail after the last attnv block
                            for fb in range(2):
                                dorow = fh == 0 and fb == 0
                                for i in range(15, -1, -1):
                                    sb, so = i // 4, (i % 4) * 128
                                    npair = (i + 1) // 2
                                    odd = (i + 1) % 2
                                    pa = ps512.tile([128, 512], F32, name="pa", tag="p512")
                                    if dorow:
                                        pr = psR.tile([128, 1], F32, name="pr", tag="pr", bufs=1)
                                    for tp in range(npair):
                                        lhs = eT[:, TRI[sb] + 2 * tp : TRI[sb] + 2 * tp + 2, so : so + 128]
                                        last = (tp == npair - 1) and not odd
                                        nc.tensor.matmul(
                                            pa[:],
                                            lhs,
                                            v_sb[:, 2 * tp : 2 * tp + 2, fb * 512 : (fb + 1) * 512],
                                            start=(tp == 0),
                                            stop=last,
                                            perf_mode=DR,
                                        )
                                        if dorow:
                                            nc.tensor.matmul(
                                                pr[:],
                                                lhs,
                                                ones8[:],
                                                start=(tp == 0),
                                                stop=last,
                                                perf_mode=DR,
                                            )
                                    if odd:
                                        lhs1 = eT[:, TRI[sb] + i, so : so + 128]
                                        nc.tensor.matmul(
                                            pa[:],
                                            lhs1,
                                            v_sb[:, i, fb * 512 : (fb + 1) * 512],
                                            start=(npair == 0),
                                            stop=True,
                                        )
                                        if dorow:
                                            nc.tensor.matmul(
                                                pr[:],
                                                lhs1,
                                                ones8[:, 0, :],
                                                start=(npair == 0),
                                                stop=True,
                                            )
                                    if dorow:
                                        rsf = astg.tile([128, 1], F32, name="rsf", tag="rsf")
                                        nc.vector.tensor_copy(out=rsf[:], in_=pr[:])
                                        nc.vector.reciprocal(recips[:, h, i : i + 1], rsf[:])
                                    stg = astg.tile([128, 512], F16, name="stg", tag="stg")
                                    nc.scalar.activation(
                                        stg[:],
                                        pa[:],
                                        AF.Copy,
                                        scale=recips[:, h, i : i + 1],
                                    )
                                    nc.sync.dma_start(
                                        att_in[h][fh][fb][i * 128 : (i + 1) * 128, :], stg[:]
                                    )
                                nc.gpsimd.collective_compute(
                                    "ReduceScatter",
                                    AL.add,
                                    replica_groups=RG,
                                    ins=[att_in[h][fh][fb][:]],
                                    outs=[att_out[h][fh][fb][:]],
                                )

            # =========== LN1 (global mean/var) ===========
            with tc.tile_pool(name="mid", bufs=1) as midp:
              h_own = midp.tile([128, 2, E], BF16, name="h_own")
              ysT = midp.tile([128, 16, RROWS], BF16, name="ysT")
              lngt = midp.tile([128, 2, E], BF16, name="lngt")
              lnbt = midp.tile([128, 2, E], BF16, name="lnbt")
              with tc.tile_pool(name="ln1", bufs=1) as lp, nc.named_scope("ln1"):
                  ys = lp.tile([128, 2, E], F32, name="ys")
                  ysb = lp.tile([128, 2, E], BF16, name="ysb")
                  yb1t = lp.tile([128, E], F32, name="yb1t")
                  nc.sync.dma_start(yb1t[:], yb1_d[:])
                  nc.sync.dma_start(lngt[:], lng_d.ap().rearrange("(t p) e -> p t e", p=128))
                  nc.sync.dma_start(lnbt[:], lnb_d.ap().rearrange("(t p) e -> p t e", p=128))
                  lngT = lp.tile([128, 16, RROWS], BF16, name="lngT")
                  nc.sync.dma_start(lngT[:], lngT_d.ap().rearrange("eo p r -> p eo r"))
                  lnbT = lp.tile([128, 16, RROWS], BF16, name="lnbT")
                  nc.sync.dma_start(lnbT[:], lnbT_d.ap().rearrange("eo p r -> p eo r"))
                  for rt in range(2):
                      xrt = lp.tile([128, E], F32, name="xrt", tag="xrt", bufs=2)
                      nc.sync.dma_start(xrt[:], xr_d[rt * 128 : (rt + 1) * 128, :])
                      nc.vector.tensor_tensor(ys[:, rt, :], xrt[:], yb1t[:], AL.add)
                  # accumulate RS chunks as they land; the last head's chunk
                  # completes a column block -> emit its stats partial
                  parts = lp.tile([128, 16], F32, name="parts")
                  sqs = lp.tile([128, 512], BF16, name="sqs", tag="sqs", bufs=2)
                  for h in range(HPC):
                      for fh in range(2):
                          for fb in range(2):
                              for rt in range(2):
                                  rof = lp.tile([128, 512], F16, name="rof", tag="rof", bufs=4)
                                  nc.sync.dma_start(
                                      rof[:], att_out[h][fh][fb][rt * 128 : (rt + 1) * 128, :]
                                  )
                                  col = fh * 1024 + fb * 512
                                  dstv = ys[:, rt, col : col + 512]
                                  nc.vector.tensor_tensor(dstv, dstv, rof[:], AL.add)
                                  if h == HPC - 1:
                                      blk = (fh * 2 + fb) * 2 + rt
                                      nc.vector.tensor_reduce(
                                          parts[:, blk : blk + 1], dstv, axis=AX.X, op=AL.add
                                      )
                                      nc.scalar.activation(
                                          sqs[:], dstv, AF.Square,
                                          accum_out=parts[:, 8 + blk : 9 + blk],
                                      )
                  pstat = psT.tile([128, 128], F32, name="pstat", tag="pt", bufs=1)
                  nc.tensor.matmul(pstat[:1, :16], onesc[:, 0:1], parts[:], start=True, stop=True)
                  st4s = lp.tile([1, 16], F32, name="st4s")
                  nc.vector.tensor_copy(out=st4s[:], in_=pstat[:1, :16])
                  st4 = lp.tile([1, 8], F32, name="st4")
                  nc.vector.memset(st4[:], 0.0)
                  nc.vector.tensor_reduce(st4[:, 0:1], st4s[:, 0:8], axis=AX.X, op=AL.add)
                  nc.vector.tensor_reduce(st4[:, 1:2], st4s[:, 8:16], axis=AX.X, op=AL.add)
                  nc.sync.dma_start(st1_in[:], st4[:])
                  nc.gpsimd.collective_compute(
                      "AllReduce", AL.add, replica_groups=RG, ins=[st1_in[:]], outs=[st1_out[:]]
                  )
                  # transpose own rows (pre-affine) while the AllReduce flies
                  for rt in range(2):
                      nc.vector.tensor_copy(out=ysb[:, rt, :], in_=ys[:, rt, :])
                  for eo in range(16):
                      for rt in range(2):
                          pth = psT.tile([128, 128], BF16, name="pth", tag="ptT")
                          nc.tensor.transpose(
                              pth[:], ysb[:, rt, eo * 128 : (eo + 1) * 128], hidb[:]
                          )
                          if (eo + rt) % 2 == 0:
                              nc.vector.tensor_copy(
                                  out=ysT[:, eo, rt * 128 : (rt + 1) * 128], in_=pth[:]
                              )
                          else:
                              nc.scalar.copy(ysT[:, eo, rt * 128 : (rt + 1) * 128], pth[:])
                  bc = _ln_scalars(nc, lp, psT, onesr, st1_out)
                  # LN affine in transposed space, AllGather per own-row half
                  for hf in range(2):
                      src = ysT[:, :, hf * 128 : (hf + 1) * 128]
                      stT = lp.tile([128, 16, 128], BF16, name="hstgT", tag="hstgT", bufs=2)
                      nc.scalar.activation(
                          stT[:], src, AF.Identity, bias=bc[:, 0:1], scale=bc[:, 1:2]
                      )
                      nc.vector.tensor_tensor(
                          stT[:], stT[:], lngT[:, :, hf * 128 : (hf + 1) * 128], AL.mult
                      )
                      nc.vector.tensor_tensor(
                          stT[:], stT[:], lnbT[:, :, hf * 128 : (hf + 1) * 128], AL.add
                      )
                      nc.sync.dma_start(
                          agt_in[hf].rearrange("eo p r -> p eo r"), stT[:]
                      )
                      nc.gpsimd.collective_compute(
                          "AllGather",
                          AL.bypass,
                          replica_groups=RG,
                          ins=[agt_in[hf][:]],
                          outs=[agt_out[hf][:]],
                      )
                  # row-major h_own for the LN2 residual (off critical path)
                  ht_f32 = lp.tile([128, E], F32, name="ht_f32", tag="htf", bufs=2)
                  for rt in range(2):
                      nc.scalar.activation(
                          ht_f32[:], ys[:, rt, :], AF.Identity,
                          bias=bc[:, 0:1], scale=bc[:, 1:2],
                      )
                      nc.vector.tensor_tensor(ht_f32[:], ht_f32[:], lngt[:, rt, :], AL.mult)
                      nc.vector.tensor_tensor(h_own[:, rt, :], ht_f32[:], lnbt[:, rt, :], AL.add)

              # =========== FFN (hidden shard 1024) ===========
              # hT's S axis is PERMUTED: col = hf*1024 + c*128 + r maps to
              # true row c*256 + hf*128 + r. FFN1/FFN2 are elementwise along
              # S so only the final staging DMA needs to undo the mapping.
              with tc.tile_pool(name="ffn", bufs=1) as fp, nc.named_scope("ffn"):
                  hT = fp.tile([128, 16, S], BF16, name="hT")
                  zT = fp.tile([128, 8, S], BF16, name="zT")
                  w1full = fp.tile([128, 8, 2048], BF16, name="w1full")
                  for ft in range(8):
                      nc.sync.dma_start(w1full[:, ft, :], w1_d[:, ft, :])
                  with tc.tile_pool(name="wst", bufs=2) as wst:
                      for hf in range(2):
                          for c in range(NCORES):
                              nc.sync.dma_start(
                                  hT[:, :, hf * 1024 + c * 128 : hf * 1024 + (c + 1) * 128],
                                  agt_out[hf][c * 16 : (c + 1) * 16].rearrange("eo p r -> p eo r"),
                              )
                      # FFN1 per gathered S-half: starts on half 0 while
                      # half 1 is still gathering
                      for hf in range(2):
                          for ft in range(8):
                              pzs = [
                                  ps512.tile([128, 512], F32, name=f"pz{sb}", tag="p512")
                                  for sb in range(2)
                              ]
                              for eo in range(16):
                                  for sb in range(2):
                                      nc.tensor.matmul(
                                          pzs[sb][:],
                                          w1full[:, ft, eo * KD : (eo + 1) * KD],
                                          hT[:, eo, hf * 1024 + sb * 512 : hf * 1024 + (sb + 1) * 512],
                                          start=(eo == 0),
                                          stop=(eo == 15),
                                      )
                              for sb in range(2):
                                  nc.scalar.activation(
                                      zT[:, ft, hf * 1024 + sb * 512 : hf * 1024 + (sb + 1) * 512],
                                      pzs[sb][:],
                                      AF.Relu,
                                      bias=b1_sb[:, ft : ft + 1],
                                  )
                      for ebx, (c0, w) in enumerate(EBW):
                          w2b = wst.tile([128, 8, w], BF16, name="w2b", tag="w2b")
                          nc.sync.dma_start(w2b[:], w2_d[:, :, c0 // 512, :])
                          for p in range(16):
                              pf = ps512.tile([128, w], F32, name="pf", tag="p512")
                              for fc in range(8):
                                  nc.tensor.matmul(
                                      pf[:],
                                      zT[:, fc, p * 128 : (p + 1) * 128],
                                      w2b[:, fc, :],
                                      start=(fc == 0),
                                      stop=(fc == 7),
                                  )
                              fstg = wst.tile([128, w], F16, name="fstg", tag="fstg", bufs=4)
                              nc.scalar.activation(fstg[:], pf[:], AF.Copy)
                              hf, cc = p // 8, p % 8
                              r0 = cc * 256 + hf * 128
                              nc.sync.dma_start(ffn_in[ebx][r0 : r0 + 128, :], fstg[:])
                          nc.gpsimd.collective_compute(
                              "ReduceScatter",
                              AL.add,
                              replica_groups=RG,
                              ins=[ffn_in[ebx][:]],
                              outs=[ffn_out[ebx][:]],
                          )

              # =========== LN2 + output ===========
              with tc.tile_pool(name="ln2", bufs=1) as l2, nc.named_scope("ln2"):
                  ys2 = l2.tile([128, 2, E], F32, name="ys2")
                  yb2t = l2.tile([128, E], F32, name="yb2t")
                  nc.sync.dma_start(yb2t[:], yb2_d[:])
                  for rt in range(2):
                      nc.vector.tensor_tensor(
                          ys2[:, rt, :], h_own[:, rt, :], yb2t[:], AL.add
                      )
                  parts2 = l2.tile([128, 16], F32, name="parts2")
                  sqs2 = l2.tile([128, 512], BF16, name="sqs2", tag="sqs2", bufs=2)
                  for ebx, (c0, w) in enumerate(EBW):
                      for rt in range(2):
                          fot = l2.tile([128, w], F16, name="fot", tag="fot", bufs=4)
                          nc.sync.dma_start(fot[:], ffn_out[ebx][rt * 128 : (rt + 1) * 128, :])
                          dstv = ys2[:, rt, c0 : c0 + w]
                          nc.vector.tensor_tensor(dstv, dstv, fot[:], AL.add)
                          blk = ebx * 2 + rt
                          nc.vector.tensor_reduce(
                              parts2[:, blk : blk + 1], dstv, axis=AX.X, op=AL.add
                          )
                          nc.scalar.activation(
                              sqs2[:, 0:w], dstv, AF.Square,
                              accum_out=parts2[:, 8 + blk : 9 + blk],
                          )
                  pstat2 = psT.tile([128, 128], F32, name="pstat2", tag="pt", bufs=1)
                  nc.tensor.matmul(pstat2[:1, :16], onesc[:, 0:1], parts2[:], start=True, stop=True)
                  st4s2 = l2.tile([1, 16], F32, name="st4s2")
                  nc.vector.tensor_copy(out=st4s2[:], in_=pstat2[:1, :16])
                  st42 = l2.tile([1, 8], F32, name="st42")
                  nc.vector.memset(st42[:], 0.0)
                  nc.vector.tensor_reduce(st42[:, 0:1], st4s2[:, 0:8], axis=AX.X, op=AL.add)
                  nc.vector.tensor_reduce(st42[:, 1:2], st4s2[:, 8:16], axis=AX.X, op=AL.add)
                  nc.sync.dma_start(st2_in[:], st42[:])
                  nc.gpsimd.collective_compute(
                      "AllReduce", AL.add, replica_groups=RG, ins=[st2_in[:]], outs=[st2_out[:]]
                  )
                  bc2 = _ln_scalars(nc, l2, psT, onesr, st2_out)
                  for rt in range(2):
                      ot = l2.tile([128, E], F32, name="ot", tag="ot", bufs=2)
                      nc.scalar.activation(
                          ot[:],
                          ys2[:, rt, :],
                          AF.Identity,
                          bias=bc2[:, 0:1],
                          scale=bc2[:, 1:2],
                      )
                      nc.vector.tensor_tensor(ot[:], ot[:], lngt[:, rt, :], AL.mult)
                      nc.vector.tensor_tensor(ot[:], ot[:], lnbt[:, rt, :], AL.add)
                      nc.sync.dma_start(out_d[rt * 128 : (rt + 1) * 128, :], ot[:])

    nc.compile()
    return nc


def _ln_scalars(nc, pool, psT, onesr, st_out):
    """AllReduced (sum, sumsq) -> bc [128, 2] = (-m*rstd, rstd) broadcast."""
    so = pool.tile([1, 8], F32, name="so", tag="so")
    nc.sync.dma_start(so[:], st_out[:])
    sc = pool.tile([1, 8], F32, name="sc", tag="sc")
    # sc0 = m, sc1 = E[y^2], sc2 = m^2, sc3 = var, sc4 = rstd, sc5 = -m*rstd
    nc.scalar.mul(sc[:, 0:1], so[:, 0:1], 1.0 / NTOT)
    nc.scalar.mul(sc[:, 1:2], so[:, 1:2], 1.0 / NTOT)
    nc.scalar.activation(sc[:, 2:3], sc[:, 0:1], AF.Square)
    nc.vector.tensor_tensor(sc[:, 3:4], sc[:, 1:2], sc[:, 2:3], AL.subtract)
    nc.vector.tensor_scalar_add(sc[:, 2:3], sc[:, 3:4], EPS)  # var + eps
    # rstd = exp(-0.5 * ln(var + eps)) (keeps ACT on the exp/ln table)
    nc.scalar.activation(sc[:, 6:7], sc[:, 2:3], AF.Ln)
    nc.scalar.activation(sc[:, 4:5], sc[:, 6:7], AF.Exp, scale=-0.5)
    nc.vector.tensor_tensor(sc[:, 7:8], sc[:, 0:1], sc[:, 4:5], AL.mult)
    nc.scalar.mul(sc[:, 5:6], sc[:, 7:8], -1.0)
    s2 = pool.tile([1, 2], F32, name="s2", tag="s2")
    nc.vector.tensor_copy(out=s2[:, 0:1], in_=sc[:, 5:6])
    nc.vector.tensor_copy(out=s2[:, 1:2], in_=sc[:, 4:5])
    pb = psT.tile([128, 128], F32, name="pb", tag="pt", bufs=1)
    nc.tensor.matmul(pb[:, :2], onesr[:], s2[:], start=True, stop=True)
    bc = pool.tile([128, 2], F32, name="bc", tag="bc")
    nc.vector.tensor_copy(out=bc[:], in_=pb[:, :2])
    return bc


_NC_CACHE = None


def _get_nc():
    global _NC_CACHE
    if _NC_CACHE is None:
        _NC_CACHE = _build()
    return _NC_CACHE


def _prep_core(c, inputs):
    f32 = np.float32
    x = np.ascontiguousarray(inputs["input"], dtype=f32)
    Wq, Wk, Wv = inputs["Wq"], inputs["Wk"], inputs["Wv"]
    bq, bk, bv = inputs["bq"], inputs["bk"], inputs["bv"]
    W1, b1, W2, b2 = inputs["W1"], inputs["b1"], inputs["W2"], inputs["b2"]
    ln_g, ln_b = inputs["ln_g"], inputs["ln_b"]
    h0 = c * HPC
    wqt = np.ascontiguousarray(
        np.stack(
            [Wq[h0 + h].reshape(16, 128, KD).transpose(1, 0, 2) for h in range(HPC)]
        ).astype(_bf16)
    )
    wkt = np.ascontiguousarray(
        np.stack(
            [Wk[h0 + h].reshape(16, 128, KD).transpose(1, 0, 2) for h in range(HPC)]
        ).astype(_bf16)
    )
    wvt = np.ascontiguousarray(
        np.stack(
            [Wv[h0 + h].reshape(16, 128, E).transpose(1, 0, 2) for h in range(HPC)]
        ).astype(_f8)
    )
    W1s = W1[:, c * FSH : (c + 1) * FSH]
    w1t = np.ascontiguousarray(
        W1s.reshape(16, 128, 8, 128).transpose(1, 2, 0, 3).reshape(128, 8, 2048).astype(_bf16)
    )
    W2s = W2[c * FSH : (c + 1) * FSH, :]
    w2t = np.ascontiguousarray(
        W2s.reshape(8, 128, 4, 512).transpose(1, 0, 2, 3).astype(_bf16)
    )
    bqs = np.ascontiguousarray((bq[h0 : h0 + HPC] * ISCALE).T, dtype=f32)
    bks = np.ascontiguousarray(bk[h0 : h0 + HPC].T, dtype=f32)
    b1s = np.ascontiguousarray(b1[c * FSH : (c + 1) * FSH].reshape(8, 128).T, dtype=f32)
    yb1 = np.ascontiguousarray(np.broadcast_to(bv.sum(axis=0), (128, E)), dtype=f32)
    yb2 = np.ascontiguousarray(np.broadcast_to(b2, (128, E)), dtype=f32)
    rows = slice(c * RROWS, (c + 1) * RROWS)
    jj, tp, sf = np.meshgrid(
        np.arange(4), np.arange(128), np.arange(512), indexing="ij"
    )
    mask = (((128 * jj + tp) <= sf).astype(f32) - 1.0) * 30.0
    xt = x.T.reshape(16, 128, S).transpose(1, 0, 2)
    xtb = np.ascontiguousarray(xt.astype(_bf16))
    xt8 = np.ascontiguousarray(xt.astype(_f8))
    return {
        "xtb": xtb,
        "xt8": xt8,
        "wqt": wqt,
        "wkt": wkt,
        "wvt": wvt,
        "w1t": w1t,
        "w2t": w2t,
        "bqs": bqs,
        "bks": bks,
        "b1s": b1s,
        "yb1": yb1,
        "yb2": yb2,
        "xr": np.ascontiguousarray(x[rows], dtype=f32),
        "lngr": np.ascontiguousarray(np.asarray(ln_g[rows], dtype=f32).astype(_bf16)),
        "lnbr": np.ascontiguousarray(np.asarray(ln_b[rows], dtype=f32).astype(_bf16)),
        "lngT": np.ascontiguousarray(
            np.asarray(ln_g[rows], dtype=f32).T.reshape(16, 128, RROWS).astype(_bf16)
        ),
        "lnbT": np.ascontiguousarray(
            np.asarray(ln_b[rows], dtype=f32).T.reshape(16, 128, RROWS).astype(_bf16)
        ),
        "mask": np.ascontiguousarray(mask),
        "ident": np.eye(128, dtype=f32),
        "ones": np.ones((128, 8), dtype=f32),
        "onesr": np.ones((1, 128), dtype=f32),
    }


def kernel(**inputs):
    nc = _get_nc()
    inputs = {k: np.asarray(v, dtype=np.float32) for k, v in inputs.items()}
    in_maps = [_prep_core(c, inputs) for c in range(NCORES)]
    res = run_bass_kernel_spmd(nc, in_maps, core_ids=list(range(NCORES)))
    out = np.concatenate([res.results[c]["out"] for c in range(NCORES)], axis=0)
    return np.ascontiguousarray(out, dtype=np.float32)



# revision 33
# speedup vs baseline: 1.4149x; 1.0368x over previous
"""Tensor-parallel decoder layer on 8 TRN2 NeuronCores.

Sharding:
  - Attention: 16 heads -> 2 per core. Per-core partial attn_out is
    ReduceScattered (fp16) so core c owns rows [256c, 256c+256).
  - Global LayerNorm (scalar mean/var over the whole [S,E] tensor):
    per-core partial (sum, sumsq) AllReduced as a tiny fp32 tensor.
  - FFN: hidden dim 8192 -> 1024 per core; partial [S,E] output
    ReduceScattered per 512-column chunk (fp16), overlapping FFN2.
  - h is AllGathered transposed (bf16) since every matmul contracting
    over E needs h^T as the moving operand.

Matmul layout notes (PE computes out = lhsT.T @ rhs, contraction on the
partition dim):
  - x^T resident in SBUF (bf16 for Q/K, fp8e4 for V) feeds projections.
  - The value path runs in fp8e4 DoubleRow (2 k-tiles per instruction,
    2x MAC rate): V projection contracts eo pairs, attn@v contracts
    t-tile pairs. exp is shifted by a constant (exp(s - SH)) so scores
    fit fp8's +/-240 range; the shift cancels exactly in num/rowsum.
  - scores are built transposed: S^T[t,s] tiles, so exp(S^T) tiles feed
    attn@v directly as lhsT with no transposes; softmax normalization is
    deferred: rowsum via a fp8 ones-column matmul, applied as a
    per-partition scale on the PSUM->SBUF copy of attn@v output.
  - causal masking folds into the score: a -30 additive mask lands on
    the PSUM tile before the exp, so masked entries underflow to 0.
"""

import math
import sys

sys.path.insert(0, "/opt/trn_rl_repo")

import numpy as np
import ml_dtypes

_bf16 = ml_dtypes.bfloat16
_f8 = ml_dtypes.float8_e4m3

import concourse.bass as bass
import concourse.mybir as mybir
import concourse.tile as tile
from concourse import bacc
from concourse.bass_utils import run_bass_kernel_spmd

S, E, H, KD, FF = 2048, 2048, 16, 128, 8192
EPS = 1e-5
NCORES = 8
HPC = H // NCORES          # heads per core = 2
FSH = FF // NCORES         # ffn hidden shard = 1024
RROWS = S // NCORES        # row shard = 256
NTOT = float(S * E)
ISCALE = 1.0 / math.sqrt(KD)

F32 = mybir.dt.float32
BF16 = mybir.dt.bfloat16
F16 = mybir.dt.float16
FP8 = mybir.dt.float8e4
AF = mybir.ActivationFunctionType
AL = mybir.AluOpType
AX = mybir.AxisListType
DR = mybir.MatmulPerfMode.DoubleRow

# packed triangular offsets for eT tiles: tile(tc, sb) at TRI[sb] + tc
TRI = [0, 4, 12, 24]
NTRI = 40
SH = 2.5  # constant exp shift: keeps exp(score - SH) within fp8e4 range


def _build():
    nc = bacc.Bacc(
        "TRN2",
        target_bir_lowering=False,
        debug=False,
        enable_asserts=True,
        num_devices=NCORES,
    )

    # ---- external I/O (per-core shards prepared on the host) ----
    xtb_d = nc.dram_tensor("xtb", [128, 16, S], BF16, kind="ExternalInput")
    xt8_d = nc.dram_tensor("xt8", [128, 16, S], FP8, kind="ExternalInput")
    wq_d = nc.dram_tensor("wqt", [HPC, 128, 16, KD], BF16, kind="ExternalInput")
    wk_d = nc.dram_tensor("wkt", [HPC, 128, 16, KD], BF16, kind="ExternalInput")
    wv_d = nc.dram_tensor("wvt", [HPC, 128, 16, E], FP8, kind="ExternalInput")
    w1_d = nc.dram_tensor("w1t", [128, 8, 2048], BF16, kind="ExternalInput")
    w2_d = nc.dram_tensor("w2t", [128, 8, 4, 512], BF16, kind="ExternalInput")
    bq_d = nc.dram_tensor("bqs", [128, HPC], F32, kind="ExternalInput")
    bk_d = nc.dram_tensor("bks", [128, HPC], F32, kind="ExternalInput")
    b1_d = nc.dram_tensor("b1s", [128, 8], F32, kind="ExternalInput")
    yb1_d = nc.dram_tensor("yb1", [128, E], F32, kind="ExternalInput")
    yb2_d = nc.dram_tensor("yb2", [128, E], F32, kind="ExternalInput")
    xr_d = nc.dram_tensor("xr", [RROWS, E], F32, kind="ExternalInput")
    lng_d = nc.dram_tensor("lngr", [RROWS, E], BF16, kind="ExternalInput")
    lnb_d = nc.dram_tensor("lnbr", [RROWS, E], BF16, kind="ExternalInput")
    lngT_d = nc.dram_tensor("lngT", [16, 128, RROWS], BF16, kind="ExternalInput")
    lnbT_d = nc.dram_tensor("lnbT", [16, 128, RROWS], BF16, kind="ExternalInput")
    mask_d = nc.dram_tensor("mask", [4, 128, 512], F32, kind="ExternalInput")
    id_d = nc.dram_tensor("ident", [128, 128], F32, kind="ExternalInput")
    ones_d = nc.dram_tensor("ones", [128, 8], F32, kind="ExternalInput")
    onesr_d = nc.dram_tensor("onesr", [1, 128], F32, kind="ExternalInput")
    out_d = nc.dram_tensor("out", [RROWS, E], F32, kind="ExternalOutput")

    RG = [list(range(NCORES))]

    with tile.TileContext(nc) as tc:
        with (
            tc.tile_pool(name="persist", bufs=1) as pp,
            tc.tile_pool(name="dram", bufs=1, space="DRAM") as dp,
            tc.tile_pool(name="ps512", bufs=5, space="PSUM") as ps512,
            tc.tile_pool(name="psT", bufs=1, space="PSUM") as psT,
            tc.tile_pool(name="psR", bufs=1, space="PSUM") as psR,
        ):
            # ---- collective bounce buffers (internal DRAM) ----
            att_in = [
                [
                    [
                        dp.tile([S, 512], F16, name=f"att_in_{h}_{fh}_{fb}", tag=f"ati{h}{fh}{fb}")
                        for fb in range(2)
                    ]
                    for fh in range(2)
                ]
                for h in range(HPC)
            ]
            att_out = [
                [
                    [
                        dp.tile([RROWS, 512], F16, name=f"att_out_{h}_{fh}_{fb}", tag=f"ato{h}{fh}{fb}")
                        for fb in range(2)
                    ]
                    for fh in range(2)
                ]
                for h in range(HPC)
            ]
            # the final attention block's RS is row-split: even/odd 128-row
            # chunks scatter to (own-rows first/second half), so two small
            # collectives replace one big one on the critical tail
            atts_in = [
                [
                    dp.tile([S // 2, 512], F16, name=f"atts_in_{fb}_{ab}", tag=f"atsi{fb}{ab}")
                    for ab in range(2)
                ]
                for fb in range(2)
            ]
            atts_out = [
                [
                    dp.tile([128, 512], F16, name=f"atts_out_{fb}_{ab}", tag=f"atso{fb}{ab}")
                    for ab in range(2)
                ]
                for fb in range(2)
            ]
            st1_in = dp.tile([1, 8], F32, name="st1_in", tag="st1i")
            st1_out = dp.tile([1, 8], F32, name="st1_out", tag="st1o", addr_space="Shared")
            st2_in = dp.tile([1, 8], F32, name="st2_in", tag="st2i")
            st2_out = dp.tile([1, 8], F32, name="st2_out", tag="st2o", addr_space="Shared")
            # transposed h AllGather, split by own-row halves so FFN1 can
            # start on the first half while the second gathers
            agt_in = [
                dp.tile([16, 128, 128], BF16, name=f"agt_in{j}", tag=f"agti{j}")
                for j in range(2)
            ]
            agt_out = [
                dp.tile([128, 128, 128], BF16, name=f"agt_out{j}", tag=f"agto{j}", addr_space="Shared")
                for j in range(2)
            ]
            # ffn output chunks: col offset/width
            EBW = [(0, 512), (512, 512), (1024, 512), (1536, 512)]
            ffn_in = [
                dp.tile([S, w], F16, name=f"ffn_in_{eb}", tag=f"ffi{eb}")
                for eb, (c0, w) in enumerate(EBW)
            ]
            ffn_out = [
                dp.tile([RROWS, w], F16, name=f"ffn_out_{eb}", tag=f"ffo{eb}")
                for eb, (c0, w) in enumerate(EBW)
            ]

            # ---- persistent small tiles ----
            ident = pp.tile([128, 128], F32, name="ident")
            nc.sync.dma_start(ident[:], id_d[:])
            hidb = pp.tile([128, 128], BF16, name="hidb")
            nc.vector.tensor_copy(out=hidb[:], in_=ident[:])
            onesc = pp.tile([128, 8], F32, name="onesc")
            nc.sync.dma_start(onesc[:], ones_d[:])
            onesr = pp.tile([1, 128], F32, name="onesr")
            nc.sync.dma_start(onesr[:], onesr_d[:])
            bq_sb = pp.tile([128, HPC], F32, name="bq_sb")
            nc.sync.dma_start(bq_sb[:], bq_d[:])
            bk_sb = pp.tile([128, HPC], F32, name="bk_sb")
            nc.sync.dma_start(bk_sb[:], bk_d[:])
            b1_sb = pp.tile([128, 8], F32, name="b1_sb")
            nc.sync.dma_start(b1_sb[:], b1_d[:])
            qkT = pp.tile([128, 2, HPC, S], BF16, name="qkT")  # [d, q/k, head, s]
            recips = pp.tile([128, HPC, 16], F32, name="recips")

            # =========== phase 0 + attention ===========
            with tc.tile_pool(name="attn", bufs=1) as ap_:
                xT8 = ap_.tile([128, 16, S], FP8, name="xT8")  # x^T fp8 for V
                maskf = ap_.tile([128, 4, 512], F32, name="maskf")  # (mask-1)*30
                shm = ap_.tile([128, 1], F32, name="shm")  # exp shift bias
                ones8 = ap_.tile([128, 2, 1], FP8, name="ones8")
                with tc.tile_pool(name="qkx", bufs=1) as qkx:
                    xT = qkx.tile([128, 16, S], BF16, name="xT")  # x^T bf16 for Q/K
                    wqk = [
                        [
                            qkx.tile([128, 16, KD], BF16, name=f"wqk{h}{qi}")
                            for qi in range(2)
                        ]
                        for h in range(HPC)
                    ]
                    with tc.tile_pool(name="prep", bufs=1) as prep, nc.named_scope("prep"):
                        # HAM warmup: back-to-back matmuls to unthrottle PE,
                        # issued while the first weight/x DMAs land
                        wtile = prep.tile([128, 512], BF16, name="wtile", tag="wtile", bufs=1)
                        nc.vector.memset(wtile[:], 0.0)
                        nc.vector.memset(shm[:], -SH)
                        nc.vector.memset(ones8[:], 1.0)
                        # interleave the first x^T chunks with the small
                        # weight DMAs so qkproj's first matmul gates on
                        # neither the bulk x transfer nor a queued weight
                        qkdmas = [
                            (wqk[h][qi], (wq_d if qi == 0 else wk_d)[h])
                            for h in range(HPC)
                            for qi in range(2)
                        ]
                        for eo in range(4):
                            nc.sync.dma_start(xT[:, eo, :], xtb_d[:, eo, :])
                            dst, src = qkdmas[eo]
                            nc.sync.dma_start(dst[:], src)
                        for _w in range(24):
                            pw = ps512.tile([128, 512], F32, name="pw", tag="p512")
                            nc.tensor.matmul(pw[:], wtile[:, :128], wtile[:], start=True, stop=True)
                        for eo in range(4, 16):
                            nc.sync.dma_start(xT[:, eo, :], xtb_d[:, eo, :])
                        nc.sync.dma_start(maskf[:], mask_d.ap().rearrange("j p s -> p j s"))
                        for eo in range(16):
                            nc.sync.dma_start(xT8[:, eo, :], xt8_d[:, eo, :])

                    # ---- Q/K projections for both heads (scaled/biased) ----
                    with nc.named_scope("qkproj"):
                        for h in range(HPC):
                            for qi, (b_sb, scl) in enumerate(
                                ((bq_sb, ISCALE), (bk_sb, 1.0))
                            ):
                                wb = wqk[h][qi]
                                pqs = [
                                    ps512.tile([128, 512], F32, name=f"pq{sb}", tag="p512")
                                    for sb in range(4)
                                ]
                                for eo in range(16):
                                    for sb in range(4):
                                        nc.tensor.matmul(
                                            pqs[sb][:],
                                            wb[:, eo, :],
                                            xT[:, eo, sb * 512 : (sb + 1) * 512],
                                            start=(eo == 0),
                                            stop=(eo == 15),
                                        )
                                for sb in range(4):
                                    nc.scalar.activation(
                                        qkT[:, qi, h, sb * 512 : (sb + 1) * 512],
                                        pqs[sb][:],
                                        AF.Identity,
                                        bias=b_sb[:, h : h + 1],
                                        scale=scl,
                                    )

                # ---- per-head attention (value path in fp8 DoubleRow) ----
                eT = ap_.tile([128, NTRI, 512], FP8, name="eT")
                v_sb = ap_.tile([128, 16, FSH], FP8, name="v_sb")
                with (
                    tc.tile_pool(name="wvb", bufs=2) as wvbp,
                    tc.tile_pool(name="astg", bufs=4) as astg,
                ):
                    for h in range(HPC):
                      with nc.named_scope(f"scores{h}"):
                        for sb in range(4):
                            for tcn in range(4 * sb + 4):
                                psc = ps512.tile([128, 512], F32, name="psc", tag="p512")
                                nc.tensor.matmul(
                                    psc[:],
                                    qkT[:, 1, h, tcn * 128 : (tcn + 1) * 128],
                                    qkT[:, 0, h, sb * 512 : (sb + 1) * 512],
                                    start=True,
                                    stop=True,
                                )
                                if tcn >= 4 * sb:
                                    # diagonal tile: -30 mask onto PSUM, then
                                    # masked entries underflow to 0 in the exp
                                    nc.vector.tensor_tensor(
                                        psc[:], psc[:], maskf[:, tcn - 4 * sb, :], AL.add
                                    )
                                nc.scalar.activation(
                                    eT[:, TRI[sb] + tcn, :], psc[:], AF.Exp, bias=shm[:]
                                )

                        # pass B: per f-half: v-projection then attn@v
                        for fh in range(2):
                          with nc.named_scope(f"vproj{h}{fh}"):
                            for fb in range(2):
                                wvb = wvbp.tile([128, 16, 512], FP8, name="wvb", tag="wvb")
                                nc.sync.dma_start(
                                    wvb[:],
                                    wv_d[h][
                                        :, :, fh * 1024 + fb * 512 : fh * 1024 + (fb + 1) * 512
                                    ],
                                )
                                for tcn in range(16):
                                    pv = ps512.tile([128, 512], F32, name="pv", tag="p512")
                                    for j in range(8):
                                        nc.tensor.matmul(
                                            pv[:],
                                            xT8[:, 2 * j : 2 * j + 2, tcn * 128 : (tcn + 1) * 128],
                                            wvb[:, 2 * j : 2 * j + 2, :],
                                            start=(j == 0),
                                            stop=(j == 7),
                                            perf_mode=DR,
                                        )
                                    if tcn % 2 == 0:
                                        nc.vector.tensor_copy(
                                            out=v_sb[:, tcn, fb * 512 : (fb + 1) * 512],
                                            in_=pv[:],
                                        )
                                    else:
                                        nc.scalar.copy(
                                            v_sb[:, tcn, fb * 512 : (fb + 1) * 512], pv[:]
                                        )

                          with nc.named_scope(f"attnv{h}{fh}"):
                            # fb-outer: each 512-col half ReduceScatters as
                            # soon as it is complete, halving the exposed
                            # tail after the last attnv block
                            lastblk = h == HPC - 1 and fh == 1
                            for fb in range(2):
                                dorow = fh == 0 and fb == 0
                                # final block: odd row-chunks first, then
                                # even, so the odd-half RS streams out early
                                iorder = (
                                    list(range(15, -1, -2)) + list(range(14, -1, -2))
                                    if lastblk
                                    else list(range(15, -1, -1))
                                )
                                for i in iorder:
                                    sb, so = i // 4, (i % 4) * 128
                                    npair = (i + 1) // 2
                                    odd = (i + 1) % 2
                                    pa = ps512.tile([128, 512], F32, name="pa", tag="p512")
                                    if dorow:
                                        pr = psR.tile([128, 1], F32, name="pr", tag="pr", bufs=1)
                                    for tp in range(npair):
                                        lhs = eT[:, TRI[sb] + 2 * tp : TRI[sb] + 2 * tp + 2, so : so + 128]
                                        last = (tp == npair - 1) and not odd
                                        nc.tensor.matmul(
                                            pa[:],
                                            lhs,
                                            v_sb[:, 2 * tp : 2 * tp + 2, fb * 512 : (fb + 1) * 512],
                                            start=(tp == 0),
                                            stop=last,
                                            perf_mode=DR,
                                        )
                                        if dorow:
                                            nc.tensor.matmul(
                                                pr[:],
                                                lhs,
                                                ones8[:],
                                                start=(tp == 0),
                                                stop=last,
                                                perf_mode=DR,
                                            )
                                    if odd:
                                        lhs1 = eT[:, TRI[sb] + i, so : so + 128]
                                        nc.tensor.matmul(
                                            pa[:],
                                            lhs1,
                                            v_sb[:, i, fb * 512 : (fb + 1) * 512],
                                            start=(npair == 0),
                                            stop=True,
                                        )
                                        if dorow:
                                            nc.tensor.matmul(
                                                pr[:],
                                                lhs1,
                                                ones8[:, 0, :],
                                                start=(npair == 0),
                                                stop=True,
                                            )
                                    if dorow:
                                        rsf = astg.tile([128, 1], F32, name="rsf", tag="rsf")
                                        nc.vector.tensor_copy(out=rsf[:], in_=pr[:])
                                        nc.vector.reciprocal(recips[:, h, i : i + 1], rsf[:])
                                    stg = astg.tile([128, 512], F16, name="stg", tag="stg", bufs=8)
                                    nc.scalar.activation(
                                        stg[:],
                                        pa[:],
                                        AF.Copy,
                                        scale=recips[:, h, i : i + 1],
                                    )
                                    if lastblk:
                                        # even chunk i=2j -> core j's first 128
                                        # rows (half 0); odd -> half 1
                                        blk = i // 2
                                        nc.sync.dma_start(
                                            atts_in[fb][i % 2][blk * 128 : (blk + 1) * 128, :],
                                            stg[:],
                                        )
                                        if i == 1:  # odd chunks all staged
                                            nc.gpsimd.collective_compute(
                                                "ReduceScatter",
                                                AL.add,
                                                replica_groups=RG,
                                                ins=[atts_in[fb][1][:]],
                                                outs=[atts_out[fb][1][:]],
                                            )
                                    else:
                                        nc.sync.dma_start(
                                            att_in[h][fh][fb][i * 128 : (i + 1) * 128, :], stg[:]
                                        )
                                if lastblk:
                                    nc.gpsimd.collective_compute(
                                        "ReduceScatter",
                                        AL.add,
                                        replica_groups=RG,
                                        ins=[atts_in[fb][0][:]],
                                        outs=[atts_out[fb][0][:]],
                                    )
                                else:
                                    nc.gpsimd.collective_compute(
                                        "ReduceScatter",
                                        AL.add,
                                        replica_groups=RG,
                                        ins=[att_in[h][fh][fb][:]],
                                        outs=[att_out[h][fh][fb][:]],
                                    )

            # =========== LN1 (global mean/var) ===========
            with tc.tile_pool(name="mid", bufs=1) as midp:
              h_own = midp.tile([128, 2, E], BF16, name="h_own")
              ysT = midp.tile([128, 16, RROWS], BF16, name="ysT")
              lngt = midp.tile([128, 2, E], BF16, name="lngt")
              lnbt = midp.tile([128, 2, E], BF16, name="lnbt")
              with tc.tile_pool(name="ln1", bufs=1) as lp, nc.named_scope("ln1"):
                  ys = lp.tile([128, 2, E], F32, name="ys")
                  ysb = lp.tile([128, 2, E], BF16, name="ysb")
                  yb1t = lp.tile([128, E], F32, name="yb1t")
                  nc.sync.dma_start(yb1t[:], yb1_d[:])
                  nc.sync.dma_start(lngt[:], lng_d.ap().rearrange("(t p) e -> p t e", p=128))
                  nc.sync.dma_start(lnbt[:], lnb_d.ap().rearrange("(t p) e -> p t e", p=128))
                  lngT = lp.tile([128, 16, RROWS], BF16, name="lngT")
                  nc.sync.dma_start(lngT[:], lngT_d.ap().rearrange("eo p r -> p eo r"))
                  lnbT = lp.tile([128, 16, RROWS], BF16, name="lnbT")
                  nc.sync.dma_start(lnbT[:], lnbT_d.ap().rearrange("eo p r -> p eo r"))
                  for rt in range(2):
                      xrt = lp.tile([128, E], F32, name="xrt", tag="xrt", bufs=2)
                      nc.sync.dma_start(xrt[:], xr_d[rt * 128 : (rt + 1) * 128, :])
                      nc.vector.tensor_tensor(ys[:, rt, :], xrt[:], yb1t[:], AL.add)
                  # accumulate RS chunks as they land; the last head's chunk
                  # completes a column block -> emit its stats partial
                  parts = lp.tile([128, 16], F32, name="parts")
                  sqs = lp.tile([128, 512], BF16, name="sqs", tag="sqs", bufs=2)
                  for h in range(HPC):
                      for fh in range(2):
                          for fb in range(2):
                              lastblk = h == HPC - 1 and fh == 1
                              # the odd-rows RS (rt=1) of the final block
                              # lands first; consume in landing order
                              for rt in ([1, 0] if lastblk else [0, 1]):
                                  rof = lp.tile([128, 512], F16, name="rof", tag="rof", bufs=4)
                                  if lastblk:
                                      nc.sync.dma_start(rof[:], atts_out[fb][rt][:])
                                  else:
                                      nc.sync.dma_start(
                                          rof[:], att_out[h][fh][fb][rt * 128 : (rt + 1) * 128, :]
                                      )
                                  col = fh * 1024 + fb * 512
                                  dstv = ys[:, rt, col : col + 512]
                                  nc.vector.tensor_tensor(dstv, dstv, rof[:], AL.add)
                                  if h == HPC - 1:
                                      blk = (fh * 2 + fb) * 2 + rt
                                      nc.vector.tensor_reduce(
                                          parts[:, blk : blk + 1], dstv, axis=AX.X, op=AL.add
                                      )
                                      nc.scalar.activation(
                                          sqs[:], dstv, AF.Square,
                                          accum_out=parts[:, 8 + blk : 9 + blk],
                                      )
                  pstat = psT.tile([128, 128], F32, name="pstat", tag="pt", bufs=1)
                  nc.tensor.matmul(pstat[:1, :16], onesc[:, 0:1], parts[:], start=True, stop=True)
                  st4s = lp.tile([1, 16], F32, name="st4s")
                  nc.vector.tensor_copy(out=st4s[:], in_=pstat[:1, :16])
                  st4 = lp.tile([1, 8], F32, name="st4")
                  nc.vector.memset(st4[:], 0.0)
                  nc.vector.tensor_reduce(st4[:, 0:1], st4s[:, 0:8], axis=AX.X, op=AL.add)
                  nc.vector.tensor_reduce(st4[:, 1:2], st4s[:, 8:16], axis=AX.X, op=AL.add)
                  nc.sync.dma_start(st1_in[:], st4[:])
                  nc.gpsimd.collective_compute(
                      "AllReduce", AL.add, replica_groups=RG, ins=[st1_in[:]], outs=[st1_out[:]]
                  )
                  # transpose own rows (pre-affine) while the AllReduce flies
                  for rt in range(2):
                      nc.vector.tensor_copy(out=ysb[:, rt, :], in_=ys[:, rt, :])
                  for eo in range(16):
                      for rt in range(2):
                          pth = psT.tile([128, 128], BF16, name="pth", tag="ptT")
                          nc.tensor.transpose(
                              pth[:], ysb[:, rt, eo * 128 : (eo + 1) * 128], hidb[:]
                          )
                          if (eo + rt) % 2 == 0:
                              nc.vector.tensor_copy(
                                  out=ysT[:, eo, rt * 128 : (rt + 1) * 128], in_=pth[:]
                              )
                          else:
                              nc.scalar.copy(ysT[:, eo, rt * 128 : (rt + 1) * 128], pth[:])
                  bc = _ln_scalars(nc, lp, psT, onesr, st1_out)
                  # LN affine in transposed space, AllGather per own-row half
                  for hf in range(2):
                      src = ysT[:, :, hf * 128 : (hf + 1) * 128]
                      stT = lp.tile([128, 16, 128], BF16, name="hstgT", tag="hstgT", bufs=2)
                      nc.scalar.activation(
                          stT[:], src, AF.Identity, bias=bc[:, 0:1], scale=bc[:, 1:2]
                      )
                      nc.vector.tensor_tensor(
                          stT[:], stT[:], lngT[:, :, hf * 128 : (hf + 1) * 128], AL.mult
                      )
                      nc.vector.tensor_tensor(
                          stT[:], stT[:], lnbT[:, :, hf * 128 : (hf + 1) * 128], AL.add
                      )
                      nc.sync.dma_start(
                          agt_in[hf].rearrange("eo p r -> p eo r"), stT[:]
                      )
                      nc.gpsimd.collective_compute(
                          "AllGather",
                          AL.bypass,
                          replica_groups=RG,
                          ins=[agt_in[hf][:]],
                          outs=[agt_out[hf][:]],
                      )
                  # row-major h_own for the LN2 residual (off critical path)
                  ht_f32 = lp.tile([128, E], F32, name="ht_f32", tag="htf", bufs=2)
                  for rt in range(2):
                      nc.scalar.activation(
                          ht_f32[:], ys[:, rt, :], AF.Identity,
                          bias=bc[:, 0:1], scale=bc[:, 1:2],
                      )
                      nc.vector.tensor_tensor(ht_f32[:], ht_f32[:], lngt[:, rt, :], AL.mult)
                      nc.vector.tensor_tensor(h_own[:, rt, :], ht_f32[:], lnbt[:, rt, :], AL.add)

              # =========== FFN (hidden shard 1024) ===========
              # hT's S axis is PERMUTED: col = hf*1024 + c*128 + r maps to
              # true row c*256 + hf*128 + r. FFN1/FFN2 are elementwise along
              # S so only the final staging DMA needs to undo the mapping.
              with tc.tile_pool(name="ffn", bufs=1) as fp, nc.named_scope("ffn"):
                  hT = fp.tile([128, 16, S], BF16, name="hT")
                  zT = fp.tile([128, 8, S], BF16, name="zT")
                  w1full = fp.tile([128, 8, 2048], BF16, name="w1full")
                  for ft in range(8):
                      nc.sync.dma_start(w1full[:, ft, :], w1_d[:, ft, :])
                  with tc.tile_pool(name="wst", bufs=2) as wst:
                      for hf in range(2):
                          for c in range(NCORES):
                              nc.sync.dma_start(
                                  hT[:, :, hf * 1024 + c * 128 : hf * 1024 + (c + 1) * 128],
                                  agt_out[hf][c * 16 : (c + 1) * 16].rearrange("eo p r -> p eo r"),
                              )
                      # FFN1 per gathered S-half: starts on half 0 while
                      # half 1 is still gathering
                      for hf in range(2):
                          for ft in range(8):
                              pzs = [
                                  ps512.tile([128, 512], F32, name=f"pz{sb}", tag="p512")
                                  for sb in range(2)
                              ]
                              for eo in range(16):
                                  for sb in range(2):
                                      nc.tensor.matmul(
                                          pzs[sb][:],
                                          w1full[:, ft, eo * KD : (eo + 1) * KD],
                                          hT[:, eo, hf * 1024 + sb * 512 : hf * 1024 + (sb + 1) * 512],
                                          start=(eo == 0),
                                          stop=(eo == 15),
                                      )
                              for sb in range(2):
                                  nc.scalar.activation(
                                      zT[:, ft, hf * 1024 + sb * 512 : hf * 1024 + (sb + 1) * 512],
                                      pzs[sb][:],
                                      AF.Relu,
                                      bias=b1_sb[:, ft : ft + 1],
                                  )
                      for ebx, (c0, w) in enumerate(EBW):
                          w2b = wst.tile([128, 8, w], BF16, name="w2b", tag="w2b")
                          nc.sync.dma_start(w2b[:], w2_d[:, :, c0 // 512, :])
                          for p in range(16):
                              pf = ps512.tile([128, w], F32, name="pf", tag="p512")
                              for fc in range(8):
                                  nc.tensor.matmul(
                                      pf[:],
                                      zT[:, fc, p * 128 : (p + 1) * 128],
                                      w2b[:, fc, :],
                                      start=(fc == 0),
                                      stop=(fc == 7),
                                  )
                              fstg = wst.tile([128, w], F16, name="fstg", tag="fstg", bufs=4)
                              nc.scalar.activation(fstg[:], pf[:], AF.Copy)
                              hf, cc = p // 8, p % 8
                              r0 = cc * 256 + hf * 128
                              nc.sync.dma_start(ffn_in[ebx][r0 : r0 + 128, :], fstg[:])
                          nc.gpsimd.collective_compute(
                              "ReduceScatter",
                              AL.add,
                              replica_groups=RG,
                              ins=[ffn_in[ebx][:]],
                              outs=[ffn_out[ebx][:]],
                          )

              # =========== LN2 + output ===========
              with tc.tile_pool(name="ln2", bufs=1) as l2, nc.named_scope("ln2"):
                  ys2 = l2.tile([128, 2, E], F32, name="ys2")
                  yb2t = l2.tile([128, E], F32, name="yb2t")
                  nc.sync.dma_start(yb2t[:], yb2_d[:])
                  for rt in range(2):
                      nc.vector.tensor_tensor(
                          ys2[:, rt, :], h_own[:, rt, :], yb2t[:], AL.add
                      )
                  parts2 = l2.tile([128, 16], F32, name="parts2")
                  sqs2 = l2.tile([128, 512], BF16, name="sqs2", tag="sqs2", bufs=2)
                  for ebx, (c0, w) in enumerate(EBW):
                      for rt in range(2):
                          fot = l2.tile([128, w], F16, name="fot", tag="fot", bufs=4)
                          nc.sync.dma_start(fot[:], ffn_out[ebx][rt * 128 : (rt + 1) * 128, :])
                          dstv = ys2[:, rt, c0 : c0 + w]
                          nc.vector.tensor_tensor(dstv, dstv, fot[:], AL.add)
                          blk = ebx * 2 + rt
                          nc.vector.tensor_reduce(
                              parts2[:, blk : blk + 1], dstv, axis=AX.X, op=AL.add
                          )
                          nc.scalar.activation(
                              sqs2[:, 0:w], dstv, AF.Square,
                              accum_out=parts2[:, 8 + blk : 9 + blk],
                          )
                  pstat2 = psT.tile([128, 128], F32, name="pstat2", tag="pt", bufs=1)
                  nc.tensor.matmul(pstat2[:1, :16], onesc[:, 0:1], parts2[:], start=True, stop=True)
                  st4s2 = l2.tile([1, 16], F32, name="st4s2")
                  nc.vector.tensor_copy(out=st4s2[:], in_=pstat2[:1, :16])
                  st42 = l2.tile([1, 8], F32, name="st42")
                  nc.vector.memset(st42[:], 0.0)
                  nc.vector.tensor_reduce(st42[:, 0:1], st4s2[:, 0:8], axis=AX.X, op=AL.add)
                  nc.vector.tensor_reduce(st42[:, 1:2], st4s2[:, 8:16], axis=AX.X, op=AL.add)
                  nc.sync.dma_start(st2_in[:], st42[:])
                  nc.gpsimd.collective_compute(
                      "AllReduce", AL.add, replica_groups=RG, ins=[st2_in[:]], outs=[st2_out[:]]
                  )
                  bc2 = _ln_scalars(nc, l2, psT, onesr, st2_out)
                  for rt in range(2):
                      ot = l2.tile([128, E], F32, name="ot", tag="ot", bufs=2)
                      nc.scalar.activation(
                          ot[:],
                          ys2[:, rt, :],
                          AF.Identity,
                          bias=bc2[:, 0:1],
                          scale=bc2[:, 1:2],
                      )
                      nc.vector.tensor_tensor(ot[:], ot[:], lngt[:, rt, :], AL.mult)
                      nc.vector.tensor_tensor(ot[:], ot[:], lnbt[:, rt, :], AL.add)
                      nc.sync.dma_start(out_d[rt * 128 : (rt + 1) * 128, :], ot[:])

    nc.compile()
    return nc


def _ln_scalars(nc, pool, psT, onesr, st_out):
    """AllReduced (sum, sumsq) -> bc [128, 2] = (-m*rstd, rstd) broadcast."""
    so = pool.tile([1, 8], F32, name="so", tag="so")
    nc.sync.dma_start(so[:], st_out[:])
    sc = pool.tile([1, 8], F32, name="sc", tag="sc")
    # sc0 = m, sc1 = E[y^2], sc2 = m^2, sc3 = var, sc4 = rstd, sc5 = -m*rstd
    nc.scalar.mul(sc[:, 0:1], so[:, 0:1], 1.0 / NTOT)
    nc.scalar.mul(sc[:, 1:2], so[:, 1:2], 1.0 / NTOT)
    nc.scalar.activation(sc[:, 2:3], sc[:, 0:1], AF.Square)
    nc.vector.tensor_tensor(sc[:, 3:4], sc[:, 1:2], sc[:, 2:3], AL.subtract)
    nc.vector.tensor_scalar_add(sc[:, 2:3], sc[:, 3:4], EPS)  # var + eps
    # rstd = 1/sqrt(var+eps): Sqrt on ACT (sqrt table also carries square/
    # identity/copy, so no further reloads), reciprocal on DVE (table-free)
    nc.scalar.activation(sc[:, 6:7], sc[:, 2:3], AF.Sqrt)
    nc.vector.reciprocal(sc[:, 4:5], sc[:, 6:7])
    nc.vector.tensor_tensor(sc[:, 7:8], sc[:, 0:1], sc[:, 4:5], AL.mult)
    nc.scalar.mul(sc[:, 5:6], sc[:, 7:8], -1.0)
    s2 = pool.tile([1, 2], F32, name="s2", tag="s2")
    nc.vector.tensor_copy(out=s2[:, 0:1], in_=sc[:, 5:6])
    nc.vector.tensor_copy(out=s2[:, 1:2], in_=sc[:, 4:5])
    pb = psT.tile([128, 128], F32, name="pb", tag="pt", bufs=1)
    nc.tensor.matmul(pb[:, :2], onesr[:], s2[:], start=True, stop=True)
    bc = pool.tile([128, 2], F32, name="bc", tag="bc")
    nc.vector.tensor_copy(out=bc[:], in_=pb[:, :2])
    return bc


_NC_CACHE = None


def _get_nc():
    global _NC_CACHE
    if _NC_CACHE is None:
        _NC_CACHE = _build()
    return _NC_CACHE


def _prep_core(c, inputs):
    f32 = np.float32
    x = np.ascontiguousarray(inputs["input"], dtype=f32)
    Wq, Wk, Wv = inputs["Wq"], inputs["Wk"], inputs["Wv"]
    bq, bk, bv = inputs["bq"], inputs["bk"], inputs["bv"]
    W1, b1, W2, b2 = inputs["W1"], inputs["b1"], inputs["W2"], inputs["b2"]
    ln_g, ln_b = inputs["ln_g"], inputs["ln_b"]
    h0 = c * HPC
    wqt = np.ascontiguousarray(
        np.stack(
            [Wq[h0 + h].reshape(16, 128, KD).transpose(1, 0, 2) for h in range(HPC)]
        ).astype(_bf16)
    )
    wkt = np.ascontiguousarray(
        np.stack(
            [Wk[h0 + h].reshape(16, 128, KD).transpose(1, 0, 2) for h in range(HPC)]
        ).astype(_bf16)
    )
    wvt = np.ascontiguousarray(
        np.stack(
            [Wv[h0 + h].reshape(16, 128, E).transpose(1, 0, 2) for h in range(HPC)]
        ).astype(_f8)
    )
    W1s = W1[:, c * FSH : (c + 1) * FSH]
    w1t = np.ascontiguousarray(
        W1s.reshape(16, 128, 8, 128).transpose(1, 2, 0, 3).reshape(128, 8, 2048).astype(_bf16)
    )
    W2s = W2[c * FSH : (c + 1) * FSH, :]
    w2t = np.ascontiguousarray(
        W2s.reshape(8, 128, 4, 512).transpose(1, 0, 2, 3).astype(_bf16)
    )
    bqs = np.ascontiguousarray((bq[h0 : h0 + HPC] * ISCALE).T, dtype=f32)
    bks = np.ascontiguousarray(bk[h0 : h0 + HPC].T, dtype=f32)
    b1s = np.ascontiguousarray(b1[c * FSH : (c + 1) * FSH].reshape(8, 128).T, dtype=f32)
    yb1 = np.ascontiguousarray(np.broadcast_to(bv.sum(axis=0), (128, E)), dtype=f32)
    yb2 = np.ascontiguousarray(np.broadcast_to(b2, (128, E)), dtype=f32)
    rows = slice(c * RROWS, (c + 1) * RROWS)
    jj, tp, sf = np.meshgrid(
        np.arange(4), np.arange(128), np.arange(512), indexing="ij"
    )
    mask = (((128 * jj + tp) <= sf).astype(f32) - 1.0) * 30.0
    xt = x.T.reshape(16, 128, S).transpose(1, 0, 2)
    xtb = np.ascontiguousarray(xt.astype(_bf16))
    xt8 = np.ascontiguousarray(xt.astype(_f8))
    return {
        "xtb": xtb,
        "xt8": xt8,
        "wqt": wqt,
        "wkt": wkt,
        "wvt": wvt,
        "w1t": w1t,
        "w2t": w2t,
        "bqs": bqs,
        "bks": bks,
        "b1s": b1s,
        "yb1": yb1,
        "yb2": yb2,
        "xr": np.ascontiguousarray(x[rows], dtype=f32),
        "lngr": np.ascontiguousarray(np.asarray(ln_g[rows], dtype=f32).astype(_bf16)),
        "lnbr": np.ascontiguousarray(np.asarray(ln_b[rows], dtype=f32).astype(_bf16)),
        "lngT": np.ascontiguousarray(
            np.asarray(ln_g[rows], dtype=f32).T.reshape(16, 128, RROWS).astype(_bf16)
        ),
        "lnbT": np.ascontiguousarray(
            np.asarray(ln_b[rows], dtype=f32).T.reshape(16, 128, RROWS).astype(_bf16)
        ),
        "mask": np.ascontiguousarray(mask),
        "ident": np.eye(128, dtype=f32),
        "ones": np.ones((128, 8), dtype=f32),
        "onesr": np.ones((1, 128), dtype=f32),
    }


def kernel(**inputs):
    nc = _get_nc()
    inputs = {k: np.asarray(v, dtype=np.float32) for k, v in inputs.items()}
    in_maps = [_prep_core(c, inputs) for c in range(NCORES)]
    res = run_bass_kernel_spmd(nc, in_maps, core_ids=list(range(NCORES)))
    out = np.concatenate([res.results[c]["out"] for c in range(NCORES)], axis=0)
    return np.ascontiguousarray(out, dtype=np.float32)



# revision 39
# speedup vs baseline: 1.4450x; 1.0213x over previous
"""Tensor-parallel decoder layer on 8 TRN2 NeuronCores.

Sharding:
  - Attention: 16 heads -> 2 per core. Per-core partial attn_out is
    ReduceScattered (fp16) so core c owns rows [256c, 256c+256).
  - Global LayerNorm (scalar mean/var over the whole [S,E] tensor):
    per-core partial (sum, sumsq) AllReduced as a tiny fp32 tensor.
  - FFN: hidden dim 8192 -> 1024 per core; partial [S,E] output
    ReduceScattered per 512-column chunk (fp16), overlapping FFN2.
  - h is AllGathered transposed (bf16) since every matmul contracting
    over E needs h^T as the moving operand.

Matmul layout notes (PE computes out = lhsT.T @ rhs, contraction on the
partition dim):
  - x^T resident in SBUF (bf16 for Q/K, fp8e4 for V) feeds projections.
  - The value path runs in fp8e4 DoubleRow (2 k-tiles per instruction,
    2x MAC rate): V projection contracts eo pairs, attn@v contracts
    t-tile pairs. exp is shifted by a constant (exp(s - SH)) so scores
    fit fp8's +/-240 range; the shift cancels exactly in num/rowsum.
  - scores are built transposed: S^T[t,s] tiles, so exp(S^T) tiles feed
    attn@v directly as lhsT with no transposes; softmax normalization is
    deferred: rowsum via a fp8 ones-column matmul, applied as a
    per-partition scale on the PSUM->SBUF copy of attn@v output.
  - causal masking folds into the score: a -30 additive mask lands on
    the PSUM tile before the exp, so masked entries underflow to 0.
"""

import math
import sys

sys.path.insert(0, "/opt/trn_rl_repo")

import numpy as np
import ml_dtypes

_bf16 = ml_dtypes.bfloat16
_f8 = ml_dtypes.float8_e4m3

import concourse.bass as bass
import concourse.mybir as mybir
import concourse.tile as tile
from concourse import bacc
from concourse.bass_utils import run_bass_kernel_spmd

S, E, H, KD, FF = 2048, 2048, 16, 128, 8192
EPS = 1e-5
NCORES = 8
HPC = H // NCORES          # heads per core = 2
FSH = FF // NCORES         # ffn hidden shard = 1024
RROWS = S // NCORES        # row shard = 256
NTOT = float(S * E)
ISCALE = 1.0 / math.sqrt(KD)

F32 = mybir.dt.float32
BF16 = mybir.dt.bfloat16
F16 = mybir.dt.float16
FP8 = mybir.dt.float8e4
AF = mybir.ActivationFunctionType
AL = mybir.AluOpType
AX = mybir.AxisListType
DR = mybir.MatmulPerfMode.DoubleRow

# packed triangular offsets for eT tiles: tile(tc, sb) at TRI[sb] + tc
TRI = [0, 4, 12, 24]
NTRI = 40
SH = 2.5  # constant exp shift: keeps exp(score - SH) within fp8e4 range


def _build():
    nc = bacc.Bacc(
        "TRN2",
        target_bir_lowering=False,
        debug=False,
        enable_asserts=True,
        num_devices=NCORES,
    )

    # ---- external I/O (per-core shards prepared on the host) ----
    xtb_d = nc.dram_tensor("xtb", [128, 16, S], BF16, kind="ExternalInput")
    xt8_d = nc.dram_tensor("xt8", [128, 16, S], FP8, kind="ExternalInput")
    wq_d = nc.dram_tensor("wqt", [HPC, 128, 16, KD], BF16, kind="ExternalInput")
    wk_d = nc.dram_tensor("wkt", [HPC, 128, 16, KD], BF16, kind="ExternalInput")
    wv_d = nc.dram_tensor("wvt", [HPC, 128, 16, E], FP8, kind="ExternalInput")
    w1_d = nc.dram_tensor("w1t", [128, 8, 2048], BF16, kind="ExternalInput")
    w2_d = nc.dram_tensor("w2t", [128, 8, 4, 512], BF16, kind="ExternalInput")
    bq_d = nc.dram_tensor("bqs", [128, HPC], F32, kind="ExternalInput")
    bk_d = nc.dram_tensor("bks", [128, HPC], F32, kind="ExternalInput")
    b1_d = nc.dram_tensor("b1s", [128, 8], F32, kind="ExternalInput")
    yb1_d = nc.dram_tensor("yb1", [128, E], F32, kind="ExternalInput")
    yb2_d = nc.dram_tensor("yb2", [128, E], F32, kind="ExternalInput")
    xr_d = nc.dram_tensor("xr", [RROWS, E], F32, kind="ExternalInput")
    lng_d = nc.dram_tensor("lngr", [RROWS, E], BF16, kind="ExternalInput")
    lnb_d = nc.dram_tensor("lnbr", [RROWS, E], BF16, kind="ExternalInput")
    lngT_d = nc.dram_tensor("lngT", [16, 128, RROWS], BF16, kind="ExternalInput")
    lnbT_d = nc.dram_tensor("lnbT", [16, 128, RROWS], BF16, kind="ExternalInput")
    mask_d = nc.dram_tensor("mask", [4, 128, 512], F32, kind="ExternalInput")
    id_d = nc.dram_tensor("ident", [128, 128], F32, kind="ExternalInput")
    ones_d = nc.dram_tensor("ones", [128, 8], F32, kind="ExternalInput")
    onesr_d = nc.dram_tensor("onesr", [1, 128], F32, kind="ExternalInput")
    out_d = nc.dram_tensor("out", [RROWS, E], F32, kind="ExternalOutput")

    RG = [list(range(NCORES))]

    with tile.TileContext(nc) as tc:
        with (
            tc.tile_pool(name="persist", bufs=1) as pp,
            tc.tile_pool(name="dram", bufs=1, space="DRAM") as dp,
            tc.tile_pool(name="ps512", bufs=5, space="PSUM") as ps512,
            tc.tile_pool(name="psT", bufs=1, space="PSUM") as psT,
            tc.tile_pool(name="psR", bufs=1, space="PSUM") as psR,
        ):
            # ---- collective bounce buffers (internal DRAM) ----
            att_in = [
                [
                    dp.tile([S, 512], F16, name=f"att_in_{fh}_{fb}", tag=f"ati{fh}{fb}")
                    for fb in range(2)
                ]
                for fh in range(2)
            ]
            att_out = [
                [
                    dp.tile([RROWS, 512], F16, name=f"att_out_{fh}_{fb}", tag=f"ato{fh}{fb}")
                    for fb in range(2)
                ]
                for fh in range(2)
            ]
            # the final attention block's RS is row-split: even/odd 128-row
            # chunks scatter to (own-rows first/second half), so two small
            # collectives replace one big one on the critical tail
            atts_in = [
                [
                    dp.tile([S // 2, 512], F16, name=f"atts_in_{fb}_{ab}", tag=f"atsi{fb}{ab}")
                    for ab in range(2)
                ]
                for fb in range(2)
            ]
            atts_out = [
                [
                    dp.tile([128, 512], F16, name=f"atts_out_{fb}_{ab}", tag=f"atso{fb}{ab}")
                    for ab in range(2)
                ]
                for fb in range(2)
            ]
            st1_in = dp.tile([1, 8], F32, name="st1_in", tag="st1i")
            st1_out = dp.tile([1, 8], F32, name="st1_out", tag="st1o", addr_space="Shared")
            st2_in = dp.tile([1, 8], F32, name="st2_in", tag="st2i")
            st2_out = dp.tile([1, 8], F32, name="st2_out", tag="st2o", addr_space="Shared")
            # transposed h AllGather, split by own-row halves so FFN1 can
            # start on the first half while the second gathers
            agt_in = [
                dp.tile([16, 128, 128], BF16, name=f"agt_in{j}", tag=f"agti{j}")
                for j in range(2)
            ]
            agt_out = [
                dp.tile([128, 128, 128], BF16, name=f"agt_out{j}", tag=f"agto{j}", addr_space="Shared")
                for j in range(2)
            ]
            # ffn output chunks: col offset/width
            EBW = [(0, 512), (512, 512), (1024, 512), (1536, 512)]
            ffn_in = [
                dp.tile([S, w], F16, name=f"ffn_in_{eb}", tag=f"ffi{eb}")
                for eb, (c0, w) in enumerate(EBW)
            ]
            ffn_out = [
                dp.tile([RROWS, w], F16, name=f"ffn_out_{eb}", tag=f"ffo{eb}")
                for eb, (c0, w) in enumerate(EBW)
            ]

            # ---- persistent small tiles ----
            ident = pp.tile([128, 128], F32, name="ident")
            nc.sync.dma_start(ident[:], id_d[:])
            hidb = pp.tile([128, 128], BF16, name="hidb")
            nc.vector.tensor_copy(out=hidb[:], in_=ident[:])
            onesc = pp.tile([128, 8], F32, name="onesc")
            nc.sync.dma_start(onesc[:], ones_d[:])
            onesr = pp.tile([1, 128], F32, name="onesr")
            nc.sync.dma_start(onesr[:], onesr_d[:])
            bq_sb = pp.tile([128, HPC], F32, name="bq_sb")
            nc.sync.dma_start(bq_sb[:], bq_d[:])
            bk_sb = pp.tile([128, HPC], F32, name="bk_sb")
            nc.sync.dma_start(bk_sb[:], bk_d[:])
            b1_sb = pp.tile([128, 8], F32, name="b1_sb")
            nc.sync.dma_start(b1_sb[:], b1_d[:])
            qkT = pp.tile([128, 2, HPC, S], BF16, name="qkT")  # [d, q/k, head, s]
            recips = pp.tile([128, HPC, 16], F32, name="recips")

            # =========== phase 0 + attention ===========
            with tc.tile_pool(name="attn", bufs=1) as ap_:
                xT8 = ap_.tile([128, 16, S], FP8, name="xT8")  # x^T fp8 for V
                maskf = ap_.tile([128, 4, 512], F32, name="maskf")  # (mask-1)*30
                shm = ap_.tile([128, 1], F32, name="shm")  # exp shift bias
                ones8 = ap_.tile([128, 2, 1], FP8, name="ones8")
                with tc.tile_pool(name="qkx", bufs=1) as qkx:
                    xT = qkx.tile([128, 16, S], BF16, name="xT")  # x^T bf16 for Q/K
                    wqk = [
                        [
                            qkx.tile([128, 16, KD], BF16, name=f"wqk{h}{qi}")
                            for qi in range(2)
                        ]
                        for h in range(HPC)
                    ]
                    with tc.tile_pool(name="prep", bufs=1) as prep, nc.named_scope("prep"):
                        # HAM warmup: back-to-back matmuls to unthrottle PE,
                        # issued while the first weight/x DMAs land
                        wtile = prep.tile([128, 512], BF16, name="wtile", tag="wtile", bufs=1)
                        nc.vector.memset(wtile[:], 0.0)
                        nc.vector.memset(shm[:], -SH)
                        nc.vector.memset(ones8[:], 1.0)
                        # interleave the first x^T chunks with the small
                        # weight DMAs so qkproj's first matmul gates on
                        # neither the bulk x transfer nor a queued weight
                        qkdmas = [
                            (wqk[h][qi], (wq_d if qi == 0 else wk_d)[h])
                            for h in range(HPC)
                            for qi in range(2)
                        ]
                        for eo in range(4):
                            nc.sync.dma_start(xT[:, eo, :], xtb_d[:, eo, :])
                            dst, src = qkdmas[eo]
                            nc.sync.dma_start(dst[:], src)
                        for _w in range(24):
                            pw = ps512.tile([128, 512], F32, name="pw", tag="p512")
                            nc.tensor.matmul(pw[:], wtile[:, :128], wtile[:], start=True, stop=True)
                        for eo in range(4, 16):
                            nc.sync.dma_start(xT[:, eo, :], xtb_d[:, eo, :])
                        nc.sync.dma_start(maskf[:], mask_d.ap().rearrange("j p s -> p j s"))
                        for eo in range(16):
                            nc.sync.dma_start(xT8[:, eo, :], xt8_d[:, eo, :])

                    # ---- Q/K projections for both heads (scaled/biased) ----
                    with nc.named_scope("qkproj"):
                        for h in range(HPC):
                            for qi, (b_sb, scl) in enumerate(
                                ((bq_sb, ISCALE), (bk_sb, 1.0))
                            ):
                                wb = wqk[h][qi]
                                pqs = [
                                    ps512.tile([128, 512], F32, name=f"pq{sb}", tag="p512")
                                    for sb in range(4)
                                ]
                                for eo in range(16):
                                    for sb in range(4):
                                        nc.tensor.matmul(
                                            pqs[sb][:],
                                            wb[:, eo, :],
                                            xT[:, eo, sb * 512 : (sb + 1) * 512],
                                            start=(eo == 0),
                                            stop=(eo == 15),
                                        )
                                for sb in range(4):
                                    nc.scalar.activation(
                                        qkT[:, qi, h, sb * 512 : (sb + 1) * 512],
                                        pqs[sb][:],
                                        AF.Identity,
                                        bias=b_sb[:, h : h + 1],
                                        scale=scl,
                                    )

                # ---- per-head attention (value path in fp8 DoubleRow) ----
                eT = ap_.tile([128, NTRI, 512], FP8, name="eT")
                v_sb = ap_.tile([128, 16, FSH], FP8, name="v_sb")
                with (
                    tc.tile_pool(name="acc", bufs=1) as accp,
                    tc.tile_pool(name="wvb", bufs=2) as wvbp,
                    tc.tile_pool(name="astg", bufs=4) as astg,
                ):
                    # both local heads sum on-chip here before the
                    # collective, halving attention ReduceScatter traffic
                    att_acc = [
                        accp.tile([128, 16, FSH], F16, name=f"att_acc{fh}")
                        for fh in range(2)
                    ]
                    for h in range(HPC):
                      with nc.named_scope(f"scores{h}"):
                        for sb in range(4):
                            for tcn in range(4 * sb + 4):
                                psc = ps512.tile([128, 512], F32, name="psc", tag="p512")
                                nc.tensor.matmul(
                                    psc[:],
                                    qkT[:, 1, h, tcn * 128 : (tcn + 1) * 128],
                                    qkT[:, 0, h, sb * 512 : (sb + 1) * 512],
                                    start=True,
                                    stop=True,
                                )
                                if tcn >= 4 * sb:
                                    # diagonal tile: -30 mask onto PSUM, then
                                    # masked entries underflow to 0 in the exp
                                    nc.vector.tensor_tensor(
                                        psc[:], psc[:], maskf[:, tcn - 4 * sb, :], AL.add
                                    )
                                nc.scalar.activation(
                                    eT[:, TRI[sb] + tcn, :], psc[:], AF.Exp, bias=shm[:]
                                )

                        # pass B: per f-half: v-projection then attn@v
                        for fh in range(2):
                          with nc.named_scope(f"vproj{h}{fh}"):
                            for fb in range(2):
                                wvb = wvbp.tile([128, 16, 512], FP8, name="wvb", tag="wvb")
                                nc.sync.dma_start(
                                    wvb[:],
                                    wv_d[h][
                                        :, :, fh * 1024 + fb * 512 : fh * 1024 + (fb + 1) * 512
                                    ],
                                )
                                for tcn in range(16):
                                    pv = ps512.tile([128, 512], F32, name="pv", tag="p512")
                                    for j in range(8):
                                        nc.tensor.matmul(
                                            pv[:],
                                            xT8[:, 2 * j : 2 * j + 2, tcn * 128 : (tcn + 1) * 128],
                                            wvb[:, 2 * j : 2 * j + 2, :],
                                            start=(j == 0),
                                            stop=(j == 7),
                                            perf_mode=DR,
                                        )
                                    if tcn % 2 == 0:
                                        nc.vector.tensor_copy(
                                            out=v_sb[:, tcn, fb * 512 : (fb + 1) * 512],
                                            in_=pv[:],
                                        )
                                    else:
                                        nc.scalar.copy(
                                            v_sb[:, tcn, fb * 512 : (fb + 1) * 512], pv[:]
                                        )

                          with nc.named_scope(f"attnv{h}{fh}"):
                            # fb-outer: each 512-col half ReduceScatters as
                            # soon as it is complete, halving the exposed
                            # tail after the last attnv block
                            finalh = h == HPC - 1
                            lastblk = finalh and fh == 1
                            for fb in range(2):
                                dorow = fh == 0 and fb == 0
                                # final block: odd row-chunks first, then
                                # even, so the odd-half RS streams out early
                                iorder = (
                                    list(range(15, -1, -2)) + list(range(14, -1, -2))
                                    if lastblk
                                    else list(range(15, -1, -1))
                                )
                                for i in iorder:
                                    sb, so = i // 4, (i % 4) * 128
                                    npair = (i + 1) // 2
                                    odd = (i + 1) % 2
                                    pa = ps512.tile([128, 512], F32, name="pa", tag="p512")
                                    if dorow:
                                        pr = psR.tile([128, 1], F32, name="pr", tag="pr", bufs=1)
                                    for tp in range(npair):
                                        lhs = eT[:, TRI[sb] + 2 * tp : TRI[sb] + 2 * tp + 2, so : so + 128]
                                        last = (tp == npair - 1) and not odd
                                        nc.tensor.matmul(
                                            pa[:],
                                            lhs,
                                            v_sb[:, 2 * tp : 2 * tp + 2, fb * 512 : (fb + 1) * 512],
                                            start=(tp == 0),
                                            stop=last,
                                            perf_mode=DR,
                                        )
                                        if dorow:
                                            nc.tensor.matmul(
                                                pr[:],
                                                lhs,
                                                ones8[:],
                                                start=(tp == 0),
                                                stop=last,
                                                perf_mode=DR,
                                            )
                                    if odd:
                                        lhs1 = eT[:, TRI[sb] + i, so : so + 128]
                                        nc.tensor.matmul(
                                            pa[:],
                                            lhs1,
                                            v_sb[:, i, fb * 512 : (fb + 1) * 512],
                                            start=(npair == 0),
                                            stop=True,
                                        )
                                        if dorow:
                                            nc.tensor.matmul(
                                                pr[:],
                                                lhs1,
                                                ones8[:, 0, :],
                                                start=(npair == 0),
                                                stop=True,
                                            )
                                    if dorow:
                                        rsf = astg.tile([128, 1], F32, name="rsf", tag="rsf")
                                        nc.vector.tensor_copy(out=rsf[:], in_=pr[:])
                                        nc.vector.reciprocal(recips[:, h, i : i + 1], rsf[:])
                                    dst_acc = att_acc[fh][:, i, fb * 512 : (fb + 1) * 512]
                                    if h == 0:
                                        # first head: scaled output straight
                                        # into the on-chip accumulator
                                        nc.scalar.activation(
                                            dst_acc, pa[:], AF.Copy,
                                            scale=recips[:, h, i : i + 1],
                                        )
                                    else:
                                        stg = astg.tile([128, 512], F16, name="stg", tag="stg", bufs=8)
                                        nc.scalar.activation(
                                            stg[:], pa[:], AF.Copy,
                                            scale=recips[:, h, i : i + 1],
                                        )
                                        nc.vector.tensor_tensor(dst_acc, dst_acc, stg[:], AL.add)
                                    if finalh:
                                        if lastblk:
                                            # even chunk i=2j -> core j's first
                                            # 128 rows (half 0); odd -> half 1
                                            blk = i // 2
                                            nc.sync.dma_start(
                                                atts_in[fb][i % 2][blk * 128 : (blk + 1) * 128, :],
                                                dst_acc,
                                            )
                                            if i == 1:  # odd chunks all staged
                                                nc.gpsimd.collective_compute(
                                                    "ReduceScatter",
                                                    AL.add,
                                                    replica_groups=RG,
                                                    ins=[atts_in[fb][1][:]],
                                                    outs=[atts_out[fb][1][:]],
                                                )
                                        else:
                                            nc.sync.dma_start(
                                                att_in[fh][fb][i * 128 : (i + 1) * 128, :],
                                                dst_acc,
                                            )
                                if finalh:
                                    if lastblk:
                                        nc.gpsimd.collective_compute(
                                            "ReduceScatter",
                                            AL.add,
                                            replica_groups=RG,
                                            ins=[atts_in[fb][0][:]],
                                            outs=[atts_out[fb][0][:]],
                                        )
                                    else:
                                        nc.gpsimd.collective_compute(
                                            "ReduceScatter",
                                            AL.add,
                                            replica_groups=RG,
                                            ins=[att_in[fh][fb][:]],
                                            outs=[att_out[fh][fb][:]],
                                        )

            # =========== LN1 (global mean/var) ===========
            with tc.tile_pool(name="mid", bufs=1) as midp:
              h_own = midp.tile([128, 2, E], BF16, name="h_own")
              ysT = midp.tile([128, 16, RROWS], BF16, name="ysT")
              lngt = midp.tile([128, 2, E], BF16, name="lngt")
              lnbt = midp.tile([128, 2, E], BF16, name="lnbt")
              with tc.tile_pool(name="ln1", bufs=1) as lp, nc.named_scope("ln1"):
                  ys = lp.tile([128, 2, E], F32, name="ys")
                  ysb = lp.tile([128, 2, E], BF16, name="ysb")
                  yb1t = lp.tile([128, E], F32, name="yb1t")
                  nc.sync.dma_start(yb1t[:], yb1_d[:])
                  nc.sync.dma_start(lngt[:], lng_d.ap().rearrange("(t p) e -> p t e", p=128))
                  nc.sync.dma_start(lnbt[:], lnb_d.ap().rearrange("(t p) e -> p t e", p=128))
                  lngT = lp.tile([128, 16, RROWS], BF16, name="lngT")
                  nc.sync.dma_start(lngT[:], lngT_d.ap().rearrange("eo p r -> p eo r"))
                  lnbT = lp.tile([128, 16, RROWS], BF16, name="lnbT")
                  nc.sync.dma_start(lnbT[:], lnbT_d.ap().rearrange("eo p r -> p eo r"))
                  for rt in range(2):
                      xrt = lp.tile([128, E], F32, name="xrt", tag="xrt", bufs=2)
                      nc.sync.dma_start(xrt[:], xr_d[rt * 128 : (rt + 1) * 128, :])
                      nc.vector.tensor_tensor(ys[:, rt, :], xrt[:], yb1t[:], AL.add)
                  # accumulate RS chunks as they land; the last head's chunk
                  # completes a column block -> emit its stats partial
                  parts = lp.tile([128, 16], F32, name="parts")
                  sqs = lp.tile([128, 512], BF16, name="sqs", tag="sqs", bufs=2)
                  for fh in range(2):
                      for fb in range(2):
                          lastblk = fh == 1
                          # the odd-rows RS (rt=1) of the final block lands
                          # first; consume in landing order
                          for rt in ([1, 0] if lastblk else [0, 1]):
                              rof = lp.tile([128, 512], F16, name="rof", tag="rof", bufs=4)
                              if lastblk:
                                  nc.sync.dma_start(rof[:], atts_out[fb][rt][:])
                              else:
                                  nc.sync.dma_start(
                                      rof[:], att_out[fh][fb][rt * 128 : (rt + 1) * 128, :]
                                  )
                              col = fh * 1024 + fb * 512
                              dstv = ys[:, rt, col : col + 512]
                              nc.vector.tensor_tensor(dstv, dstv, rof[:], AL.add)
                              blk = (fh * 2 + fb) * 2 + rt
                              nc.vector.tensor_reduce(
                                  parts[:, blk : blk + 1], dstv, axis=AX.X, op=AL.add
                              )
                              nc.scalar.activation(
                                  sqs[:], dstv, AF.Square,
                                  accum_out=parts[:, 8 + blk : 9 + blk],
                              )
                  pstat = psT.tile([128, 128], F32, name="pstat", tag="pt", bufs=1)
                  nc.tensor.matmul(pstat[:1, :16], onesc[:, 0:1], parts[:], start=True, stop=True)
                  st4s = lp.tile([1, 16], F32, name="st4s")
                  nc.vector.tensor_copy(out=st4s[:], in_=pstat[:1, :16])
                  st4 = lp.tile([1, 8], F32, name="st4")
                  nc.vector.memset(st4[:], 0.0)
                  nc.vector.tensor_reduce(st4[:, 0:1], st4s[:, 0:8], axis=AX.X, op=AL.add)
                  nc.vector.tensor_reduce(st4[:, 1:2], st4s[:, 8:16], axis=AX.X, op=AL.add)
                  nc.sync.dma_start(st1_in[:], st4[:])
                  nc.gpsimd.collective_compute(
                      "AllReduce", AL.add, replica_groups=RG, ins=[st1_in[:]], outs=[st1_out[:]]
                  )
                  # transpose own rows (pre-affine) while the AllReduce flies
                  for rt in range(2):
                      nc.vector.tensor_copy(out=ysb[:, rt, :], in_=ys[:, rt, :])
                  for eo in range(16):
                      for rt in range(2):
                          pth = psT.tile([128, 128], BF16, name="pth", tag="ptT")
                          nc.tensor.transpose(
                              pth[:], ysb[:, rt, eo * 128 : (eo + 1) * 128], hidb[:]
                          )
                          if (eo + rt) % 2 == 0:
                              nc.vector.tensor_copy(
                                  out=ysT[:, eo, rt * 128 : (rt + 1) * 128], in_=pth[:]
                              )
                          else:
                              nc.scalar.copy(ysT[:, eo, rt * 128 : (rt + 1) * 128], pth[:])
                  bc = _ln_scalars(nc, lp, psT, onesr, st1_out)
                  # LN affine in transposed space, AllGather per own-row half
                  for hf in range(2):
                      src = ysT[:, :, hf * 128 : (hf + 1) * 128]
                      stT = lp.tile([128, 16, 128], BF16, name="hstgT", tag="hstgT", bufs=2)
                      nc.scalar.activation(
                          stT[:], src, AF.Identity, bias=bc[:, 0:1], scale=bc[:, 1:2]
                      )
                      nc.vector.tensor_tensor(
                          stT[:], stT[:], lngT[:, :, hf * 128 : (hf + 1) * 128], AL.mult
                      )
                      nc.vector.tensor_tensor(
                          stT[:], stT[:], lnbT[:, :, hf * 128 : (hf + 1) * 128], AL.add
                      )
                      nc.sync.dma_start(
                          agt_in[hf].rearrange("eo p r -> p eo r"), stT[:]
                      )
                      nc.gpsimd.collective_compute(
                          "AllGather",
                          AL.bypass,
                          replica_groups=RG,
                          ins=[agt_in[hf][:]],
                          outs=[agt_out[hf][:]],
                      )
                  # row-major h_own for the LN2 residual (off critical path)
                  ht_f32 = lp.tile([128, E], F32, name="ht_f32", tag="htf", bufs=2)
                  for rt in range(2):
                      nc.scalar.activation(
                          ht_f32[:], ys[:, rt, :], AF.Identity,
                          bias=bc[:, 0:1], scale=bc[:, 1:2],
                      )
                      nc.vector.tensor_tensor(ht_f32[:], ht_f32[:], lngt[:, rt, :], AL.mult)
                      nc.vector.tensor_tensor(h_own[:, rt, :], ht_f32[:], lnbt[:, rt, :], AL.add)

              # =========== FFN (hidden shard 1024) ===========
              # hT's S axis is PERMUTED: col = hf*1024 + c*128 + r maps to
              # true row c*256 + hf*128 + r. FFN1/FFN2 are elementwise along
              # S so only the final staging DMA needs to undo the mapping.
              with tc.tile_pool(name="ffn", bufs=1) as fp, nc.named_scope("ffn"):
                  hT = fp.tile([128, 16, S], BF16, name="hT")
                  zT = fp.tile([128, 8, S], BF16, name="zT")
                  w1full = fp.tile([128, 8, 2048], BF16, name="w1full")
                  for ft in range(8):
                      nc.sync.dma_start(w1full[:, ft, :], w1_d[:, ft, :])
                  with tc.tile_pool(name="wst", bufs=2) as wst:
                      for hf in range(2):
                          for c in range(NCORES):
                              nc.sync.dma_start(
                                  hT[:, :, hf * 1024 + c * 128 : hf * 1024 + (c + 1) * 128],
                                  agt_out[hf][c * 16 : (c + 1) * 16].rearrange("eo p r -> p eo r"),
                              )
                      # FFN1 per gathered S-half: starts on half 0 while
                      # half 1 is still gathering
                      for hf in range(2):
                          for ft in range(8):
                              pzs = [
                                  ps512.tile([128, 512], F32, name=f"pz{sb}", tag="p512")
                                  for sb in range(2)
                              ]
                              for eo in range(16):
                                  for sb in range(2):
                                      nc.tensor.matmul(
                                          pzs[sb][:],
                                          w1full[:, ft, eo * KD : (eo + 1) * KD],
                                          hT[:, eo, hf * 1024 + sb * 512 : hf * 1024 + (sb + 1) * 512],
                                          start=(eo == 0),
                                          stop=(eo == 15),
                                      )
                              for sb in range(2):
                                  nc.scalar.activation(
                                      zT[:, ft, hf * 1024 + sb * 512 : hf * 1024 + (sb + 1) * 512],
                                      pzs[sb][:],
                                      AF.Relu,
                                      bias=b1_sb[:, ft : ft + 1],
                                  )
                      for ebx, (c0, w) in enumerate(EBW):
                          w2b = wst.tile([128, 8, w], BF16, name="w2b", tag="w2b")
                          nc.sync.dma_start(w2b[:], w2_d[:, :, c0 // 512, :])
                          for p in range(16):
                              pf = ps512.tile([128, w], F32, name="pf", tag="p512")
                              for fc in range(8):
                                  nc.tensor.matmul(
                                      pf[:],
                                      zT[:, fc, p * 128 : (p + 1) * 128],
                                      w2b[:, fc, :],
                                      start=(fc == 0),
                                      stop=(fc == 7),
                                  )
                              fstg = wst.tile([128, w], F16, name="fstg", tag="fstg", bufs=4)
                              nc.scalar.activation(fstg[:], pf[:], AF.Copy)
                              hf, cc = p // 8, p % 8
                              r0 = cc * 256 + hf * 128
                              nc.sync.dma_start(ffn_in[ebx][r0 : r0 + 128, :], fstg[:])
                          nc.gpsimd.collective_compute(
                              "ReduceScatter",
                              AL.add,
                              replica_groups=RG,
                              ins=[ffn_in[ebx][:]],
                              outs=[ffn_out[ebx][:]],
                          )

              # =========== LN2 + output ===========
              with tc.tile_pool(name="ln2", bufs=1) as l2, nc.named_scope("ln2"):
                  ys2 = l2.tile([128, 2, E], F32, name="ys2")
                  yb2t = l2.tile([128, E], F32, name="yb2t")
                  nc.sync.dma_start(yb2t[:], yb2_d[:])
                  for rt in range(2):
                      nc.vector.tensor_tensor(
                          ys2[:, rt, :], h_own[:, rt, :], yb2t[:], AL.add
                      )
                  parts2 = l2.tile([128, 16], F32, name="parts2")
                  sqs2 = l2.tile([128, 512], BF16, name="sqs2", tag="sqs2", bufs=2)
                  for ebx, (c0, w) in enumerate(EBW):
                      for rt in range(2):
                          fot = l2.tile([128, w], F16, name="fot", tag="fot", bufs=4)
                          nc.sync.dma_start(fot[:], ffn_out[ebx][rt * 128 : (rt + 1) * 128, :])
                          dstv = ys2[:, rt, c0 : c0 + w]
                          nc.vector.tensor_tensor(dstv, dstv, fot[:], AL.add)
                          blk = ebx * 2 + rt
                          nc.vector.tensor_reduce(
                              parts2[:, blk : blk + 1], dstv, axis=AX.X, op=AL.add
                          )
                          nc.scalar.activation(
                              sqs2[:, 0:w], dstv, AF.Square,
                              accum_out=parts2[:, 8 + blk : 9 + blk],
                          )
                  pstat2 = psT.tile([128, 128], F32, name="pstat2", tag="pt", bufs=1)
                  nc.tensor.matmul(pstat2[:1, :16], onesc[:, 0:1], parts2[:], start=True, stop=True)
                  st4s2 = l2.tile([1, 16], F32, name="st4s2")
                  nc.vector.tensor_copy(out=st4s2[:], in_=pstat2[:1, :16])
                  st42 = l2.tile([1, 8], F32, name="st42")
                  nc.vector.memset(st42[:], 0.0)
                  nc.vector.tensor_reduce(st42[:, 0:1], st4s2[:, 0:8], axis=AX.X, op=AL.add)
                  nc.vector.tensor_reduce(st42[:, 1:2], st4s2[:, 8:16], axis=AX.X, op=AL.add)
                  nc.sync.dma_start(st2_in[:], st42[:])
                  nc.gpsimd.collective_compute(
                      "AllReduce", AL.add, replica_groups=RG, ins=[st2_in[:]], outs=[st2_out[:]]
                  )
                  bc2 = _ln_scalars(nc, l2, psT, onesr, st2_out)
                  for rt in range(2):
                      ot = l2.tile([128, E], F32, name="ot", tag="ot", bufs=2)
                      nc.scalar.activation(
                          ot[:],
                          ys2[:, rt, :],
                          AF.Identity,
                          bias=bc2[:, 0:1],
                          scale=bc2[:, 1:2],
                      )
                      nc.vector.tensor_tensor(ot[:], ot[:], lngt[:, rt, :], AL.mult)
                      nc.vector.tensor_tensor(ot[:], ot[:], lnbt[:, rt, :], AL.add)
                      nc.sync.dma_start(out_d[rt * 128 : (rt + 1) * 128, :], ot[:])

    nc.compile()
    return nc


def _ln_scalars(nc, pool, psT, onesr, st_out):
    """AllReduced (sum, sumsq) -> bc [128, 2] = (-m*rstd, rstd) broadcast."""
    so = pool.tile([1, 8], F32, name="so", tag="so")
    nc.sync.dma_start(so[:], st_out[:])
    sc = pool.tile([1, 8], F32, name="sc", tag="sc")
    # sc0 = m, sc1 = E[y^2], sc2 = m^2, sc3 = var, sc4 = rstd, sc5 = -m*rstd
    nc.scalar.mul(sc[:, 0:1], so[:, 0:1], 1.0 / NTOT)
    nc.scalar.mul(sc[:, 1:2], so[:, 1:2], 1.0 / NTOT)
    nc.scalar.activation(sc[:, 2:3], sc[:, 0:1], AF.Square)
    nc.vector.tensor_tensor(sc[:, 3:4], sc[:, 1:2], sc[:, 2:3], AL.subtract)
    nc.vector.tensor_scalar_add(sc[:, 2:3], sc[:, 3:4], EPS)  # var + eps
    # rstd = 1/sqrt(var+eps): Sqrt on ACT (sqrt table also carries square/
    # identity/copy, so no further reloads), reciprocal on DVE (table-free)
    nc.scalar.activation(sc[:, 6:7], sc[:, 2:3], AF.Sqrt)
    nc.vector.reciprocal(sc[:, 4:5], sc[:, 6:7])
    nc.vector.tensor_tensor(sc[:, 7:8], sc[:, 0:1], sc[:, 4:5], AL.mult)
    nc.scalar.mul(sc[:, 5:6], sc[:, 7:8], -1.0)
    s2 = pool.tile([1, 2], F32, name="s2", tag="s2")
    nc.vector.tensor_copy(out=s2[:, 0:1], in_=sc[:, 5:6])
    nc.vector.tensor_copy(out=s2[:, 1:2], in_=sc[:, 4:5])
    pb = psT.tile([128, 128], F32, name="pb", tag="pt", bufs=1)
    nc.tensor.matmul(pb[:, :2], onesr[:], s2[:], start=True, stop=True)
    bc = pool.tile([128, 2], F32, name="bc", tag="bc")
    nc.vector.tensor_copy(out=bc[:], in_=pb[:, :2])
    return bc


_NC_CACHE = None


def _get_nc():
    global _NC_CACHE
    if _NC_CACHE is None:
        _NC_CACHE = _build()
    return _NC_CACHE


def _prep_core(c, inputs):
    f32 = np.float32
    x = np.ascontiguousarray(inputs["input"], dtype=f32)
    Wq, Wk, Wv = inputs["Wq"], inputs["Wk"], inputs["Wv"]
    bq, bk, bv = inputs["bq"], inputs["bk"], inputs["bv"]
    W1, b1, W2, b2 = inputs["W1"], inputs["b1"], inputs["W2"], inputs["b2"]
    ln_g, ln_b = inputs["ln_g"], inputs["ln_b"]
    h0 = c * HPC
    wqt = np.ascontiguousarray(
        np.stack(
            [Wq[h0 + h].reshape(16, 128, KD).transpose(1, 0, 2) for h in range(HPC)]
        ).astype(_bf16)
    )
    wkt = np.ascontiguousarray(
        np.stack(
            [Wk[h0 + h].reshape(16, 128, KD).transpose(1, 0, 2) for h in range(HPC)]
        ).astype(_bf16)
    )
    wvt = np.ascontiguousarray(
        np.stack(
            [Wv[h0 + h].reshape(16, 128, E).transpose(1, 0, 2) for h in range(HPC)]
        ).astype(_f8)
    )
    W1s = W1[:, c * FSH : (c + 1) * FSH]
    w1t = np.ascontiguousarray(
        W1s.reshape(16, 128, 8, 128).transpose(1, 2, 0, 3).reshape(128, 8, 2048).astype(_bf16)
    )
    W2s = W2[c * FSH : (c + 1) * FSH, :]
    w2t = np.ascontiguousarray(
        W2s.reshape(8, 128, 4, 512).transpose(1, 0, 2, 3).astype(_bf16)
    )
    bqs = np.ascontiguousarray((bq[h0 : h0 + HPC] * ISCALE).T, dtype=f32)
    bks = np.ascontiguousarray(bk[h0 : h0 + HPC].T, dtype=f32)
    b1s = np.ascontiguousarray(b1[c * FSH : (c + 1) * FSH].reshape(8, 128).T, dtype=f32)
    yb1 = np.ascontiguousarray(np.broadcast_to(bv.sum(axis=0), (128, E)), dtype=f32)
    yb2 = np.ascontiguousarray(np.broadcast_to(b2, (128, E)), dtype=f32)
    rows = slice(c * RROWS, (c + 1) * RROWS)
    jj, tp, sf = np.meshgrid(
        np.arange(4), np.arange(128), np.arange(512), indexing="ij"
    )
    mask = (((128 * jj + tp) <= sf).astype(f32) - 1.0) * 30.0
    xt = x.T.reshape(16, 128, S).transpose(1, 0, 2)
    xtb = np.ascontiguousarray(xt.astype(_bf16))
    xt8 = np.ascontiguousarray(xt.astype(_f8))
    return {
        "xtb": xtb,
        "xt8": xt8,
        "wqt": wqt,
        "wkt": wkt,
        "wvt": wvt,
        "w1t": w1t,
        "w2t": w2t,
        "bqs": bqs,
        "bks": bks,
        "b1s": b1s,
        "yb1": yb1,
        "yb2": yb2,
        "xr": np.ascontiguousarray(x[rows], dtype=f32),
        "lngr": np.ascontiguousarray(np.asarray(ln_g[rows], dtype=f32).astype(_bf16)),
        "lnbr": np.ascontiguousarray(np.asarray(ln_b[rows], dtype=f32).astype(_bf16)),
        "lngT": np.ascontiguousarray(
            np.asarray(ln_g[rows], dtype=f32).T.reshape(16, 128, RROWS).astype(_bf16)
        ),
        "lnbT": np.ascontiguousarray(
            np.asarray(ln_b[rows], dtype=f32).T.reshape(16, 128, RROWS).astype(_bf16)
        ),
        "mask": np.ascontiguousarray(mask),
        "ident": np.eye(128, dtype=f32),
        "ones": np.ones((128, 8), dtype=f32),
        "onesr": np.ones((1, 128), dtype=f32),
    }


def kernel(**inputs):
    nc = _get_nc()
    inputs = {k: np.asarray(v, dtype=np.float32) for k, v in inputs.items()}
    in_maps = [_prep_core(c, inputs) for c in range(NCORES)]
    res = run_bass_kernel_spmd(nc, in_maps, core_ids=list(range(NCORES)))
    out = np.concatenate([res.results[c]["out"] for c in range(NCORES)], axis=0)
    return np.ascontiguousarray(out, dtype=np.float32)



# revision 44
# speedup vs baseline: 1.4523x; 1.0051x over previous
"""Tensor-parallel decoder layer on 8 TRN2 NeuronCores.

Sharding:
  - Attention: 16 heads -> 2 per core. Per-core partial attn_out is
    ReduceScattered (fp16) so core c owns rows [256c, 256c+256).
  - Global LayerNorm (scalar mean/var over the whole [S,E] tensor):
    per-core partial (sum, sumsq) AllReduced as a tiny fp32 tensor.
  - FFN: hidden dim 8192 -> 1024 per core; partial [S,E] output
    ReduceScattered per 512-column chunk (fp16), overlapping FFN2.
  - h is AllGathered transposed (bf16) since every matmul contracting
    over E needs h^T as the moving operand.

Matmul layout notes (PE computes out = lhsT.T @ rhs, contraction on the
partition dim):
  - x^T resident in SBUF (bf16 for Q/K, fp8e4 for V) feeds projections.
  - The value path runs in fp8e4 DoubleRow (2 k-tiles per instruction,
    2x MAC rate): V projection contracts eo pairs, attn@v contracts
    t-tile pairs. exp is shifted by a constant (exp(s - SH)) so scores
    fit fp8's +/-240 range; the shift cancels exactly in num/rowsum.
  - scores are built transposed: S^T[t,s] tiles, so exp(S^T) tiles feed
    attn@v directly as lhsT with no transposes; softmax normalization is
    deferred: rowsum via a fp8 ones-column matmul, applied as a
    per-partition scale on the PSUM->SBUF copy of attn@v output.
  - causal masking folds into the score: a -30 additive mask lands on
    the PSUM tile before the exp, so masked entries underflow to 0.
"""

import math
import sys

sys.path.insert(0, "/opt/trn_rl_repo")

import numpy as np
import ml_dtypes

_bf16 = ml_dtypes.bfloat16
_f8 = ml_dtypes.float8_e4m3

import concourse.bass as bass
import concourse.mybir as mybir
import concourse.tile as tile
from concourse import bacc
from concourse.bass_utils import run_bass_kernel_spmd

S, E, H, KD, FF = 2048, 2048, 16, 128, 8192
EPS = 1e-5
NCORES = 8
HPC = H // NCORES          # heads per core = 2
FSH = FF // NCORES         # ffn hidden shard = 1024
RROWS = S // NCORES        # row shard = 256
NTOT = float(S * E)
ISCALE = 1.0 / math.sqrt(KD)

F32 = mybir.dt.float32
BF16 = mybir.dt.bfloat16
F16 = mybir.dt.float16
FP8 = mybir.dt.float8e4
AF = mybir.ActivationFunctionType
AL = mybir.AluOpType
AX = mybir.AxisListType
DR = mybir.MatmulPerfMode.DoubleRow

# packed triangular offsets for eT tiles: tile(tc, sb) at TRI[sb] + tc
TRI = [0, 4, 12, 24]
NTRI = 40
SH = 2.5  # constant exp shift: keeps exp(score - SH) within fp8e4 range


def _build():
    nc = bacc.Bacc(
        "TRN2",
        target_bir_lowering=False,
        debug=False,
        enable_asserts=True,
        num_devices=NCORES,
    )

    # ---- external I/O (per-core shards prepared on the host) ----
    xtb_d = nc.dram_tensor("xtb", [128, 16, S], BF16, kind="ExternalInput")
    xt8_d = nc.dram_tensor("xt8", [128, 16, S], FP8, kind="ExternalInput")
    wq_d = nc.dram_tensor("wqt", [HPC, 128, 16, KD], BF16, kind="ExternalInput")
    wk_d = nc.dram_tensor("wkt", [HPC, 128, 16, KD], BF16, kind="ExternalInput")
    wv_d = nc.dram_tensor("wvt", [HPC, 128, 16, E], FP8, kind="ExternalInput")
    w1_d = nc.dram_tensor("w1t", [128, 8, 2048], BF16, kind="ExternalInput")
    w2_d = nc.dram_tensor("w2t", [128, 8, 4, 512], BF16, kind="ExternalInput")
    bq_d = nc.dram_tensor("bqs", [128, HPC], F32, kind="ExternalInput")
    bk_d = nc.dram_tensor("bks", [128, HPC], F32, kind="ExternalInput")
    b1_d = nc.dram_tensor("b1s", [128, 8], F32, kind="ExternalInput")
    yb1_d = nc.dram_tensor("yb1", [128, E], F32, kind="ExternalInput")
    yb2_d = nc.dram_tensor("yb2", [128, E], F32, kind="ExternalInput")
    xr_d = nc.dram_tensor("xr", [RROWS, E], F32, kind="ExternalInput")
    lng_d = nc.dram_tensor("lngr", [RROWS, E], BF16, kind="ExternalInput")
    lnb_d = nc.dram_tensor("lnbr", [RROWS, E], BF16, kind="ExternalInput")
    lngT_d = nc.dram_tensor("lngT", [16, 128, RROWS], BF16, kind="ExternalInput")
    lnbT_d = nc.dram_tensor("lnbT", [16, 128, RROWS], BF16, kind="ExternalInput")
    mask_d = nc.dram_tensor("mask", [4, 128, 512], F32, kind="ExternalInput")
    id_d = nc.dram_tensor("ident", [128, 128], F32, kind="ExternalInput")
    ones_d = nc.dram_tensor("ones", [128, 8], F32, kind="ExternalInput")
    onesr_d = nc.dram_tensor("onesr", [1, 128], F32, kind="ExternalInput")
    out_d = nc.dram_tensor("out", [RROWS, E], F32, kind="ExternalOutput")

    RG = [list(range(NCORES))]

    with tile.TileContext(nc) as tc:
        with (
            tc.tile_pool(name="persist", bufs=1) as pp,
            tc.tile_pool(name="dram", bufs=1, space="DRAM") as dp,
            tc.tile_pool(name="ps512", bufs=5, space="PSUM") as ps512,
            tc.tile_pool(name="psT", bufs=1, space="PSUM") as psT,
            tc.tile_pool(name="psR", bufs=1, space="PSUM") as psR,
        ):
            # ---- collective bounce buffers (internal DRAM) ----
            att_in = [
                [
                    dp.tile([S, 512], F16, name=f"att_in_{fh}_{fb}", tag=f"ati{fh}{fb}")
                    for fb in range(2)
                ]
                for fh in range(2)
            ]
            att_out = [
                [
                    dp.tile([RROWS, 512], F16, name=f"att_out_{fh}_{fb}", tag=f"ato{fh}{fb}")
                    for fb in range(2)
                ]
                for fh in range(2)
            ]

            st1_in = dp.tile([1, 8], F32, name="st1_in", tag="st1i")
            st1_out = dp.tile([1, 8], F32, name="st1_out", tag="st1o", addr_space="Shared")
            st2_in = dp.tile([1, 8], F32, name="st2_in", tag="st2i")
            st2_out = dp.tile([1, 8], F32, name="st2_out", tag="st2o", addr_space="Shared")
            # transposed h AllGather, split by own-row halves so FFN1 can
            # start on the first half while the second gathers
            agt_in = [
                dp.tile([16, 128, 128], BF16, name=f"agt_in{j}", tag=f"agti{j}")
                for j in range(2)
            ]
            agt_out = [
                dp.tile([128, 128, 128], BF16, name=f"agt_out{j}", tag=f"agto{j}", addr_space="Shared")
                for j in range(2)
            ]
            # ffn output chunks: col offset/width
            EBW = [(0, 512), (512, 512), (1024, 512), (1536, 512)]
            ffn_in = [
                dp.tile([S, w], F16, name=f"ffn_in_{eb}", tag=f"ffi{eb}")
                for eb, (c0, w) in enumerate(EBW)
            ]
            ffn_out = [
                dp.tile([RROWS, w], F16, name=f"ffn_out_{eb}", tag=f"ffo{eb}")
                for eb, (c0, w) in enumerate(EBW)
            ]

            # ---- persistent small tiles ----
            ident = pp.tile([128, 128], F32, name="ident")
            nc.sync.dma_start(ident[:], id_d[:])
            hidb = pp.tile([128, 128], BF16, name="hidb")
            nc.vector.tensor_copy(out=hidb[:], in_=ident[:])
            onesc = pp.tile([128, 8], F32, name="onesc")
            nc.sync.dma_start(onesc[:], ones_d[:])
            onesr = pp.tile([1, 128], F32, name="onesr")
            nc.sync.dma_start(onesr[:], onesr_d[:])
            bq_sb = pp.tile([128, HPC], F32, name="bq_sb")
            nc.sync.dma_start(bq_sb[:], bq_d[:])
            bk_sb = pp.tile([128, HPC], F32, name="bk_sb")
            nc.sync.dma_start(bk_sb[:], bk_d[:])
            b1_sb = pp.tile([128, 8], F32, name="b1_sb")
            nc.sync.dma_start(b1_sb[:], b1_d[:])
            qkT = pp.tile([128, 2, HPC, S], BF16, name="qkT")  # [d, q/k, head, s]
            recips = pp.tile([128, HPC, 16], F32, name="recips")

            # =========== phase 0 + attention ===========
            with tc.tile_pool(name="attn", bufs=1) as ap_:
                xT8 = ap_.tile([128, 16, S], FP8, name="xT8")  # x^T fp8 for V
                maskf = ap_.tile([128, 4, 512], F32, name="maskf")  # (mask-1)*30
                shm = ap_.tile([128, 1], F32, name="shm")  # exp shift bias
                ones8 = ap_.tile([128, 2, 1], FP8, name="ones8")
                with tc.tile_pool(name="qkx", bufs=1) as qkx:
                    xT = qkx.tile([128, 16, S], BF16, name="xT")  # x^T bf16 for Q/K
                    wqk = [
                        [
                            qkx.tile([128, 16, KD], BF16, name=f"wqk{h}{qi}")
                            for qi in range(2)
                        ]
                        for h in range(HPC)
                    ]
                    with tc.tile_pool(name="prep", bufs=1) as prep, nc.named_scope("prep"):
                        # HAM warmup: back-to-back matmuls to unthrottle PE,
                        # issued while the first weight/x DMAs land
                        wtile = prep.tile([128, 512], BF16, name="wtile", tag="wtile", bufs=1)
                        nc.vector.memset(wtile[:], 0.0)
                        nc.vector.memset(shm[:], -SH)
                        nc.vector.memset(ones8[:], 1.0)
                        # interleave the first x^T chunks with the small
                        # weight DMAs so qkproj's first matmul gates on
                        # neither the bulk x transfer nor a queued weight
                        qkdmas = [
                            (wqk[h][qi], (wq_d if qi == 0 else wk_d)[h])
                            for h in range(HPC)
                            for qi in range(2)
                        ]
                        for eo in range(4):
                            nc.sync.dma_start(xT[:, eo, :], xtb_d[:, eo, :])
                            dst, src = qkdmas[eo]
                            nc.sync.dma_start(dst[:], src)
                        for _w in range(24):
                            pw = ps512.tile([128, 512], F32, name="pw", tag="p512")
                            nc.tensor.matmul(pw[:], wtile[:, :128], wtile[:], start=True, stop=True)
                        for eo in range(4, 16):
                            nc.sync.dma_start(xT[:, eo, :], xtb_d[:, eo, :])
                        nc.sync.dma_start(maskf[:], mask_d.ap().rearrange("j p s -> p j s"))
                        for eo in range(16):
                            nc.sync.dma_start(xT8[:, eo, :], xt8_d[:, eo, :])

                    # ---- Q/K projections for both heads (scaled/biased) ----
                    with nc.named_scope("qkproj"):
                        for h in range(HPC):
                            for qi, (b_sb, scl) in enumerate(
                                ((bq_sb, ISCALE), (bk_sb, 1.0))
                            ):
                                wb = wqk[h][qi]
                                pqs = [
                                    ps512.tile([128, 512], F32, name=f"pq{sb}", tag="p512")
                                    for sb in range(4)
                                ]
                                for eo in range(16):
                                    for sb in range(4):
                                        nc.tensor.matmul(
                                            pqs[sb][:],
                                            wb[:, eo, :],
                                            xT[:, eo, sb * 512 : (sb + 1) * 512],
                                            start=(eo == 0),
                                            stop=(eo == 15),
                                        )
                                for sb in range(4):
                                    nc.scalar.activation(
                                        qkT[:, qi, h, sb * 512 : (sb + 1) * 512],
                                        pqs[sb][:],
                                        AF.Identity,
                                        bias=b_sb[:, h : h + 1],
                                        scale=scl,
                                    )

                # ---- per-head attention (value path in fp8 DoubleRow) ----
                eT = ap_.tile([128, NTRI, 512], FP8, name="eT")
                v_sb = ap_.tile([128, 16, FSH], FP8, name="v_sb")
                with (
                    tc.tile_pool(name="acc", bufs=1) as accp,
                    tc.tile_pool(name="wvb", bufs=2) as wvbp,
                    tc.tile_pool(name="astg", bufs=4) as astg,
                ):
                    # both local heads sum on-chip here before the
                    # collective, halving attention ReduceScatter traffic
                    att_acc = [
                        accp.tile([128, 16, FSH], F16, name=f"att_acc{fh}")
                        for fh in range(2)
                    ]
                    for h in range(HPC):
                      with nc.named_scope(f"scores{h}"):
                        for sb in range(4):
                            for tcn in range(4 * sb + 4):
                                psc = ps512.tile([128, 512], F32, name="psc", tag="p512")
                                nc.tensor.matmul(
                                    psc[:],
                                    qkT[:, 1, h, tcn * 128 : (tcn + 1) * 128],
                                    qkT[:, 0, h, sb * 512 : (sb + 1) * 512],
                                    start=True,
                                    stop=True,
                                )
                                if tcn >= 4 * sb:
                                    # diagonal tile: -30 mask onto PSUM, then
                                    # masked entries underflow to 0 in the exp
                                    nc.vector.tensor_tensor(
                                        psc[:], psc[:], maskf[:, tcn - 4 * sb, :], AL.add
                                    )
                                nc.scalar.activation(
                                    eT[:, TRI[sb] + tcn, :], psc[:], AF.Exp, bias=shm[:]
                                )

                        # pass B: per f-half: v-projection then attn@v
                        for fh in range(2):
                          with nc.named_scope(f"vproj{h}{fh}"):
                            for fb in range(2):
                                wvb = wvbp.tile([128, 16, 512], FP8, name="wvb", tag="wvb")
                                nc.sync.dma_start(
                                    wvb[:],
                                    wv_d[h][
                                        :, :, fh * 1024 + fb * 512 : fh * 1024 + (fb + 1) * 512
                                    ],
                                )
                                for tcn in range(16):
                                    pv = ps512.tile([128, 512], F32, name="pv", tag="p512")
                                    for j in range(8):
                                        nc.tensor.matmul(
                                            pv[:],
                                            xT8[:, 2 * j : 2 * j + 2, tcn * 128 : (tcn + 1) * 128],
                                            wvb[:, 2 * j : 2 * j + 2, :],
                                            start=(j == 0),
                                            stop=(j == 7),
                                            perf_mode=DR,
                                        )
                                    if tcn % 2 == 0:
                                        nc.vector.tensor_copy(
                                            out=v_sb[:, tcn, fb * 512 : (fb + 1) * 512],
                                            in_=pv[:],
                                        )
                                    else:
                                        nc.scalar.copy(
                                            v_sb[:, tcn, fb * 512 : (fb + 1) * 512], pv[:]
                                        )

                          with nc.named_scope(f"attnv{h}{fh}"):
                            # fb-outer: each 512-col half ReduceScatters as
                            # soon as it is complete, halving the exposed
                            # tail after the last attnv block
                            finalh = h == HPC - 1
                            for fb in range(2):
                                dorow = fh == 0 and fb == 0
                                for i in range(15, -1, -1):
                                    sb, so = i // 4, (i % 4) * 128
                                    npair = (i + 1) // 2
                                    odd = (i + 1) % 2
                                    pa = ps512.tile([128, 512], F32, name="pa", tag="p512")
                                    if dorow:
                                        pr = psR.tile([128, 1], F32, name="pr", tag="pr", bufs=1)
                                    for tp in range(npair):
                                        lhs = eT[:, TRI[sb] + 2 * tp : TRI[sb] + 2 * tp + 2, so : so + 128]
                                        last = (tp == npair - 1) and not odd
                                        nc.tensor.matmul(
                                            pa[:],
                                            lhs,
                                            v_sb[:, 2 * tp : 2 * tp + 2, fb * 512 : (fb + 1) * 512],
                                            start=(tp == 0),
                                            stop=last,
                                            perf_mode=DR,
                                        )
                                        if dorow:
                                            nc.tensor.matmul(
                                                pr[:],
                                                lhs,
                                                ones8[:],
                                                start=(tp == 0),
                                                stop=last,
                                                perf_mode=DR,
                                            )
                                    if odd:
                                        lhs1 = eT[:, TRI[sb] + i, so : so + 128]
                                        nc.tensor.matmul(
                                            pa[:],
                                            lhs1,
                                            v_sb[:, i, fb * 512 : (fb + 1) * 512],
                                            start=(npair == 0),
                                            stop=True,
                                        )
                                        if dorow:
                                            nc.tensor.matmul(
                                                pr[:],
                                                lhs1,
                                                ones8[:, 0, :],
                                                start=(npair == 0),
                                                stop=True,
                                            )
                                    if dorow:
                                        rsf = astg.tile([128, 1], F32, name="rsf", tag="rsf")
                                        nc.vector.tensor_copy(out=rsf[:], in_=pr[:])
                                        nc.vector.reciprocal(recips[:, h, i : i + 1], rsf[:])
                                    dst_acc = att_acc[fh][:, i, fb * 512 : (fb + 1) * 512]
                                    if h == 0:
                                        # first head: scaled output straight
                                        # into the on-chip accumulator
                                        nc.scalar.activation(
                                            dst_acc, pa[:], AF.Copy,
                                            scale=recips[:, h, i : i + 1],
                                        )
                                    else:
                                        stg = astg.tile([128, 512], F16, name="stg", tag="stg", bufs=8)
                                        nc.scalar.activation(
                                            stg[:], pa[:], AF.Copy,
                                            scale=recips[:, h, i : i + 1],
                                        )
                                        nc.vector.tensor_tensor(dst_acc, dst_acc, stg[:], AL.add)
                                    if finalh:
                                        nc.sync.dma_start(
                                            att_in[fh][fb][i * 128 : (i + 1) * 128, :],
                                            dst_acc,
                                        )
                                if finalh:
                                    nc.gpsimd.collective_compute(
                                        "ReduceScatter",
                                        AL.add,
                                        replica_groups=RG,
                                        ins=[att_in[fh][fb][:]],
                                        outs=[att_out[fh][fb][:]],
                                    )

            # =========== LN1 (global mean/var) ===========
            with tc.tile_pool(name="mid", bufs=1) as midp:
              h_own = midp.tile([128, 2, E], BF16, name="h_own")
              ysT = midp.tile([128, 16, RROWS], BF16, name="ysT")
              lngt = midp.tile([128, 2, E], BF16, name="lngt")
              lnbt = midp.tile([128, 2, E], BF16, name="lnbt")
              with tc.tile_pool(name="ln1", bufs=1) as lp, nc.named_scope("ln1"):
                  ys = lp.tile([128, 2, E], F32, name="ys")
                  ysb = lp.tile([128, 2, E], BF16, name="ysb")
                  yb1t = lp.tile([128, E], F32, name="yb1t")
                  nc.sync.dma_start(yb1t[:], yb1_d[:])
                  nc.sync.dma_start(lngt[:], lng_d.ap().rearrange("(t p) e -> p t e", p=128))
                  nc.sync.dma_start(lnbt[:], lnb_d.ap().rearrange("(t p) e -> p t e", p=128))
                  lngT = lp.tile([128, 16, RROWS], BF16, name="lngT")
                  nc.sync.dma_start(lngT[:], lngT_d.ap().rearrange("eo p r -> p eo r"))
                  lnbT = lp.tile([128, 16, RROWS], BF16, name="lnbT")
                  nc.sync.dma_start(lnbT[:], lnbT_d.ap().rearrange("eo p r -> p eo r"))
                  for rt in range(2):
                      xrt = lp.tile([128, E], F32, name="xrt", tag="xrt", bufs=2)
                      nc.sync.dma_start(xrt[:], xr_d[rt * 128 : (rt + 1) * 128, :])
                      nc.vector.tensor_tensor(ys[:, rt, :], xrt[:], yb1t[:], AL.add)
                  # accumulate RS chunks as they land; the last head's chunk
                  # completes a column block -> emit its stats partial
                  parts = lp.tile([128, 16], F32, name="parts")
                  sqs = lp.tile([128, 512], BF16, name="sqs", tag="sqs", bufs=2)
                  for fh in range(2):
                      for fb in range(2):
                          for rt in range(2):
                              rof = lp.tile([128, 512], F16, name="rof", tag="rof", bufs=4)
                              nc.sync.dma_start(
                                  rof[:], att_out[fh][fb][rt * 128 : (rt + 1) * 128, :]
                              )
                              col = fh * 1024 + fb * 512
                              dstv = ys[:, rt, col : col + 512]
                              nc.vector.tensor_tensor(dstv, dstv, rof[:], AL.add)
                              blk = (fh * 2 + fb) * 2 + rt
                              nc.vector.tensor_reduce(
                                  parts[:, blk : blk + 1], dstv, axis=AX.X, op=AL.add
                              )
                              nc.scalar.activation(
                                  sqs[:], dstv, AF.Square,
                                  accum_out=parts[:, 8 + blk : 9 + blk],
                              )
                  pstat = psT.tile([128, 128], F32, name="pstat", tag="pt", bufs=1)
                  nc.tensor.matmul(pstat[:1, :16], onesc[:, 0:1], parts[:], start=True, stop=True)
                  st4s = lp.tile([1, 16], F32, name="st4s")
                  nc.vector.tensor_copy(out=st4s[:], in_=pstat[:1, :16])
                  st4 = lp.tile([1, 8], F32, name="st4")
                  nc.vector.memset(st4[:], 0.0)
                  nc.vector.tensor_reduce(st4[:, 0:1], st4s[:, 0:8], axis=AX.X, op=AL.add)
                  nc.vector.tensor_reduce(st4[:, 1:2], st4s[:, 8:16], axis=AX.X, op=AL.add)
                  nc.sync.dma_start(st1_in[:], st4[:])
                  nc.gpsimd.collective_compute(
                      "AllReduce", AL.add, replica_groups=RG, ins=[st1_in[:]], outs=[st1_out[:]]
                  )
                  # transpose own rows (pre-affine) while the AllReduce
                  # flies; rt=0 first so the first AllGather half (which
                  # only carries rt=0 rows) is gated on half the work
                  for rt in range(2):
                      nc.vector.tensor_copy(out=ysb[:, rt, :], in_=ys[:, rt, :])
                      for eo in range(16):
                          pth = psT.tile([128, 128], BF16, name="pth", tag="ptT")
                          nc.tensor.transpose(
                              pth[:], ysb[:, rt, eo * 128 : (eo + 1) * 128], hidb[:]
                          )
                          if eo % 2 == 0:
                              nc.vector.tensor_copy(
                                  out=ysT[:, eo, rt * 128 : (rt + 1) * 128], in_=pth[:]
                              )
                          else:
                              nc.scalar.copy(ysT[:, eo, rt * 128 : (rt + 1) * 128], pth[:])
                  bc = _ln_scalars(nc, lp, psT, onesr, st1_out)
                  # LN affine in transposed space, AllGather per own-row half
                  for hf in range(2):
                      src = ysT[:, :, hf * 128 : (hf + 1) * 128]
                      stT = lp.tile([128, 16, 128], BF16, name="hstgT", tag="hstgT", bufs=2)
                      nc.scalar.activation(
                          stT[:], src, AF.Identity, bias=bc[:, 0:1], scale=bc[:, 1:2]
                      )
                      nc.vector.tensor_tensor(
                          stT[:], stT[:], lngT[:, :, hf * 128 : (hf + 1) * 128], AL.mult
                      )
                      nc.vector.tensor_tensor(
                          stT[:], stT[:], lnbT[:, :, hf * 128 : (hf + 1) * 128], AL.add
                      )
                      nc.sync.dma_start(
                          agt_in[hf].rearrange("eo p r -> p eo r"), stT[:]
                      )
                      nc.gpsimd.collective_compute(
                          "AllGather",
                          AL.bypass,
                          replica_groups=RG,
                          ins=[agt_in[hf][:]],
                          outs=[agt_out[hf][:]],
                      )
                  # row-major h_own for the LN2 residual (off critical path)
                  ht_f32 = lp.tile([128, E], F32, name="ht_f32", tag="htf", bufs=2)
                  for rt in range(2):
                      nc.scalar.activation(
                          ht_f32[:], ys[:, rt, :], AF.Identity,
                          bias=bc[:, 0:1], scale=bc[:, 1:2],
                      )
                      nc.vector.tensor_tensor(ht_f32[:], ht_f32[:], lngt[:, rt, :], AL.mult)
                      nc.vector.tensor_tensor(h_own[:, rt, :], ht_f32[:], lnbt[:, rt, :], AL.add)

              # =========== FFN (hidden shard 1024) ===========
              # hT's S axis is PERMUTED: col = hf*1024 + c*128 + r maps to
              # true row c*256 + hf*128 + r. FFN1/FFN2 are elementwise along
              # S so only the final staging DMA needs to undo the mapping.
              with tc.tile_pool(name="ffn", bufs=1) as fp, nc.named_scope("ffn"):
                  hT = fp.tile([128, 16, S], BF16, name="hT")
                  zT = fp.tile([128, 8, S], BF16, name="zT")
                  w1full = fp.tile([128, 8, 2048], BF16, name="w1full")
                  for ft in range(8):
                      nc.sync.dma_start(w1full[:, ft, :], w1_d[:, ft, :])
                  with tc.tile_pool(name="wst", bufs=2) as wst:
                      for hf in range(2):
                          for c in range(NCORES):
                              nc.sync.dma_start(
                                  hT[:, :, hf * 1024 + c * 128 : hf * 1024 + (c + 1) * 128],
                                  agt_out[hf][c * 16 : (c + 1) * 16].rearrange("eo p r -> p eo r"),
                              )
                      # FFN1 per gathered S-half: starts on half 0 while
                      # half 1 is still gathering
                      for hf in range(2):
                          for ft in range(8):
                              pzs = [
                                  ps512.tile([128, 512], F32, name=f"pz{sb}", tag="p512")
                                  for sb in range(2)
                              ]
                              for eo in range(16):
                                  for sb in range(2):
                                      nc.tensor.matmul(
                                          pzs[sb][:],
                                          w1full[:, ft, eo * KD : (eo + 1) * KD],
                                          hT[:, eo, hf * 1024 + sb * 512 : hf * 1024 + (sb + 1) * 512],
                                          start=(eo == 0),
                                          stop=(eo == 15),
                                      )
                              for sb in range(2):
                                  nc.scalar.activation(
                                      zT[:, ft, hf * 1024 + sb * 512 : hf * 1024 + (sb + 1) * 512],
                                      pzs[sb][:],
                                      AF.Relu,
                                      bias=b1_sb[:, ft : ft + 1],
                                  )
                      for ebx, (c0, w) in enumerate(EBW):
                          w2b = wst.tile([128, 8, w], BF16, name="w2b", tag="w2b")
                          nc.sync.dma_start(w2b[:], w2_d[:, :, c0 // 512, :])
                          for p in range(16):
                              pf = ps512.tile([128, w], F32, name="pf", tag="p512")
                              for fc in range(8):
                                  nc.tensor.matmul(
                                      pf[:],
                                      zT[:, fc, p * 128 : (p + 1) * 128],
                                      w2b[:, fc, :],
                                      start=(fc == 0),
                                      stop=(fc == 7),
                                  )
                              fstg = wst.tile([128, w], F16, name="fstg", tag="fstg", bufs=4)
                              nc.scalar.activation(fstg[:], pf[:], AF.Copy)
                              hf, cc = p // 8, p % 8
                              r0 = cc * 256 + hf * 128
                              nc.sync.dma_start(ffn_in[ebx][r0 : r0 + 128, :], fstg[:])
                          nc.gpsimd.collective_compute(
                              "ReduceScatter",
                              AL.add,
                              replica_groups=RG,
                              ins=[ffn_in[ebx][:]],
                              outs=[ffn_out[ebx][:]],
                          )

              # =========== LN2 + output ===========
              with tc.tile_pool(name="ln2", bufs=1) as l2, nc.named_scope("ln2"):
                  ys2 = l2.tile([128, 2, E], F32, name="ys2")
                  yb2t = l2.tile([128, E], F32, name="yb2t")
                  nc.sync.dma_start(yb2t[:], yb2_d[:])
                  for rt in range(2):
                      nc.vector.tensor_tensor(
                          ys2[:, rt, :], h_own[:, rt, :], yb2t[:], AL.add
                      )
                  parts2 = l2.tile([128, 16], F32, name="parts2")
                  sqs2 = l2.tile([128, 512], BF16, name="sqs2", tag="sqs2", bufs=2)
                  for ebx, (c0, w) in enumerate(EBW):
                      for rt in range(2):
                          fot = l2.tile([128, w], F16, name="fot", tag="fot", bufs=4)
                          nc.sync.dma_start(fot[:], ffn_out[ebx][rt * 128 : (rt + 1) * 128, :])
                          dstv = ys2[:, rt, c0 : c0 + w]
                          nc.vector.tensor_tensor(dstv, dstv, fot[:], AL.add)
                          blk = ebx * 2 + rt
                          nc.vector.tensor_reduce(
                              parts2[:, blk : blk + 1], dstv, axis=AX.X, op=AL.add
                          )
                          nc.scalar.activation(
                              sqs2[:, 0:w], dstv, AF.Square,
                              accum_out=parts2[:, 8 + blk : 9 + blk],
                          )
                  pstat2 = psT.tile([128, 128], F32, name="pstat2", tag="pt", bufs=1)
                  nc.tensor.matmul(pstat2[:1, :16], onesc[:, 0:1], parts2[:], start=True, stop=True)
                  st4s2 = l2.tile([1, 16], F32, name="st4s2")
                  nc.vector.tensor_copy(out=st4s2[:], in_=pstat2[:1, :16])
                  st42 = l2.tile([1, 8], F32, name="st42")
                  nc.vector.memset(st42[:], 0.0)
                  nc.vector.tensor_reduce(st42[:, 0:1], st4s2[:, 0:8], axis=AX.X, op=AL.add)
                  nc.vector.tensor_reduce(st42[:, 1:2], st4s2[:, 8:16], axis=AX.X, op=AL.add)
                  nc.sync.dma_start(st2_in[:], st42[:])
                  nc.gpsimd.collective_compute(
                      "AllReduce", AL.add, replica_groups=RG, ins=[st2_in[:]], outs=[st2_out[:]]
                  )
                  bc2 = _ln_scalars(nc, l2, psT, onesr, st2_out)
                  for rt in range(2):
                      ot = l2.tile([128, E], F32, name="ot", tag="ot", bufs=2)
                      nc.scalar.activation(
                          ot[:],
                          ys2[:, rt, :],
                          AF.Identity,
                          bias=bc2[:, 0:1],
                          scale=bc2[:, 1:2],
                      )
                      nc.vector.tensor_tensor(ot[:], ot[:], lngt[:, rt, :], AL.mult)
                      nc.vector.tensor_tensor(ot[:], ot[:], lnbt[:, rt, :], AL.add)
                      nc.sync.dma_start(out_d[rt * 128 : (rt + 1) * 128, :], ot[:])

    nc.compile()
    return nc


def _ln_scalars(nc, pool, psT, onesr, st_out):
    """AllReduced (sum, sumsq) -> bc [128, 2] = (-m*rstd, rstd) broadcast."""
    so = pool.tile([1, 8], F32, name="so", tag="so")
    nc.sync.dma_start(so[:], st_out[:])
    sc = pool.tile([1, 8], F32, name="sc", tag="sc")
    # sc0 = m, sc1 = E[y^2], sc2 = m^2, sc3 = var, sc4 = rstd, sc5 = -m*rstd
    nc.scalar.mul(sc[:, 0:1], so[:, 0:1], 1.0 / NTOT)
    nc.scalar.mul(sc[:, 1:2], so[:, 1:2], 1.0 / NTOT)
    nc.scalar.activation(sc[:, 2:3], sc[:, 0:1], AF.Square)
    nc.vector.tensor_tensor(sc[:, 3:4], sc[:, 1:2], sc[:, 2:3], AL.subtract)
    nc.vector.tensor_scalar_add(sc[:, 2:3], sc[:, 3:4], EPS)  # var + eps
    # rstd = 1/sqrt(var+eps): Sqrt on ACT (sqrt table also carries square/
    # identity/copy, so no further reloads), reciprocal on DVE (table-free)
    nc.scalar.activation(sc[:, 6:7], sc[:, 2:3], AF.Sqrt)
    nc.vector.reciprocal(sc[:, 4:5], sc[:, 6:7])
    nc.vector.tensor_tensor(sc[:, 7:8], sc[:, 0:1], sc[:, 4:5], AL.mult)
    nc.scalar.mul(sc[:, 5:6], sc[:, 7:8], -1.0)
    s2 = pool.tile([1, 2], F32, name="s2", tag="s2")
    nc.vector.tensor_copy(out=s2[:, 0:1], in_=sc[:, 5:6])
    nc.vector.tensor_copy(out=s2[:, 1:2], in_=sc[:, 4:5])
    pb = psT.tile([128, 128], F32, name="pb", tag="pt", bufs=1)
    nc.tensor.matmul(pb[:, :2], onesr[:], s2[:], start=True, stop=True)
    bc = pool.tile([128, 2], F32, name="bc", tag="bc")
    nc.vector.tensor_copy(out=bc[:], in_=pb[:, :2])
    return bc


_NC_CACHE = None


def _get_nc():
    global _NC_CACHE
    if _NC_CACHE is None:
        _NC_CACHE = _build()
    return _NC_CACHE


def _prep_core(c, inputs):
    f32 = np.float32
    x = np.ascontiguousarray(inputs["input"], dtype=f32)
    Wq, Wk, Wv = inputs["Wq"], inputs["Wk"], inputs["Wv"]
    bq, bk, bv = inputs["bq"], inputs["bk"], inputs["bv"]
    W1, b1, W2, b2 = inputs["W1"], inputs["b1"], inputs["W2"], inputs["b2"]
    ln_g, ln_b = inputs["ln_g"], inputs["ln_b"]
    h0 = c * HPC
    wqt = np.ascontiguousarray(
        np.stack(
            [Wq[h0 + h].reshape(16, 128, KD).transpose(1, 0, 2) for h in range(HPC)]
        ).astype(_bf16)
    )
    wkt = np.ascontiguousarray(
        np.stack(
            [Wk[h0 + h].reshape(16, 128, KD).transpose(1, 0, 2) for h in range(HPC)]
        ).astype(_bf16)
    )
    wvt = np.ascontiguousarray(
        np.stack(
            [Wv[h0 + h].reshape(16, 128, E).transpose(1, 0, 2) for h in range(HPC)]
        ).astype(_f8)
    )
    W1s = W1[:, c * FSH : (c + 1) * FSH]
    w1t = np.ascontiguousarray(
        W1s.reshape(16, 128, 8, 128).transpose(1, 2, 0, 3).reshape(128, 8, 2048).astype(_bf16)
    )
    W2s = W2[c * FSH : (c + 1) * FSH, :]
    w2t = np.ascontiguousarray(
        W2s.reshape(8, 128, 4, 512).transpose(1, 0, 2, 3).astype(_bf16)
    )
    bqs = np.ascontiguousarray((bq[h0 : h0 + HPC] * ISCALE).T, dtype=f32)
    bks = np.ascontiguousarray(bk[h0 : h0 + HPC].T, dtype=f32)
    b1s = np.ascontiguousarray(b1[c * FSH : (c + 1) * FSH].reshape(8, 128).T, dtype=f32)
    yb1 = np.ascontiguousarray(np.broadcast_to(bv.sum(axis=0), (128, E)), dtype=f32)
    yb2 = np.ascontiguousarray(np.broadcast_to(b2, (128, E)), dtype=f32)
    rows = slice(c * RROWS, (c + 1) * RROWS)
    jj, tp, sf = np.meshgrid(
        np.arange(4), np.arange(128), np.arange(512), indexing="ij"
    )
    mask = (((128 * jj + tp) <= sf).astype(f32) - 1.0) * 30.0
    xt = x.T.reshape(16, 128, S).transpose(1, 0, 2)
    xtb = np.ascontiguousarray(xt.astype(_bf16))
    xt8 = np.ascontiguousarray(xt.astype(_f8))
    return {
        "xtb": xtb,
        "xt8": xt8,
        "wqt": wqt,
        "wkt": wkt,
        "wvt": wvt,
        "w1t": w1t,
        "w2t": w2t,
        "bqs": bqs,
        "bks": bks,
        "b1s": b1s,
        "yb1": yb1,
        "yb2": yb2,
        "xr": np.ascontiguousarray(x[rows], dtype=f32),
        "lngr": np.ascontiguousarray(np.asarray(ln_g[rows], dtype=f32).astype(_bf16)),
        "lnbr": np.ascontiguousarray(np.asarray(ln_b[rows], dtype=f32).astype(_bf16)),
        "lngT": np.ascontiguousarray(
            np.asarray(ln_g[rows], dtype=f32).T.reshape(16, 128, RROWS).astype(_bf16)
        ),
        "lnbT": np.ascontiguousarray(
            np.asarray(ln_b[rows], dtype=f32).T.reshape(16, 128, RROWS).astype(_bf16)
        ),
        "mask": np.ascontiguousarray(mask),
        "ident": np.eye(128, dtype=f32),
        "ones": np.ones((128, 8), dtype=f32),
        "onesr": np.ones((1, 128), dtype=f32),
    }


def kernel(**inputs):
    nc = _get_nc()
    inputs = {k: np.asarray(v, dtype=np.float32) for k, v in inputs.items()}
    in_maps = [_prep_core(c, inputs) for c in range(NCORES)]
    res = run_bass_kernel_spmd(nc, in_maps, core_ids=list(range(NCORES)))
    out = np.concatenate([res.results[c]["out"] for c in range(NCORES)], axis=0)
    return np.ascontiguousarray(out, dtype=np.float32)



# revision 46
# speedup vs baseline: 1.4698x; 1.0121x over previous
"""Tensor-parallel decoder layer on 8 TRN2 NeuronCores.

Sharding:
  - Attention: 16 heads -> 2 per core. Per-core partial attn_out is
    ReduceScattered (fp16) so core c owns rows [256c, 256c+256).
  - Global LayerNorm (scalar mean/var over the whole [S,E] tensor):
    per-core partial (sum, sumsq) AllReduced as a tiny fp32 tensor.
  - FFN: hidden dim 8192 -> 1024 per core; partial [S,E] output
    ReduceScattered per 512-column chunk (fp16), overlapping FFN2.
  - h is AllGathered transposed (bf16) since every matmul contracting
    over E needs h^T as the moving operand.

Matmul layout notes (PE computes out = lhsT.T @ rhs, contraction on the
partition dim):
  - x^T resident in SBUF (bf16 for Q/K, fp8e4 for V) feeds projections.
  - The value path runs in fp8e4 DoubleRow (2 k-tiles per instruction,
    2x MAC rate): V projection contracts eo pairs, attn@v contracts
    t-tile pairs. exp is shifted by a constant (exp(s - SH)) so scores
    fit fp8's +/-240 range; the shift cancels exactly in num/rowsum.
  - scores are built transposed: S^T[t,s] tiles, so exp(S^T) tiles feed
    attn@v directly as lhsT with no transposes; softmax normalization is
    deferred: rowsum via a fp8 ones-column matmul, applied as a
    per-partition scale on the PSUM->SBUF copy of attn@v output.
  - causal masking folds into the score: a -30 additive mask lands on
    the PSUM tile before the exp, so masked entries underflow to 0.
"""

import math
import sys

sys.path.insert(0, "/opt/trn_rl_repo")

import numpy as np
import ml_dtypes

_bf16 = ml_dtypes.bfloat16
_f8 = ml_dtypes.float8_e4m3

import concourse.bass as bass
import concourse.mybir as mybir
import concourse.tile as tile
from concourse import bacc
from concourse.bass_utils import run_bass_kernel_spmd

S, E, H, KD, FF = 2048, 2048, 16, 128, 8192
EPS = 1e-5
NCORES = 8
HPC = H // NCORES          # heads per core = 2
FSH = FF // NCORES         # ffn hidden shard = 1024
RROWS = S // NCORES        # row shard = 256
NTOT = float(S * E)
ISCALE = 1.0 / math.sqrt(KD)

F32 = mybir.dt.float32
BF16 = mybir.dt.bfloat16
F16 = mybir.dt.float16
FP8 = mybir.dt.float8e4
AF = mybir.ActivationFunctionType
AL = mybir.AluOpType
AX = mybir.AxisListType
DR = mybir.MatmulPerfMode.DoubleRow

# packed triangular offsets for eT tiles: tile(tc, sb) at TRI[sb] + tc
TRI = [0, 4, 12, 24]
NTRI = 40
SH = 2.5  # constant exp shift: keeps exp(score - SH) within fp8e4 range


def _build():
    nc = bacc.Bacc(
        "TRN2",
        target_bir_lowering=False,
        debug=False,
        enable_asserts=True,
        num_devices=NCORES,
    )

    # ---- external I/O (per-core shards prepared on the host) ----
    xtb_d = nc.dram_tensor("xtb", [128, 16, S], BF16, kind="ExternalInput")
    xt8_d = nc.dram_tensor("xt8", [128, 16, S], FP8, kind="ExternalInput")
    wq_d = nc.dram_tensor("wqt", [HPC, 128, 16, KD], BF16, kind="ExternalInput")
    wk_d = nc.dram_tensor("wkt", [HPC, 128, 16, KD], BF16, kind="ExternalInput")
    wv_d = nc.dram_tensor("wvt", [HPC, 128, 16, E], FP8, kind="ExternalInput")
    w1_d = nc.dram_tensor("w1t", [128, 8, 2048], BF16, kind="ExternalInput")
    w2_d = nc.dram_tensor("w2t", [128, 8, 4, 512], BF16, kind="ExternalInput")
    bq_d = nc.dram_tensor("bqs", [128, HPC], F32, kind="ExternalInput")
    bk_d = nc.dram_tensor("bks", [128, HPC], F32, kind="ExternalInput")
    b1_d = nc.dram_tensor("b1s", [128, 8], F32, kind="ExternalInput")
    yb1_d = nc.dram_tensor("yb1", [128, E], F32, kind="ExternalInput")
    yb2_d = nc.dram_tensor("yb2", [128, E], F32, kind="ExternalInput")
    xr_d = nc.dram_tensor("xr", [RROWS, E], F32, kind="ExternalInput")
    lng_d = nc.dram_tensor("lngr", [RROWS, E], BF16, kind="ExternalInput")
    lnb_d = nc.dram_tensor("lnbr", [RROWS, E], BF16, kind="ExternalInput")
    lngT_d = nc.dram_tensor("lngT", [16, 128, RROWS], BF16, kind="ExternalInput")
    lnbT_d = nc.dram_tensor("lnbT", [16, 128, RROWS], BF16, kind="ExternalInput")
    mask_d = nc.dram_tensor("mask", [4, 128, 512], F32, kind="ExternalInput")
    id_d = nc.dram_tensor("ident", [128, 128], F32, kind="ExternalInput")
    ones_d = nc.dram_tensor("ones", [128, 8], F32, kind="ExternalInput")
    onesr_d = nc.dram_tensor("onesr", [1, 128], F32, kind="ExternalInput")
    out_d = nc.dram_tensor("out", [RROWS, E], F32, kind="ExternalOutput")

    RG = [list(range(NCORES))]

    with tile.TileContext(nc) as tc:
        with (
            tc.tile_pool(name="persist", bufs=1) as pp,
            tc.tile_pool(name="dram", bufs=1, space="DRAM") as dp,
            tc.tile_pool(name="ps512", bufs=5, space="PSUM") as ps512,
            tc.tile_pool(name="psT", bufs=1, space="PSUM") as psT,
            tc.tile_pool(name="psR", bufs=1, space="PSUM") as psR,
        ):
            # ---- collective bounce buffers (internal DRAM) ----
            att_in = [
                [
                    dp.tile([S, 512], F16, name=f"att_in_{fh}_{fb}", tag=f"ati{fh}{fb}")
                    for fb in range(2)
                ]
                for fh in range(2)
            ]
            att_out = [
                [
                    dp.tile([RROWS, 512], F16, name=f"att_out_{fh}_{fb}", tag=f"ato{fh}{fb}")
                    for fb in range(2)
                ]
                for fh in range(2)
            ]

            st1_in = dp.tile([1, 8], F32, name="st1_in", tag="st1i")
            st1_out = dp.tile([1, 8], F32, name="st1_out", tag="st1o", addr_space="Shared")
            st2_in = dp.tile([1, 8], F32, name="st2_in", tag="st2i")
            st2_out = dp.tile([1, 8], F32, name="st2_out", tag="st2o", addr_space="Shared")
            # transposed h AllGather, split by own-row halves so FFN1 can
            # start on the first half while the second gathers
            agt_in = [
                dp.tile([16, 128, 128], BF16, name=f"agt_in{j}", tag=f"agti{j}")
                for j in range(2)
            ]
            agt_out = [
                dp.tile([128, 128, 128], BF16, name=f"agt_out{j}", tag=f"agto{j}", addr_space="Shared")
                for j in range(2)
            ]
            # ffn output chunks: col offset/width
            EBW = [(0, 512), (512, 512), (1024, 512), (1536, 512)]
            ffn_in = [
                dp.tile([S, w], F16, name=f"ffn_in_{eb}", tag=f"ffi{eb}")
                for eb, (c0, w) in enumerate(EBW)
            ]
            ffn_out = [
                dp.tile([RROWS, w], F16, name=f"ffn_out_{eb}", tag=f"ffo{eb}")
                for eb, (c0, w) in enumerate(EBW)
            ]

            # ---- persistent small tiles ----
            ident = pp.tile([128, 128], F32, name="ident")
            nc.sync.dma_start(ident[:], id_d[:])
            hidb = pp.tile([128, 128], BF16, name="hidb")
            nc.vector.tensor_copy(out=hidb[:], in_=ident[:])
            onesc = pp.tile([128, 8], F32, name="onesc")
            nc.sync.dma_start(onesc[:], ones_d[:])
            onesr = pp.tile([1, 128], F32, name="onesr")
            nc.sync.dma_start(onesr[:], onesr_d[:])
            bq_sb = pp.tile([128, HPC], F32, name="bq_sb")
            nc.sync.dma_start(bq_sb[:], bq_d[:])
            bk_sb = pp.tile([128, HPC], F32, name="bk_sb")
            nc.sync.dma_start(bk_sb[:], bk_d[:])
            b1_sb = pp.tile([128, 8], F32, name="b1_sb")
            nc.sync.dma_start(b1_sb[:], b1_d[:])
            qkT = pp.tile([128, 2, HPC, S], BF16, name="qkT")  # [d, q/k, head, s]
            recips = pp.tile([128, HPC, 16], F32, name="recips")

            # =========== phase 0 + attention ===========
            with tc.tile_pool(name="attn", bufs=1) as ap_:
                xT8 = ap_.tile([128, 16, S], FP8, name="xT8")  # x^T fp8 for V
                maskf = ap_.tile([128, 4, 512], F32, name="maskf")  # (mask-1)*30
                shm = ap_.tile([128, 1], F32, name="shm")  # exp shift bias
                ones8 = ap_.tile([128, 2, 1], FP8, name="ones8")
                with tc.tile_pool(name="qkx", bufs=1) as qkx:
                    xT = qkx.tile([128, 16, S], BF16, name="xT")  # x^T bf16 for Q/K
                    wqk = [
                        [
                            qkx.tile([128, 16, KD], BF16, name=f"wqk{h}{qi}")
                            for qi in range(2)
                        ]
                        for h in range(HPC)
                    ]
                    with tc.tile_pool(name="prep", bufs=1) as prep, nc.named_scope("prep"):
                        # HAM warmup: back-to-back matmuls to unthrottle PE,
                        # issued while the first weight/x DMAs land
                        wtile = prep.tile([128, 512], BF16, name="wtile", tag="wtile", bufs=1)
                        nc.vector.memset(wtile[:], 0.0)
                        nc.vector.memset(shm[:], -SH)
                        nc.vector.memset(ones8[:], 1.0)
                        # interleave the first x^T chunks with the small
                        # weight DMAs so qkproj's first matmul gates on
                        # neither the bulk x transfer nor a queued weight
                        qkdmas = [
                            (wqk[h][qi], (wq_d if qi == 0 else wk_d)[h])
                            for h in range(HPC)
                            for qi in range(2)
                        ]
                        for eo in range(4):
                            nc.sync.dma_start(xT[:, eo, :], xtb_d[:, eo, :])
                            dst, src = qkdmas[eo]
                            nc.sync.dma_start(dst[:], src)
                        for _w in range(24):
                            pw = ps512.tile([128, 512], F32, name="pw", tag="p512")
                            nc.tensor.matmul(pw[:], wtile[:, :128], wtile[:], start=True, stop=True)
                        for eo in range(4, 16):
                            nc.sync.dma_start(xT[:, eo, :], xtb_d[:, eo, :])
                        nc.sync.dma_start(maskf[:], mask_d.ap().rearrange("j p s -> p j s"))
                        for eo in range(16):
                            nc.sync.dma_start(xT8[:, eo, :], xt8_d[:, eo, :])

                    # ---- Q/K projections for both heads (scaled/biased) ----
                    with nc.named_scope("qkproj"):
                        for h in range(HPC):
                            for qi, (b_sb, scl) in enumerate(
                                ((bq_sb, ISCALE), (bk_sb, 1.0))
                            ):
                                wb = wqk[h][qi]
                                pqs = [
                                    ps512.tile([128, 512], F32, name=f"pq{sb}", tag="p512")
                                    for sb in range(4)
                                ]
                                for eo in range(16):
                                    for sb in range(4):
                                        nc.tensor.matmul(
                                            pqs[sb][:],
                                            wb[:, eo, :],
                                            xT[:, eo, sb * 512 : (sb + 1) * 512],
                                            start=(eo == 0),
                                            stop=(eo == 15),
                                        )
                                for sb in range(4):
                                    nc.scalar.activation(
                                        qkT[:, qi, h, sb * 512 : (sb + 1) * 512],
                                        pqs[sb][:],
                                        AF.Identity,
                                        bias=b_sb[:, h : h + 1],
                                        scale=scl,
                                    )

                # ---- per-head attention (value path in fp8 DoubleRow) ----
                eT = ap_.tile([128, NTRI, 512], FP8, name="eT")
                v_sb = ap_.tile([128, 16, FSH], FP8, name="v_sb")
                with (
                    tc.tile_pool(name="acc", bufs=1) as accp,
                    tc.tile_pool(name="wvb", bufs=2) as wvbp,
                    tc.tile_pool(name="astg", bufs=4) as astg,
                ):
                    # both local heads sum on-chip here before the
                    # collective, halving attention ReduceScatter traffic
                    att_acc = [
                        accp.tile([128, 16, FSH], F16, name=f"att_acc{fh}")
                        for fh in range(2)
                    ]
                    for h in range(HPC):
                      with nc.named_scope(f"scores{h}"):
                        for sb in range(4):
                            for tcn in range(4 * sb + 4):
                                psc = ps512.tile([128, 512], F32, name="psc", tag="p512")
                                nc.tensor.matmul(
                                    psc[:],
                                    qkT[:, 1, h, tcn * 128 : (tcn + 1) * 128],
                                    qkT[:, 0, h, sb * 512 : (sb + 1) * 512],
                                    start=True,
                                    stop=True,
                                )
                                if tcn >= 4 * sb:
                                    # diagonal tile: -30 mask onto PSUM, then
                                    # masked entries underflow to 0 in the exp
                                    nc.vector.tensor_tensor(
                                        psc[:], psc[:], maskf[:, tcn - 4 * sb, :], AL.add
                                    )
                                nc.scalar.activation(
                                    eT[:, TRI[sb] + tcn, :], psc[:], AF.Exp, bias=shm[:]
                                )

                        # pass B: per 512-col strip: v-projection for the
                        # strip, then its attn@v immediately, so each
                        # strip's ReduceScatter starts as early as possible
                        for fh in range(2):
                            finalh = h == HPC - 1
                            for fb in range(2):
                              with nc.named_scope(f"vproj{h}{fh}{fb}"):
                                wvb = wvbp.tile([128, 16, 512], FP8, name="wvb", tag="wvb")
                                nc.sync.dma_start(
                                    wvb[:],
                                    wv_d[h][
                                        :, :, fh * 1024 + fb * 512 : fh * 1024 + (fb + 1) * 512
                                    ],
                                )
                                for tcn in range(16):
                                    pv = ps512.tile([128, 512], F32, name="pv", tag="p512")
                                    for j in range(8):
                                        nc.tensor.matmul(
                                            pv[:],
                                            xT8[:, 2 * j : 2 * j + 2, tcn * 128 : (tcn + 1) * 128],
                                            wvb[:, 2 * j : 2 * j + 2, :],
                                            start=(j == 0),
                                            stop=(j == 7),
                                            perf_mode=DR,
                                        )
                                    if tcn % 2 == 0:
                                        nc.vector.tensor_copy(
                                            out=v_sb[:, tcn, fb * 512 : (fb + 1) * 512],
                                            in_=pv[:],
                                        )
                                    else:
                                        nc.scalar.copy(
                                            v_sb[:, tcn, fb * 512 : (fb + 1) * 512], pv[:]
                                        )

                              with nc.named_scope(f"attnv{h}{fh}{fb}"):
                                dorow = fh == 0 and fb == 0
                                for i in range(15, -1, -1):
                                    sb, so = i // 4, (i % 4) * 128
                                    npair = (i + 1) // 2
                                    odd = (i + 1) % 2
                                    pa = ps512.tile([128, 512], F32, name="pa", tag="p512")
                                    if dorow:
                                        pr = psR.tile([128, 1], F32, name="pr", tag="pr", bufs=1)
                                    for tp in range(npair):
                                        lhs = eT[:, TRI[sb] + 2 * tp : TRI[sb] + 2 * tp + 2, so : so + 128]
                                        last = (tp == npair - 1) and not odd
                                        nc.tensor.matmul(
                                            pa[:],
                                            lhs,
                                            v_sb[:, 2 * tp : 2 * tp + 2, fb * 512 : (fb + 1) * 512],
                                            start=(tp == 0),
                                            stop=last,
                                            perf_mode=DR,
                                        )
                                        if dorow:
                                            nc.tensor.matmul(
                                                pr[:],
                                                lhs,
                                                ones8[:],
                                                start=(tp == 0),
                                                stop=last,
                                                perf_mode=DR,
                                            )
                                    if odd:
                                        lhs1 = eT[:, TRI[sb] + i, so : so + 128]
                                        nc.tensor.matmul(
                                            pa[:],
                                            lhs1,
                                            v_sb[:, i, fb * 512 : (fb + 1) * 512],
                                            start=(npair == 0),
                                            stop=True,
                                        )
                                        if dorow:
                                            nc.tensor.matmul(
                                                pr[:],
                                                lhs1,
                                                ones8[:, 0, :],
                                                start=(npair == 0),
                                                stop=True,
                                            )
                                    if dorow:
                                        rsf = astg.tile([128, 1], F32, name="rsf", tag="rsf")
                                        nc.vector.tensor_copy(out=rsf[:], in_=pr[:])
                                        nc.vector.reciprocal(recips[:, h, i : i + 1], rsf[:])
                                    dst_acc = att_acc[fh][:, i, fb * 512 : (fb + 1) * 512]
                                    if h == 0:
                                        # first head: scaled output straight
                                        # into the on-chip accumulator
                                        nc.scalar.activation(
                                            dst_acc, pa[:], AF.Copy,
                                            scale=recips[:, h, i : i + 1],
                                        )
                                    else:
                                        stg = astg.tile([128, 512], F16, name="stg", tag="stg", bufs=8)
                                        nc.scalar.activation(
                                            stg[:], pa[:], AF.Copy,
                                            scale=recips[:, h, i : i + 1],
                                        )
                                        nc.vector.tensor_tensor(dst_acc, dst_acc, stg[:], AL.add)
                                    if finalh:
                                        nc.sync.dma_start(
                                            att_in[fh][fb][i * 128 : (i + 1) * 128, :],
                                            dst_acc,
                                        )
                                if finalh:
                                    nc.gpsimd.collective_compute(
                                        "ReduceScatter",
                                        AL.add,
                                        replica_groups=RG,
                                        ins=[att_in[fh][fb][:]],
                                        outs=[att_out[fh][fb][:]],
                                    )

            # =========== LN1 (global mean/var) ===========
            with tc.tile_pool(name="mid", bufs=1) as midp:
              h_own = midp.tile([128, 2, E], BF16, name="h_own")
              ysT = midp.tile([128, 16, RROWS], BF16, name="ysT")
              lngt = midp.tile([128, 2, E], BF16, name="lngt")
              lnbt = midp.tile([128, 2, E], BF16, name="lnbt")
              with tc.tile_pool(name="ln1", bufs=1) as lp, nc.named_scope("ln1"):
                  ys = lp.tile([128, 2, E], F32, name="ys")
                  ysb = lp.tile([128, 2, E], BF16, name="ysb")
                  yb1t = lp.tile([128, E], F32, name="yb1t")
                  nc.sync.dma_start(yb1t[:], yb1_d[:])
                  nc.sync.dma_start(lngt[:], lng_d.ap().rearrange("(t p) e -> p t e", p=128))
                  nc.sync.dma_start(lnbt[:], lnb_d.ap().rearrange("(t p) e -> p t e", p=128))
                  lngT = lp.tile([128, 16, RROWS], BF16, name="lngT")
                  nc.sync.dma_start(lngT[:], lngT_d.ap().rearrange("eo p r -> p eo r"))
                  lnbT = lp.tile([128, 16, RROWS], BF16, name="lnbT")
                  nc.sync.dma_start(lnbT[:], lnbT_d.ap().rearrange("eo p r -> p eo r"))
                  for rt in range(2):
                      xrt = lp.tile([128, E], F32, name="xrt", tag="xrt", bufs=2)
                      nc.sync.dma_start(xrt[:], xr_d[rt * 128 : (rt + 1) * 128, :])
                      nc.vector.tensor_tensor(ys[:, rt, :], xrt[:], yb1t[:], AL.add)
                  # accumulate RS chunks as they land; the last head's chunk
                  # completes a column block -> emit its stats partial
                  parts = lp.tile([128, 16], F32, name="parts")
                  sqs = lp.tile([128, 512], BF16, name="sqs", tag="sqs", bufs=2)
                  for fh in range(2):
                      for fb in range(2):
                          for rt in range(2):
                              rof = lp.tile([128, 512], F16, name="rof", tag="rof", bufs=4)
                              nc.sync.dma_start(
                                  rof[:], att_out[fh][fb][rt * 128 : (rt + 1) * 128, :]
                              )
                              col = fh * 1024 + fb * 512
                              dstv = ys[:, rt, col : col + 512]
                              nc.vector.tensor_tensor(dstv, dstv, rof[:], AL.add)
                              blk = (fh * 2 + fb) * 2 + rt
                              nc.vector.tensor_reduce(
                                  parts[:, blk : blk + 1], dstv, axis=AX.X, op=AL.add
                              )
                              nc.scalar.activation(
                                  sqs[:], dstv, AF.Square,
                                  accum_out=parts[:, 8 + blk : 9 + blk],
                              )
                  pstat = psT.tile([128, 128], F32, name="pstat", tag="pt", bufs=1)
                  nc.tensor.matmul(pstat[:1, :16], onesc[:, 0:1], parts[:], start=True, stop=True)
                  st4s = lp.tile([1, 16], F32, name="st4s")
                  nc.vector.tensor_copy(out=st4s[:], in_=pstat[:1, :16])
                  st4 = lp.tile([1, 8], F32, name="st4")
                  nc.vector.memset(st4[:], 0.0)
                  nc.vector.tensor_reduce(st4[:, 0:1], st4s[:, 0:8], axis=AX.X, op=AL.add)
                  nc.vector.tensor_reduce(st4[:, 1:2], st4s[:, 8:16], axis=AX.X, op=AL.add)
                  nc.sync.dma_start(st1_in[:], st4[:])
                  nc.gpsimd.collective_compute(
                      "AllReduce", AL.add, replica_groups=RG, ins=[st1_in[:]], outs=[st1_out[:]]
                  )
                  # per own-row half: transpose (while the AllReduce flies),
                  # LN affine in transposed space, AllGather. bc is computed
                  # between the halves so AG half 0 launches before half 1's
                  # transposes even start.
                  bc = None
                  for rt in range(2):
                      nc.vector.tensor_copy(out=ysb[:, rt, :], in_=ys[:, rt, :])
                      for eo in range(16):
                          pth = psT.tile([128, 128], BF16, name="pth", tag="ptT")
                          nc.tensor.transpose(
                              pth[:], ysb[:, rt, eo * 128 : (eo + 1) * 128], hidb[:]
                          )
                          if eo % 2 == 0:
                              nc.vector.tensor_copy(
                                  out=ysT[:, eo, rt * 128 : (rt + 1) * 128], in_=pth[:]
                              )
                          else:
                              nc.scalar.copy(ysT[:, eo, rt * 128 : (rt + 1) * 128], pth[:])
                      if bc is None:
                          bc = _ln_scalars(nc, lp, psT, onesr, st1_out)
                      src = ysT[:, :, rt * 128 : (rt + 1) * 128]
                      stT = lp.tile([128, 16, 128], BF16, name="hstgT", tag="hstgT", bufs=2)
                      nc.scalar.activation(
                          stT[:], src, AF.Identity, bias=bc[:, 0:1], scale=bc[:, 1:2]
                      )
                      nc.vector.tensor_tensor(
                          stT[:], stT[:], lngT[:, :, rt * 128 : (rt + 1) * 128], AL.mult
                      )
                      nc.vector.tensor_tensor(
                          stT[:], stT[:], lnbT[:, :, rt * 128 : (rt + 1) * 128], AL.add
                      )
                      nc.sync.dma_start(
                          agt_in[rt].rearrange("eo p r -> p eo r"), stT[:]
                      )
                      nc.gpsimd.collective_compute(
                          "AllGather",
                          AL.bypass,
                          replica_groups=RG,
                          ins=[agt_in[rt][:]],
                          outs=[agt_out[rt][:]],
                      )
                  # row-major h_own for the LN2 residual (off critical path)
                  ht_f32 = lp.tile([128, E], F32, name="ht_f32", tag="htf", bufs=2)
                  for rt in range(2):
                      nc.scalar.activation(
                          ht_f32[:], ys[:, rt, :], AF.Identity,
                          bias=bc[:, 0:1], scale=bc[:, 1:2],
                      )
                      nc.vector.tensor_tensor(ht_f32[:], ht_f32[:], lngt[:, rt, :], AL.mult)
                      nc.vector.tensor_tensor(h_own[:, rt, :], ht_f32[:], lnbt[:, rt, :], AL.add)

              # =========== FFN (hidden shard 1024) ===========
              # hT's S axis is PERMUTED: col = hf*1024 + c*128 + r maps to
              # true row c*256 + hf*128 + r. FFN1/FFN2 are elementwise along
              # S so only the final staging DMA needs to undo the mapping.
              with tc.tile_pool(name="ffn", bufs=1) as fp, nc.named_scope("ffn"):
                  hT = fp.tile([128, 16, S], BF16, name="hT")
                  zT = fp.tile([128, 8, S], BF16, name="zT")
                  w1full = fp.tile([128, 8, 2048], BF16, name="w1full")
                  for ft in range(8):
                      nc.sync.dma_start(w1full[:, ft, :], w1_d[:, ft, :])
                  with tc.tile_pool(name="wst", bufs=2) as wst:
                      for hf in range(2):
                          for c in range(NCORES):
                              nc.sync.dma_start(
                                  hT[:, :, hf * 1024 + c * 128 : hf * 1024 + (c + 1) * 128],
                                  agt_out[hf][c * 16 : (c + 1) * 16].rearrange("eo p r -> p eo r"),
                              )
                      # FFN1 per gathered S-half: starts on half 0 while
                      # half 1 is still gathering
                      for hf in range(2):
                          for ft in range(8):
                              pzs = [
                                  ps512.tile([128, 512], F32, name=f"pz{sb}", tag="p512")
                                  for sb in range(2)
                              ]
                              for eo in range(16):
                                  for sb in range(2):
                                      nc.tensor.matmul(
                                          pzs[sb][:],
                                          w1full[:, ft, eo * KD : (eo + 1) * KD],
                                          hT[:, eo, hf * 1024 + sb * 512 : hf * 1024 + (sb + 1) * 512],
                                          start=(eo == 0),
                                          stop=(eo == 15),
                                      )
                              for sb in range(2):
                                  nc.scalar.activation(
                                      zT[:, ft, hf * 1024 + sb * 512 : hf * 1024 + (sb + 1) * 512],
                                      pzs[sb][:],
                                      AF.Relu,
                                      bias=b1_sb[:, ft : ft + 1],
                                  )
                      for ebx, (c0, w) in enumerate(EBW):
                          w2b = wst.tile([128, 8, w], BF16, name="w2b", tag="w2b")
                          nc.sync.dma_start(w2b[:], w2_d[:, :, c0 // 512, :])
                          for p in range(16):
                              pf = ps512.tile([128, w], F32, name="pf", tag="p512")
                              for fc in range(8):
                                  nc.tensor.matmul(
                                      pf[:],
                                      zT[:, fc, p * 128 : (p + 1) * 128],
                                      w2b[:, fc, :],
                                      start=(fc == 0),
                                      stop=(fc == 7),
                                  )
                              fstg = wst.tile([128, w], F16, name="fstg", tag="fstg", bufs=4)
                              nc.scalar.activation(fstg[:], pf[:], AF.Copy)
                              hf, cc = p // 8, p % 8
                              r0 = cc * 256 + hf * 128
                              nc.sync.dma_start(ffn_in[ebx][r0 : r0 + 128, :], fstg[:])
                          nc.gpsimd.collective_compute(
                              "ReduceScatter",
                              AL.add,
                              replica_groups=RG,
                              ins=[ffn_in[ebx][:]],
                              outs=[ffn_out[ebx][:]],
                          )

              # =========== LN2 + output ===========
              with tc.tile_pool(name="ln2", bufs=1) as l2, nc.named_scope("ln2"):
                  ys2 = l2.tile([128, 2, E], F32, name="ys2")
                  yb2t = l2.tile([128, E], F32, name="yb2t")
                  nc.sync.dma_start(yb2t[:], yb2_d[:])
                  for rt in range(2):
                      nc.vector.tensor_tensor(
                          ys2[:, rt, :], h_own[:, rt, :], yb2t[:], AL.add
                      )
                  parts2 = l2.tile([128, 16], F32, name="parts2")
                  sqs2 = l2.tile([128, 512], BF16, name="sqs2", tag="sqs2", bufs=2)
                  for ebx, (c0, w) in enumerate(EBW):
                      for rt in range(2):
                          fot = l2.tile([128, w], F16, name="fot", tag="fot", bufs=4)
                          nc.sync.dma_start(fot[:], ffn_out[ebx][rt * 128 : (rt + 1) * 128, :])
                          dstv = ys2[:, rt, c0 : c0 + w]
                          nc.vector.tensor_tensor(dstv, dstv, fot[:], AL.add)
                          blk = ebx * 2 + rt
                          nc.vector.tensor_reduce(
                              parts2[:, blk : blk + 1], dstv, axis=AX.X, op=AL.add
                          )
                          nc.scalar.activation(
                              sqs2[:, 0:w], dstv, AF.Square,
                              accum_out=parts2[:, 8 + blk : 9 + blk],
                          )
                  pstat2 = psT.tile([128, 128], F32, name="pstat2", tag="pt", bufs=1)
                  nc.tensor.matmul(pstat2[:1, :16], onesc[:, 0:1], parts2[:], start=True, stop=True)
                  st4s2 = l2.tile([1, 16], F32, name="st4s2")
                  nc.vector.tensor_copy(out=st4s2[:], in_=pstat2[:1, :16])
                  st42 = l2.tile([1, 8], F32, name="st42")
                  nc.vector.memset(st42[:], 0.0)
                  nc.vector.tensor_reduce(st42[:, 0:1], st4s2[:, 0:8], axis=AX.X, op=AL.add)
                  nc.vector.tensor_reduce(st42[:, 1:2], st4s2[:, 8:16], axis=AX.X, op=AL.add)
                  nc.sync.dma_start(st2_in[:], st42[:])
                  nc.gpsimd.collective_compute(
                      "AllReduce", AL.add, replica_groups=RG, ins=[st2_in[:]], outs=[st2_out[:]]
                  )
                  bc2 = _ln_scalars(nc, l2, psT, onesr, st2_out)
                  for rt in range(2):
                      ot = l2.tile([128, E], F32, name="ot", tag="ot", bufs=2)
                      nc.scalar.activation(
                          ot[:],
                          ys2[:, rt, :],
                          AF.Identity,
                          bias=bc2[:, 0:1],
                          scale=bc2[:, 1:2],
                      )
                      nc.vector.tensor_tensor(ot[:], ot[:], lngt[:, rt, :], AL.mult)
                      nc.vector.tensor_tensor(ot[:], ot[:], lnbt[:, rt, :], AL.add)
                      nc.sync.dma_start(out_d[rt * 128 : (rt + 1) * 128, :], ot[:])

    nc.compile()
    return nc


def _ln_scalars(nc, pool, psT, onesr, st_out):
    """AllReduced (sum, sumsq) -> bc [128, 2] = (-m*rstd, rstd) broadcast."""
    so = pool.tile([1, 8], F32, name="so", tag="so")
    nc.sync.dma_start(so[:], st_out[:])
    sc = pool.tile([1, 8], F32, name="sc", tag="sc")
    # sc0 = m, sc1 = E[y^2], sc2 = m^2, sc3 = var, sc4 = rstd, sc5 = -m*rstd
    nc.scalar.mul(sc[:, 0:1], so[:, 0:1], 1.0 / NTOT)
    nc.scalar.mul(sc[:, 1:2], so[:, 1:2], 1.0 / NTOT)
    nc.scalar.activation(sc[:, 2:3], sc[:, 0:1], AF.Square)
    nc.vector.tensor_tensor(sc[:, 3:4], sc[:, 1:2], sc[:, 2:3], AL.subtract)
    nc.vector.tensor_scalar_add(sc[:, 2:3], sc[:, 3:4], EPS)  # var + eps
    # rstd = 1/sqrt(var+eps): Sqrt on ACT (sqrt table also carries square/
    # identity/copy, so no further reloads), reciprocal on DVE (table-free)
    nc.scalar.activation(sc[:, 6:7], sc[:, 2:3], AF.Sqrt)
    nc.vector.reciprocal(sc[:, 4:5], sc[:, 6:7])
    nc.vector.tensor_tensor(sc[:, 7:8], sc[:, 0:1], sc[:, 4:5], AL.mult)
    nc.scalar.mul(sc[:, 5:6], sc[:, 7:8], -1.0)
    s2 = pool.tile([1, 2], F32, name="s2", tag="s2")
    nc.vector.tensor_copy(out=s2[:, 0:1], in_=sc[:, 5:6])
    nc.vector.tensor_copy(out=s2[:, 1:2], in_=sc[:, 4:5])
    pb = psT.tile([128, 128], F32, name="pb", tag="pt", bufs=1)
    nc.tensor.matmul(pb[:, :2], onesr[:], s2[:], start=True, stop=True)
    bc = pool.tile([128, 2], F32, name="bc", tag="bc")
    nc.vector.tensor_copy(out=bc[:], in_=pb[:, :2])
    return bc


_NC_CACHE = None


def _get_nc():
    global _NC_CACHE
    if _NC_CACHE is None:
        _NC_CACHE = _build()
    return _NC_CACHE


def _prep_core(c, inputs):
    f32 = np.float32
    x = np.ascontiguousarray(inputs["input"], dtype=f32)
    Wq, Wk, Wv = inputs["Wq"], inputs["Wk"], inputs["Wv"]
    bq, bk, bv = inputs["bq"], inputs["bk"], inputs["bv"]
    W1, b1, W2, b2 = inputs["W1"], inputs["b1"], inputs["W2"], inputs["b2"]
    ln_g, ln_b = inputs["ln_g"], inputs["ln_b"]
    h0 = c * HPC
    wqt = np.ascontiguousarray(
        np.stack(
            [Wq[h0 + h].reshape(16, 128, KD).transpose(1, 0, 2) for h in range(HPC)]
        ).astype(_bf16)
    )
    wkt = np.ascontiguousarray(
        np.stack(
            [Wk[h0 + h].reshape(16, 128, KD).transpose(1, 0, 2) for h in range(HPC)]
        ).astype(_bf16)
    )
    wvt = np.ascontiguousarray(
        np.stack(
            [Wv[h0 + h].reshape(16, 128, E).transpose(1, 0, 2) for h in range(HPC)]
        ).astype(_f8)
    )
    W1s = W1[:, c * FSH : (c + 1) * FSH]
    w1t = np.ascontiguousarray(
        W1s.reshape(16, 128, 8, 128).transpose(1, 2, 0, 3).reshape(128, 8, 2048).astype(_bf16)
    )
    W2s = W2[c * FSH : (c + 1) * FSH, :]
    w2t = np.ascontiguousarray(
        W2s.reshape(8, 128, 4, 512).transpose(1, 0, 2, 3).astype(_bf16)
    )
    bqs = np.ascontiguousarray((bq[h0 : h0 + HPC] * ISCALE).T, dtype=f32)
    bks = np.ascontiguousarray(bk[h0 : h0 + HPC].T, dtype=f32)
    b1s = np.ascontiguousarray(b1[c * FSH : (c + 1) * FSH].reshape(8, 128).T, dtype=f32)
    yb1 = np.ascontiguousarray(np.broadcast_to(bv.sum(axis=0), (128, E)), dtype=f32)
    yb2 = np.ascontiguousarray(np.broadcast_to(b2, (128, E)), dtype=f32)
    rows = slice(c * RROWS, (c + 1) * RROWS)
    jj, tp, sf = np.meshgrid(
        np.arange(4), np.arange(128), np.arange(512), indexing="ij"
    )
    mask = (((128 * jj + tp) <= sf).astype(f32) - 1.0) * 30.0
    xt = x.T.reshape(16, 128, S).transpose(1, 0, 2)
    xtb = np.ascontiguousarray(xt.astype(_bf16))
    xt8 = np.ascontiguousarray(xt.astype(_f8))
    return {
        "xtb": xtb,
        "xt8": xt8,
        "wqt": wqt,
        "wkt": wkt,
        "wvt": wvt,
        "w1t": w1t,
        "w2t": w2t,
        "bqs": bqs,
        "bks": bks,
        "b1s": b1s,
        "yb1": yb1,
        "yb2": yb2,
        "xr": np.ascontiguousarray(x[rows], dtype=f32),
        "lngr": np.ascontiguousarray(np.asarray(ln_g[rows], dtype=f32).astype(_bf16)),
        "lnbr": np.ascontiguousarray(np.asarray(ln_b[rows], dtype=f32).astype(_bf16)),
        "lngT": np.ascontiguousarray(
            np.asarray(ln_g[rows], dtype=f32).T.reshape(16, 128, RROWS).astype(_bf16)
        ),
        "lnbT": np.ascontiguousarray(
            np.asarray(ln_b[rows], dtype=f32).T.reshape(16, 128, RROWS).astype(_bf16)
        ),
        "mask": np.ascontiguousarray(mask),
        "ident": np.eye(128, dtype=f32),
        "ones": np.ones((128, 8), dtype=f32),
        "onesr": np.ones((1, 128), dtype=f32),
    }


def kernel(**inputs):
    nc = _get_nc()
    inputs = {k: np.asarray(v, dtype=np.float32) for k, v in inputs.items()}
    in_maps = [_prep_core(c, inputs) for c in range(NCORES)]
    res = run_bass_kernel_spmd(nc, in_maps, core_ids=list(range(NCORES)))
    out = np.concatenate([res.results[c]["out"] for c in range(NCORES)], axis=0)
    return np.ascontiguousarray(out, dtype=np.float32)

